# revision 4
# baseline (speedup 1.0000x reference)
"""GAT forward (2-layer graph attention, B=4 N=2048 F=128 H=8 D=64 C=32)
as a Bass/Tile SPMD kernel on 8 Trainium2 NeuronCores.

Sharding: core c -> (batch b=c//2, query-row half c%2).  Each core computes
attention for its 1024 query rows over all 2048 keys for all 8 heads
(layer 1) and for the output head (layer 2).  The only cross-core exchange
is a 2-rank AllGather of the layer-2 projections [g1|g2|Wh2] ([1024,34] f32)
within each (2b, 2b+1) pair.

Layout: attention logits are built TRANSPOSED, e^T[j (keys) = partitions,
i (queries) = free], so the PV matmul needs no operand transposes
(lhsT = Wh[j,d] stationary, rhs = p[j,i] moving, out = h^T[d,i]) and
softmax row sums come from a ones-column appended to Wh (PSUM row D).

The N^2 elementwise work (mask+f1+f2, leaky-relu, exp over 144 [128,1024]
logit tiles) is the bottleneck; everything runs in bf16 (2x DVE tensor-
tensor throughput; ACT is dtype-independent) and every logit PAIR of tiles
is routed down one of three pipelines to saturate ACT+DVE+GPSIMD jointly:

  P3 : u = mb + f1rep       (DVE tensor_tensor, bf16 2x)
       z = Prelu(u + f2col) (ACT, per-sub bias)     p = Exp(z)  (ACT)
  P4 : same but the combine runs on GPSIMD tensor_add
  P7 : exp(prelu(u)) == max(exp(u), exp(0.2u)) and exp(u) factors rank-1:
       q1 = (mb01 * exp(f2_j)) * exp(f1_i)   (DVE STT mult,mult)
       q2 = (mb01 * exp(.2 f2)) * exp(.2 f1) (DVE STT)
       p  = max(q1, q2)                      (GPSIMD tensor_max; no ACT!)

Softmax division is deferred to the per-head-pair epilogue (1/S via Ln/Exp
of the row-sum), fused with ELU via elu(v)+1 = relu(v) + exp(min(v,0)),
the +1 folded into a rank-1 correction matmul of the layer-2 projection.
"""

import numpy as np
import ml_dtypes

import concourse.bass as bass
import concourse.tile as tile
from concourse import mybir
from concourse.bass_utils import run_bass_kernel_spmd

F32 = mybir.dt.float32
BF16 = mybir.dt.bfloat16

B, N, F, H, D, C = 4, 2048, 128, 8, 64, 32
I = N // 2          # query rows per core
JT = N // 128       # key tiles
IC = I // 128       # query-row 128-chunks per core
KT = (H * D) // 128 # hidden-dim 128-chunks
ALPHA = 0.2
BIG = 512.0         # mask bias; exp(lrelu(-BIG+eps)) underflows to 0
N_CORES = 8
REPLICA_GROUPS = [[0, 1], [2, 3], [4, 5], [6, 7]]

ADD = mybir.AluOpType.add
MAX = mybir.AluOpType.max
MULT = mybir.AluOpType.mult
# NOTE: hardware "Lrelu" has a fixed 0.01 slope and ignores alpha;
# "Prelu" honors alpha (verified on HW) — it is the configurable leaky relu.
ACT_LRELU = mybir.ActivationFunctionType.Prelu
ACT_EXP = mybir.ActivationFunctionType.Exp
ACT_LN = mybir.ActivationFunctionType.Ln
ACT_COPY = mybir.ActivationFunctionType.Copy

N_HEADS_ALL = H + 1          # 8 layer-1 heads + the layer-2 output head
PAIRS_PER_HEAD = JT // 2     # 8
N_PAIRS = N_HEADS_ALL * PAIRS_PER_HEAD  # 72


def _split_multiwaits(nc):
    """Pinned walrus accepts only one sync-wait per instruction; Tile's exit
    drain (and occasionally others) carries several.  Hoist extras onto
    single-wait Drains on the same engine immediately before the owner."""
    n_fixed = 0
    for fn in nc.m.functions:
        for bb in fn.blocks:
            for name in [i.name for i in bb.instructions]:
                idx = [i.name for i in bb.instructions].index(name)
                inst = bb.instructions[idx]
                si = inst.sync_info
                if si is None or len(si.on_wait) <= 1:
                    continue
                waits = list(si.on_wait)
                for k, w in enumerate(waits[:-1]):
                    nd = mybir.InstDrain(
                        name=f"waitfix-{inst.name}-{k}", ins=[], outs=[])
                    nd.engine = inst.engine
                    nd.sync_info = mybir.SyncInfo(on_wait=[w], on_update=[])
                    nc.register_instruction(nd, overwrite=True)
                    bb.instructions.insert(idx + k, nd)
                inst.sync_info = mybir.SyncInfo(
                    on_wait=waits[-1:], on_update=list(si.on_update))
                n_fixed += 1
    return n_fixed


def _make_route(cfg):
    """Per-pair pipeline assignment: 3 (DVE+ACT), 4 (GPS+ACT), 7 (factored,
    DVE+GPS, no ACT).  Counts spread evenly across the 9 'heads' (8 L1 + L2)
    and interleaved within each head so all engines stay concurrently busy."""
    n4 = int(cfg.get("p4", 24))
    n7 = int(cfg.get("p7", 33))
    n3 = N_PAIRS - n4 - n7
    assert n3 >= 0
    # distribute counts per head proportionally
    per_head = [[0, 0, 0] for _ in range(N_HEADS_ALL)]  # [n3, n4, n7]
    for idx, cnt in enumerate((n3, n4, n7)):
        base, rem = divmod(cnt, N_HEADS_ALL)
        for hh in range(N_HEADS_ALL):
            per_head[hh][idx] = base + (1 if hh < rem else 0)
    # fix overflow per head (must sum to PAIRS_PER_HEAD)
    for hh in range(N_HEADS_ALL):
        while sum(per_head[hh]) > PAIRS_PER_HEAD:
            per_head[hh][int(np.argmax(per_head[hh]))] -= 1
        while sum(per_head[hh]) < PAIRS_PER_HEAD:
            per_head[hh][int(np.argmin(per_head[hh]))] += 1
    route = []
    for hh in range(N_HEADS_ALL):
        c3, c4, c7 = per_head[hh]
        seq = []
        # interleave: alternate 7s with 3/4s
        pool = [7] * c7 + [4] * c4 + [3] * c3
        # round-robin interleave by taking from ends
        mix, lo, hi = [], 0, len(pool) - 1
        toggle = True
        while lo <= hi:
            if toggle:
                mix.append(pool[lo]); lo += 1
            else:
                mix.append(pool[hi]); hi -= 1
            toggle = not toggle
        seq = mix
        route.extend(seq)
    return route


def build_program(with_collective=True, cfg=None, repeat=1):
    cfg = dict(cfg or {})
    route = _make_route(cfg)

    nc = bass.Bass("TRN2", target_bir_lowering=False, debug=False,
                   enable_asserts=False, num_devices=N_CORES)

    xt_d = nc.dram_tensor("xt", [F, N], BF16, kind="ExternalInput")
    xtl_d = nc.dram_tensor("xtl", [F, I], BF16, kind="ExternalInput")
    mb_d = nc.dram_tensor("mb", [JT, 128, I], BF16, kind="ExternalInput")
    mq_d = nc.dram_tensor("mq", [JT, 128, I], BF16, kind="ExternalInput")
    wext_d = nc.dram_tensor("wext", [H, F, D + 2], BF16, kind="ExternalInput")
    a1rep_d = nc.dram_tensor("a1rep", [H, F, 128], BF16, kind="ExternalInput")
    woext_d = nc.dram_tensor("woext", [KT, 128, C + 2], BF16,
                             kind="ExternalInput")
    wcorr_d = nc.dram_tensor("wcorr", [1, C + 2], BF16, kind="ExternalInput")
    ident_d = nc.dram_tensor("ident", [128, 128], F32, kind="ExternalInput")
    outp_d = nc.dram_tensor("outp", [I, C], F32, kind="ExternalOutput")

    with tile.TileContext(nc) as tc:
        if repeat > 1:
            def body(iv, unroll=None):
                _build_body(nc, tc, xt_d, xtl_d, mb_d, mq_d, wext_d, a1rep_d,
                            woext_d, wcorr_d, ident_d, outp_d,
                            with_collective, route, cfg)
            with tc.For_i(0, repeat, 1) as iv:
                body(iv)
        else:
            _build_body(nc, tc, xt_d, xtl_d, mb_d, mq_d, wext_d, a1rep_d,
                        woext_d, wcorr_d, ident_d, outp_d,
                        with_collective, route, cfg)
    _split_multiwaits(nc)
    return nc


def _emit_pair(nc, work, workp, mode, subs, pair_args):
    """Emit one logit pair.  subs = [(jt, mb_ap, mq_ap, f2col_ap, e1col_ap,
    e2col_ap)]; pair_args = (f1rep, B1, B2)."""
    f1rep, B1, B2 = pair_args
    if mode == 7:
        q1 = work.tile([128, 2, I], BF16, tag="q1", bufs=2)
        q2 = work.tile([128, 2, I], BF16, tag="q2", bufs=2)
        for k, (jt, mb_ap, mq_ap, f2c, e1c, e2c) in enumerate(subs):
            nc.vector.scalar_tensor_tensor(
                out=q1[:, k, :], in0=mq_ap, scalar=e1c, in1=B1[:],
                op0=MULT, op1=MULT)
        for k, (jt, mb_ap, mq_ap, f2c, e1c, e2c) in enumerate(subs):
            nc.vector.scalar_tensor_tensor(
                out=q2[:, k, :], in0=mq_ap, scalar=e2c, in1=B2[:],
                op0=MULT, op1=MULT)
        p = workp.tile([128, 2, I], BF16, tag="p")
        nc.gpsimd.tensor_max(p[:], q1[:], q2[:])
        return p
    u = work.tile([128, 2, I], BF16, tag="u")
    for k, (jt, mb_ap, mq_ap, f2c, e1c, e2c) in enumerate(subs):
        if mode == 4:
            nc.gpsimd.tensor_add(u[:, k, :], mb_ap, f1rep[:])
        else:
            nc.vector.tensor_tensor(out=u[:, k, :], in0=mb_ap, in1=f1rep[:],
                                    op=ADD)
    for k, (jt, mb_ap, mq_ap, f2c, e1c, e2c) in enumerate(subs):
        nc.scalar.activation(u[:, k, :], u[:, k, :], ACT_LRELU,
                             bias=f2c, alpha=ALPHA)
    p = workp.tile([128, 2, I], BF16, tag="p")
    nc.scalar.activation(p[:], u[:], ACT_EXP)
    return p


def _build_body(nc, tc, xt_d, xtl_d, mb_d, mq_d, wext_d, a1rep_d, woext_d,
                wcorr_d, ident_d, outp_d, with_collective, route, cfg):
    from contextlib import ExitStack
    ctx = ExitStack()
    f1rep_act = bool(cfg.get("f1rep_act", True))
    rbc_act = bool(cfg.get("rbc_act", True))
    with ctx:
        singles = ctx.enter_context(tc.tile_pool(name="singles", bufs=1))
        psA = ctx.enter_context(tc.tile_pool(name="psA", bufs=2, space="PSUM"))
        psB = ctx.enter_context(tc.tile_pool(name="psB", bufs=1, space="PSUM"))
        psC = ctx.enter_context(tc.tile_pool(name="psC", bufs=2, space="PSUM"))
        dram = ctx.enter_context(tc.tile_pool(name="dram", bufs=1,
                                              space="DRAM"))

        # ---------------- persistent loads ----------------
        mb_s = singles.tile([128, JT, I], BF16)
        mq_s = singles.tile([128, JT, I], BF16)
        for jt in range(2):
            nc.sync.dma_start(out=mb_s[:, jt, :], in_=mb_d.ap()[jt])
            nc.sync.dma_start(out=mq_s[:, jt, :], in_=mq_d.ap()[jt])
        xtl_s = singles.tile([F, I], BF16)
        nc.sync.dma_start(out=xtl_s[:], in_=xtl_d.ap())
        a1rep_s = singles.tile([F, H, 128], BF16)
        nc.sync.dma_start(out=a1rep_s[:],
                          in_=a1rep_d.ap().rearrange("h f e -> f h e"))
        for jt in range(2, JT):
            nc.sync.dma_start(out=mb_s[:, jt, :], in_=mb_d.ap()[jt])
            nc.sync.dma_start(out=mq_s[:, jt, :], in_=mq_d.ap()[jt])
        wcorr_s = singles.tile([1, C + 2], BF16)
        nc.sync.dma_start(out=wcorr_s[:], in_=wcorr_d.ap())
        ident_s = singles.tile([128, 128], F32)
        nc.sync.dma_start(out=ident_s[:], in_=ident_d.ap())
        woext_s = singles.tile([128, KT, C + 2], BF16)
        nc.sync.dma_start(out=woext_s[:],
                          in_=woext_d.ap().rearrange("k f e -> f k e"))

        ones_s = singles.tile([1, 128], BF16)
        nc.gpsimd.memset(ones_s[:], 1.0)
        sel2_s = singles.tile([2, 128], BF16)
        nc.gpsimd.memset(sel2_s[:], 0.0)
        nc.gpsimd.memset(sel2_s[0:1, 0:D], 1.0)
        nc.gpsimd.memset(sel2_s[1:2, D:128], 1.0)

        whbuf = singles.tile([128, H, JT, D + 1], BF16)
        nc.gpsimd.memset(whbuf[:, :, :, D:D + 1], 1.0)
        fcol = singles.tile([128, H, JT, 1], F32)
        fexp1 = singles.tile([128, H, JT, 1], F32)
        fexp2 = singles.tile([128, H, JT, 1], F32)
        hcatT = singles.tile([128, KT, I], BF16)

        # ---------------- phase 0: Wh + f columns for all heads ----------
        with tc.tile_pool(name="ph0", bufs=2) as ph0:
            xt_s = ph0.tile([F, N], BF16, tag="xt", bufs=1)
            nc.sync.dma_start(out=xt_s[:], in_=xt_d.ap())
            wext_s = ph0.tile([F, H, D + 2], BF16, tag="wext", bufs=1)
            nc.sync.dma_start(out=wext_s[:],
                              in_=wext_d.ap().rearrange("h f e -> f h e"))
            g = 0
            for h in range(H):
                for jg in range(JT // 4):
                    whp = psA.tile([128, 4, D + 2], F32, tag="ph")
                    for k in range(4):
                        jt = jg * 4 + k
                        nc.tensor.matmul(whp[:, k, :],
                                         lhsT=xt_s[:, jt * 128:(jt + 1) * 128],
                                         rhs=wext_s[:, h, :])
                    dst = whbuf[:, h, jg * 4:(jg + 1) * 4, 0:D]
                    if g % 2 == 0:
                        nc.scalar.activation(dst, whp[:, :, 0:D], ACT_COPY)
                    else:
                        nc.vector.tensor_copy(out=dst, in_=whp[:, :, 0:D])
                    nc.vector.tensor_copy(
                        out=fcol[:, h, jg * 4:(jg + 1) * 4, :],
                        in_=whp[:, :, D + 1:D + 2])
                    g += 1
        nc.scalar.activation(fexp1[:], fcol[:], ACT_EXP)
        nc.scalar.activation(fexp2[:], fcol[:], ACT_EXP, scale=ALPHA)

        work = ctx.enter_context(tc.tile_pool(name="work", bufs=3))
        workp = ctx.enter_context(tc.tile_pool(name="workp", bufs=3))
        ep1 = ctx.enter_context(tc.tile_pool(name="ep1", bufs=1))
        ep2 = ctx.enter_context(tc.tile_pool(name="ep2", bufs=2))
        epL2 = ctx.enter_context(tc.tile_pool(name="epL2", bufs=1))

        # ---------------- layer 1 ----------------
        hT_prev = None
        for h in range(H):
            head_modes = route[h * PAIRS_PER_HEAD:(h + 1) * PAIRS_PER_HEAD]
            need_f1rep = any(m in (3, 4) for m in head_modes)
            need_B = any(m == 7 for m in head_modes)

            f1p = psB.tile([128, I], F32, tag="rep")
            for hf in range(I // 512):
                sl = slice(hf * 512, (hf + 1) * 512)
                nc.tensor.matmul(f1p[:, sl], lhsT=a1rep_s[:, h, :],
                                 rhs=xtl_s[:, sl])
            f1rep_s = B1_s = B2_s = None
            if need_f1rep:
                f1rep_s = ep2.tile([128, I], BF16, tag="f1rep")
                if f1rep_act:
                    nc.scalar.activation(f1rep_s[:], f1p[:], ACT_COPY)
                else:
                    nc.vector.tensor_copy(out=f1rep_s[:], in_=f1p[:])
            if need_B:
                B1_s = ep2.tile([128, I], BF16, tag="B1")
                nc.scalar.activation(B1_s[:], f1p[:], ACT_EXP)
                B2_s = ep2.tile([128, I], BF16, tag="B2")
                nc.scalar.activation(B2_s[:], f1p[:], ACT_EXP, scale=ALPHA)

            hT = psC.tile([D + 1, I], F32, tag="acc")
            for jp in range(PAIRS_PER_HEAD):
                mode = head_modes[jp]
                subs = []
                for k in range(2):
                    jt = jp * 2 + k
                    subs.append((jt, mb_s[:, jt, :], mq_s[:, jt, :],
                                 fcol[:, h, jt, :], fexp1[:, h, jt, :],
                                 fexp2[:, h, jt, :]))
                p = _emit_pair(nc, work, workp, mode, subs,
                               (f1rep_s, B1_s, B2_s))
                for k in range(2):
                    jt = jp * 2 + k
                    for hf in range(I // 512):
                        sl = slice(hf * 512, (hf + 1) * 512)
                        nc.tensor.matmul(hT[:, sl],
                                         lhsT=whbuf[:, h, jt, :],
                                         rhs=p[:, k, sl],
                                         start=(jt == 0), stop=(jt == JT - 1))

            if h % 2 == 0:
                hT_prev = hT
                continue

            # ---- epilogue for head pair (h-1, h) ----
            hTa, hTb = hT_prev, hT
            lnS = ep1.tile([2, I], F32, tag="lnS")
            nc.scalar.activation(lnS[0:1, :], hTa[D:D + 1, :], ACT_LN)
            nc.scalar.activation(lnS[1:2, :], hTb[D:D + 1, :], ACT_LN)
            r = ep1.tile([2, I], BF16, tag="r")
            nc.scalar.activation(r[:], lnS[:], ACT_EXP, scale=-1.0)
            rbcp = psB.tile([128, I], F32, tag="rep")
            for hf in range(I // 512):
                sl = slice(hf * 512, (hf + 1) * 512)
                nc.tensor.matmul(rbcp[:, sl], lhsT=sel2_s[:], rhs=r[:, sl])
            rbc_s = ep1.tile([128, I], BF16, tag="rbc")
            if rbc_act:
                nc.scalar.activation(rbc_s[:], rbcp[:], ACT_COPY)
            else:
                nc.vector.tensor_copy(out=rbc_s[:], in_=rbcp[:])
            v = ep1.tile([128, I], BF16, tag="v")
            nc.vector.tensor_tensor(out=v[0:D, :], in0=hTa[0:D, :],
                                    in1=rbc_s[0:D, :], op=MULT)
            nc.vector.tensor_tensor(out=v[D:128, :], in0=hTb[0:D, :],
                                    in1=rbc_s[D:128, :], op=MULT)
            t = ep1.tile([128, I], BF16, tag="t")
            nc.vector.tensor_scalar_min(t[:], v[:], 0.0)
            nc.scalar.activation(t[:], t[:], ACT_EXP)
            nc.vector.scalar_tensor_tensor(
                out=hcatT[:, h // 2, :], in0=v[:], scalar=0.0, in1=t[:],
                op0=MAX, op1=ADD)

        # ---------------- layer 2 projection + gather ----------------
        wh2loc = singles.tile([128, IC, C + 2], F32)
        gin = dram.tile([I, C + 2], F32)
        for ic in range(IC):
            w2p = psA.tile([128, 4, D + 2], F32, tag="ph")
            for kt in range(KT):
                nc.tensor.matmul(
                    w2p[:, 0, 0:C + 2],
                    lhsT=hcatT[:, kt, ic * 128:(ic + 1) * 128],
                    rhs=woext_s[:, kt, :],
                    start=(kt == 0), stop=False)
            nc.tensor.matmul(w2p[:, 0, 0:C + 2], lhsT=ones_s[0:1, :],
                             rhs=wcorr_s[:], start=False, stop=True)
            nc.vector.tensor_copy(out=wh2loc[:, ic, :], in_=w2p[:, 0, 0:C + 2])
            nc.sync.dma_start(out=gin[ic * 128:(ic + 1) * 128, :],
                              in_=wh2loc[:, ic, :])

        gout = dram.tile([N, C + 2], F32)
        if with_collective:
            nc.gpsimd.collective_compute(
                "AllGather", mybir.AluOpType.bypass,
                replica_groups=REPLICA_GROUPS,
                ins=[gin.opt()], outs=[gout.opt()])
        else:  # timing-model variant: fake the exchange with two local copies
            nc.sync.dma_start(out=gout[0:I, :], in_=gin[:])
            nc.sync.dma_start(out=gout[I:N, :], in_=gin[:])

        # g1 row (local queries) -> replicated [128, I]
        g1rowp = psB.tile([128, I], F32, tag="rep")
        for ic in range(IC):
            nc.tensor.transpose(g1rowp[0:1, ic * 128:(ic + 1) * 128],
                                in_=wh2loc[:, ic, 0:1], identity=ident_s[:])
        g1row_s = epL2.tile([1, I], BF16, tag="g1row")
        nc.scalar.activation(g1row_s[:], g1rowp[0:1, :], ACT_COPY)
        g1rp = psB.tile([128, I], F32, tag="rep")
        for hf in range(I // 512):
            sl = slice(hf * 512, (hf + 1) * 512)
            nc.tensor.matmul(g1rp[:, sl], lhsT=ones_s[0:1, :],
                             rhs=g1row_s[0:1, sl])
        l2_modes = route[H * PAIRS_PER_HEAD:]
        g1rep_s = B1L2 = B2L2 = None
        if any(m in (3, 4) for m in l2_modes):
            g1rep_s = singles.tile([128, I], BF16)
            nc.vector.tensor_copy(out=g1rep_s[:], in_=g1rp[:])
        if any(m == 7 for m in l2_modes):
            B1L2 = singles.tile([128, I], BF16)
            nc.scalar.activation(B1L2[:], g1rp[:], ACT_EXP)
            B2L2 = singles.tile([128, I], BF16)
            nc.scalar.activation(B2L2[:], g1rp[:], ACT_EXP, scale=ALPHA)

        # gathered rows: [g1, g2, Wh2(32)] f32 -> bf16 [+ ones col]
        wh2tmp = singles.tile([128, JT, C + 2], F32)
        for jt in range(JT):
            nc.sync.dma_start(out=wh2tmp[:, jt, :],
                              in_=gout[jt * 128:(jt + 1) * 128, :])
        wh2gr = singles.tile([128, JT, C + 3], BF16)
        nc.gpsimd.memset(wh2gr[:, :, C + 2:C + 3], 1.0)
        nc.vector.tensor_copy(out=wh2gr[:, :, 0:C + 2], in_=wh2tmp[:])
        its1 = singles.tile([128, JT, 1], F32)
        nc.scalar.activation(its1[:], wh2tmp[:, :, 1:2], ACT_EXP)
        its2 = singles.tile([128, JT, 1], F32)
        nc.scalar.activation(its2[:], wh2tmp[:, :, 1:2], ACT_EXP, scale=ALPHA)

        # ---------------- layer 2 attention ----------------
        o2T = psC.tile([D + 1, I], F32, tag="acc")
        for jp in range(PAIRS_PER_HEAD):
            mode = l2_modes[jp]
            subs = []
            for k in range(2):
                jt = jp * 2 + k
                subs.append((jt, mb_s[:, jt, :], mq_s[:, jt, :],
                             wh2tmp[:, jt, 1:2], its1[:, jt, :],
                             its2[:, jt, :]))
            p = _emit_pair(nc, work, workp, mode, subs,
                           (g1rep_s, B1L2, B2L2))
            for k in range(2):
                jt = jp * 2 + k
                for hf in range(I // 512):
                    sl = slice(hf * 512, (hf + 1) * 512)
                    nc.tensor.matmul(o2T[0:C + 1, sl],
                                     lhsT=wh2gr[:, jt, 2:C + 3],
                                     rhs=p[:, k, sl],
                                     start=(jt == 0), stop=(jt == JT - 1))

        # ---------------- finalize ----------------
        r2ln = epL2.tile([1, I], F32, tag="lnS2")
        nc.scalar.activation(r2ln[:], o2T[C:C + 1, :], ACT_LN)
        r2 = epL2.tile([1, I], BF16, tag="r2")
        nc.scalar.activation(r2[:], r2ln[:], ACT_EXP, scale=-1.0)
        rbc2p = psB.tile([128, I], F32, tag="rep")
        for hf in range(I // 512):
            sl = slice(hf * 512, (hf + 1) * 512)
            nc.tensor.matmul(rbc2p[0:C, sl], lhsT=ones_s[0:1, 0:C],
                             rhs=r2[0:1, sl])
        rbc2_s = epL2.tile([C, I], F32, tag="rbc2")
        nc.vector.tensor_copy(out=rbc2_s[:], in_=rbc2p[0:C, :])
        oT_s = epL2.tile([C, I], F32, tag="oT")
        nc.vector.tensor_tensor(out=oT_s[:], in0=o2T[0:C, :], in1=rbc2_s[:],
                                op=MULT)
        for k in range(IC):
            ofp = psA.tile([128, 4, D + 2], F32, tag="ph")
            nc.tensor.transpose(ofp[:, 0, 0:C],
                                in_=oT_s[:, k * 128:(k + 1) * 128],
                                identity=ident_s[0:C, 0:C])
            ofs = ep2.tile([128, C], F32, tag="ofs")
            nc.vector.tensor_copy(out=ofs[:], in_=ofp[:, 0, 0:C])
            nc.sync.dma_start(out=outp_d.ap()[k * 128:(k + 1) * 128, :],
                              in_=ofs[:])


# --------------------------------------------------------------------------
# host side
# --------------------------------------------------------------------------

def shard_inputs(x, adj, W, a1, a2, Wo, ao1, ao2):
    x = np.asarray(x, np.float32)
    adj = np.asarray(adj)
    W = np.asarray(W, np.float32)
    a1 = np.asarray(a1, np.float32)
    a2 = np.asarray(a2, np.float32)
    Wo = np.asarray(Wo, np.float32)
    ao1 = np.asarray(ao1, np.float32)
    ao2 = np.asarray(ao2, np.float32)
    BF = ml_dtypes.bfloat16

    wvec1 = np.einsum("hfd,hd->hf", W, a1)          # [H, F]
    wvec2 = np.einsum("hfd,hd->hf", W, a2)
    wext = np.concatenate([W, wvec1[:, :, None], wvec2[:, :, None]],
                          axis=2).astype(BF)
    a1rep = np.repeat(wvec1[:, :, None], 128, axis=2).astype(BF)
    wo1 = Wo @ ao1                                   # [512]
    wo2 = Wo @ ao2
    woflat = np.concatenate([wo1[:, None], wo2[:, None], Wo], 1)  # [512, 34]
    woext = woflat.reshape(KT, 128, C + 2).astype(BF)
    wcorr = (-woflat.sum(0))[None, :].astype(BF)
    ident = np.eye(128, dtype=np.float32)

    in_maps = []
    for c in range(N_CORES):
        b, half = c // 2, c % 2
        i0 = half * I
        xt = np.ascontiguousarray(x[b].T).astype(BF)   # [F, N]
        xtl = np.ascontiguousarray(xt[:, i0:i0 + I])
        adjt = adj[b, i0:i0 + I, :].T                # [N, I] = (j, i)
        mb = np.where(adjt > 0, np.float32(0.0), np.float32(-BIG))
        mb = np.ascontiguousarray(mb.reshape(JT, 128, I)).astype(BF)
        mq = np.where(adjt > 0, np.float32(1.0), np.float32(0.0))
        mq = np.ascontiguousarray(mq.reshape(JT, 128, I)).astype(BF)
        in_maps.append({
            "xt": xt, "xtl": xtl, "mb": mb, "mq": mq, "wext": wext,
            "a1rep": a1rep, "woext": woext, "wcorr": wcorr, "ident": ident,
        })
    return in_maps


# Engine routing chosen by cost-model sweep (TimelineSim).
DEFAULT_CFG = {"p4": 24, "p7": 33, "f1rep_act": True, "rbc_act": True}

_CACHE = {}


def _program():
    if "nc" not in _CACHE:
        _CACHE["nc"] = build_program(with_collective=True, cfg=DEFAULT_CFG)
    return _CACHE["nc"]


def kernel(**inputs):
    nc = _program()
    in_maps = shard_inputs(**inputs)
    res = run_bass_kernel_spmd(nc, in_maps, list(range(N_CORES)))
    _CACHE["last_results"] = res
    out = np.empty((B, N, C), np.float32)
    for c in range(N_CORES):
        b, half = c // 2, c % 2
        out[b, half * I:(half + 1) * I, :] = res.results[c]["outp"]
    return out


# revision 16
# speedup vs baseline: 1.0015x; 1.0015x over previous
"""GAT forward (2-layer graph attention, B=4 N=2048 F=128 H=8 D=64 C=32)
as a Bass/Tile SPMD kernel on 8 Trainium2 NeuronCores.

Sharding: core c -> (batch b=c//2, query-row half c%2).  Each core computes
attention for its 1024 query rows over all 2048 keys for all 8 heads
(layer 1) and for the output head (layer 2).  The only cross-core exchange
is a 2-rank AllGather of the layer-2 projections [g1|g2|Wh2] ([1024,34] f32)
within each (2b, 2b+1) pair.

Layout: attention logits are built TRANSPOSED, e^T[j (keys) = partitions,
i (queries) = free], so the PV matmul needs no operand transposes
(lhsT = Wh[j,d] stationary, rhs = p[j,i] moving, out = h^T[d,i]) and
softmax row sums come from a ones-column appended to Wh (PSUM row D).

The N^2 elementwise work (mask+f1+f2, leaky-relu, exp over 144 [128,1024]
logit tiles) is the bottleneck; everything runs in bf16 (2x DVE tensor-
tensor throughput; ACT is dtype-independent) and every logit PAIR of tiles
is routed down one of three pipelines to saturate ACT+DVE+GPSIMD jointly:

  P3 : u = mb + f1rep       (DVE tensor_tensor, bf16 2x)
       z = Prelu(u + f2col) (ACT, per-sub bias)     p = Exp(z)  (ACT)
  P4 : same but the combine runs on GPSIMD tensor_add
  P7 : exp(prelu(u)) == max(exp(u), exp(0.2u)) and exp(u) factors rank-1:
       q1 = (mb01 * exp(f2_j)) * exp(f1_i)   (DVE STT mult,mult)
       q2 = (mb01 * exp(.2 f2)) * exp(.2 f1) (DVE STT)
       p  = max(q1, q2)                      (GPSIMD tensor_max; no ACT!)

Softmax division is deferred to the per-head-pair epilogue (1/S via Ln/Exp
of the row-sum), fused with ELU via elu(v)+1 = relu(v) + exp(min(v,0)),
the +1 folded into a rank-1 correction matmul of the layer-2 projection.
"""

import numpy as np
import ml_dtypes

import concourse.bass as bass
import concourse.tile as tile
from concourse import mybir
from concourse.bass_utils import run_bass_kernel_spmd

F32 = mybir.dt.float32
BF16 = mybir.dt.bfloat16

B, N, F, H, D, C = 4, 2048, 128, 8, 64, 32
I = N // 2          # query rows per core
JT = N // 128       # key tiles
IC = I // 128       # query-row 128-chunks per core
KT = (H * D) // 128 # hidden-dim 128-chunks
ALPHA = 0.2
BIG = 512.0         # mask bias; exp(lrelu(-BIG+eps)) underflows to 0
N_CORES = 8
REPLICA_GROUPS = [[0, 1], [2, 3], [4, 5], [6, 7]]

ADD = mybir.AluOpType.add
MAX = mybir.AluOpType.max
MULT = mybir.AluOpType.mult
# NOTE: hardware "Lrelu" has a fixed 0.01 slope and ignores alpha;
# "Prelu" honors alpha (verified on HW) — it is the configurable leaky relu.
ACT_LRELU = mybir.ActivationFunctionType.Prelu
ACT_EXP = mybir.ActivationFunctionType.Exp
ACT_LN = mybir.ActivationFunctionType.Ln
ACT_COPY = mybir.ActivationFunctionType.Copy

N_HEADS_ALL = H + 1          # 8 layer-1 heads + the layer-2 output head
PAIRS_PER_HEAD = JT // 2     # 8
N_PAIRS = N_HEADS_ALL * PAIRS_PER_HEAD  # 72


def _split_multiwaits(nc):
    """Pinned walrus accepts only one sync-wait per instruction; Tile's exit
    drain (and occasionally others) carries several.  Hoist extras onto
    single-wait Drains on the same engine immediately before the owner."""
    n_fixed = 0
    for fn in nc.m.functions:
        for bb in fn.blocks:
            for name in [i.name for i in bb.instructions]:
                idx = [i.name for i in bb.instructions].index(name)
                inst = bb.instructions[idx]
                si = inst.sync_info
                if si is None or len(si.on_wait) <= 1:
                    continue
                waits = list(si.on_wait)
                for k, w in enumerate(waits[:-1]):
                    nd = mybir.InstDrain(
                        name=f"waitfix-{inst.name}-{k}", ins=[], outs=[])
                    nd.engine = inst.engine
                    nd.sync_info = mybir.SyncInfo(on_wait=[w], on_update=[])
                    nc.register_instruction(nd, overwrite=True)
                    bb.instructions.insert(idx + k, nd)
                inst.sync_info = mybir.SyncInfo(
                    on_wait=waits[-1:], on_update=list(si.on_update))
                n_fixed += 1
    return n_fixed


def _make_route(cfg):
    """Per-pair pipeline assignment:
      3 (DVE-add + ACT prelu/exp), 4 (GPS-add + ACT prelu/exp),
      7 (factored rank-1 exps on DVE + GPS max, no ACT),
      2 (DVE STT-add + DVE prelu + ACT exp only).
    Counts spread across the 9 'heads' (8 L1 + L2); p7 optionally
    concentrated into few heads (fewer exp(f1)-prep ops on ACT)."""
    if "route" in cfg:                      # explicit per-pair override
        route = list(cfg["route"])
        assert len(route) == N_PAIRS
        return route
    n4 = int(cfg.get("p4", 24))
    n7 = int(cfg.get("p7", 33))
    n2 = int(cfg.get("p2", 0))
    n3 = N_PAIRS - n4 - n7 - n2
    assert n3 >= 0
    conc = int(cfg.get("p7_conc", 0))  # 0 = spread; 1 = concentrate P7
    per_head = [[0, 0, 0, 0] for _ in range(N_HEADS_ALL)]  # [n3, n4, n7, n2]
    if conc:
        # fill P7 into heads round-robin starting at head 1, PAIRS_PER_HEAD
        # per head; distribute the rest evenly over remaining slots
        order = [1, 3, 5, 7, 8, 0, 2, 4, 6]
        left7 = n7
        for hh in order:
            take = min(left7, PAIRS_PER_HEAD)
            per_head[hh][2] = take
            left7 -= take
            if left7 == 0:
                break
        slots = [PAIRS_PER_HEAD - per_head[hh][2] for hh in range(N_HEADS_ALL)]
        for idx, cnt in ((1, n4), (3, n2), (0, n3)):
            left = cnt
            while left > 0:
                done = True
                for hh in range(N_HEADS_ALL):
                    used = sum(per_head[hh])
                    if used < PAIRS_PER_HEAD and left > 0:
                        per_head[hh][idx] += 1
                        left -= 1
                        done = False
                if done:
                    break
    else:
        for idx, cnt in enumerate((n3, n4, n7, n2)):
            base, rem = divmod(cnt, N_HEADS_ALL)
            for hh in range(N_HEADS_ALL):
                per_head[hh][idx] = base + (1 if hh < rem else 0)
        for hh in range(N_HEADS_ALL):
            while sum(per_head[hh]) > PAIRS_PER_HEAD:
                per_head[hh][int(np.argmax(per_head[hh]))] -= 1
            while sum(per_head[hh]) < PAIRS_PER_HEAD:
                per_head[hh][int(np.argmin(per_head[hh]))] += 1
    route = []
    for hh in range(N_HEADS_ALL):
        c3, c4, c7, c2 = per_head[hh]
        pool = [7] * c7 + [4] * c4 + [3] * c3 + [2] * c2
        mix, lo, hi = [], 0, len(pool) - 1
        toggle = True
        while lo <= hi:
            if toggle:
                mix.append(pool[lo]); lo += 1
            else:
                mix.append(pool[hi]); hi -= 1
            toggle = not toggle
        route.extend(mix)
    return route


def build_program(with_collective=True, cfg=None, repeat=1):
    cfg = dict(cfg or {})
    route = _make_route(cfg)

    nc = bass.Bass("TRN2", target_bir_lowering=False, debug=False,
                   enable_asserts=False, num_devices=N_CORES)

    xt_d = nc.dram_tensor("xt", [F, N], BF16, kind="ExternalInput")
    xtl_d = nc.dram_tensor("xtl", [F, I], BF16, kind="ExternalInput")
    mb_d = nc.dram_tensor("mb", [JT, 128, I], BF16, kind="ExternalInput")
    mq_d = nc.dram_tensor("mq", [JT, 128, I], BF16, kind="ExternalInput")
    wext_d = nc.dram_tensor("wext", [H, F, D + 2], BF16, kind="ExternalInput")
    a1rep_d = nc.dram_tensor("a1rep", [H, F, 128], BF16, kind="ExternalInput")
    woext_d = nc.dram_tensor("woext", [KT, 128, C + 2], BF16,
                             kind="ExternalInput")
    wcorr_d = nc.dram_tensor("wcorr", [1, C + 2], BF16, kind="ExternalInput")
    ident_d = nc.dram_tensor("ident", [128, 128], F32, kind="ExternalInput")
    outp_d = nc.dram_tensor("outp", [I, C], F32, kind="ExternalOutput")

    with tile.TileContext(nc) as tc:
        if repeat > 1:
            def body(iv, unroll=None):
                _build_body(nc, tc, xt_d, xtl_d, mb_d, mq_d, wext_d, a1rep_d,
                            woext_d, wcorr_d, ident_d, outp_d,
                            with_collective, route, cfg)
            with tc.For_i(0, repeat, 1) as iv:
                body(iv)
        else:
            _build_body(nc, tc, xt_d, xtl_d, mb_d, mq_d, wext_d, a1rep_d,
                        woext_d, wcorr_d, ident_d, outp_d,
                        with_collective, route, cfg)
    _split_multiwaits(nc)
    return nc


def _emit_pair(nc, work, workp, mode, subs, pair_args):
    """Emit one logit pair.  subs = [(jt, mb_ap, mq_ap, f2col_ap, e1col_ap,
    e2col_ap)]; pair_args = (f1rep, B1, B2)."""
    f1rep, B1, B2 = pair_args
    if mode == 7:
        q1 = work.tile([128, 2, I], BF16, tag="q1", bufs=2)
        q2 = work.tile([128, 2, I], BF16, tag="q2", bufs=2)
        for k, (jt, mb_ap, mq_ap, f2c, e1c, e2c) in enumerate(subs):
            nc.vector.scalar_tensor_tensor(
                out=q1[:, k, :], in0=mq_ap, scalar=e1c, in1=B1[:],
                op0=MULT, op1=MULT)
        for k, (jt, mb_ap, mq_ap, f2c, e1c, e2c) in enumerate(subs):
            nc.vector.scalar_tensor_tensor(
                out=q2[:, k, :], in0=mq_ap, scalar=e2c, in1=B2[:],
                op0=MULT, op1=MULT)
        p = workp.tile([128, 2, I], BF16, tag="p")
        nc.gpsimd.tensor_max(p[:], q1[:], q2[:])
        return p
    if mode == 2:
        # combine WITH f2 (STT) so prelu can run on DVE; exp is the only ACT op
        u = work.tile([128, 2, I], BF16, tag="u")
        for k, (jt, mb_ap, mq_ap, f2c, e1c, e2c) in enumerate(subs):
            nc.vector.scalar_tensor_tensor(
                out=u[:, k, :], in0=mb_ap, scalar=f2c, in1=f1rep[:],
                op0=ADD, op1=ADD)
        w = work.tile([128, 2, I], BF16, tag="q1", bufs=2)
        nc.vector.tensor_scalar_mul(w[:], u[:], ALPHA)     # bf16 4x
        nc.vector.tensor_tensor(out=u[:], in0=u[:], in1=w[:], op=MAX)
        p = workp.tile([128, 2, I], BF16, tag="p")
        nc.scalar.activation(p[:], u[:], ACT_EXP)
        return p
    u = work.tile([128, 2, I], BF16, tag="u")
    for k, (jt, mb_ap, mq_ap, f2c, e1c, e2c) in enumerate(subs):
        if mode == 4:
            nc.gpsimd.tensor_add(u[:, k, :], mb_ap, f1rep[:])
        else:
            nc.vector.tensor_tensor(out=u[:, k, :], in0=mb_ap, in1=f1rep[:],
                                    op=ADD)
    for k, (jt, mb_ap, mq_ap, f2c, e1c, e2c) in enumerate(subs):
        nc.scalar.activation(u[:, k, :], u[:, k, :], ACT_LRELU,
                             bias=f2c, alpha=ALPHA)
    p = workp.tile([128, 2, I], BF16, tag="p")
    nc.scalar.activation(p[:], u[:], ACT_EXP)
    return p


def _copy_engine(nc, eng, out, in_):
    if eng == "act":
        nc.scalar.activation(out, in_, ACT_COPY)
    elif eng == "gps":
        nc.gpsimd.tensor_copy(out=out, in_=in_)
    else:
        nc.vector.tensor_copy(out=out, in_=in_)


def _build_body(nc, tc, xt_d, xtl_d, mb_d, mq_d, wext_d, a1rep_d, woext_d,
                wcorr_d, ident_d, outp_d, with_collective, route, cfg):
    from contextlib import ExitStack
    ctx = ExitStack()
    f1rep_eng = cfg.get("f1rep_eng", "gps")
    rbc_eng = cfg.get("rbc_eng", "gps")
    ph0_engs = cfg.get("ph0_engs", ("act", "dve"))
    ep_v_gps = bool(cfg.get("ep_v_gps", False))
    with ctx:
        singles = ctx.enter_context(tc.tile_pool(name="singles", bufs=1))
        psA = ctx.enter_context(tc.tile_pool(name="psA", bufs=2, space="PSUM"))
        psB = ctx.enter_context(tc.tile_pool(name="psB", bufs=1, space="PSUM"))
        psC = ctx.enter_context(tc.tile_pool(name="psC", bufs=2, space="PSUM"))
        dram = ctx.enter_context(tc.tile_pool(name="dram", bufs=1,
                                              space="DRAM"))

        # ---------------- persistent loads ----------------
        mb_s = singles.tile([128, JT, I], BF16)
        mq_s = singles.tile([128, JT, I], BF16)
        for jt in range(2):
            nc.sync.dma_start(out=mb_s[:, jt, :], in_=mb_d.ap()[jt])
            nc.sync.dma_start(out=mq_s[:, jt, :], in_=mq_d.ap()[jt])
        xtl_s = singles.tile([F, I], BF16)
        nc.sync.dma_start(out=xtl_s[:], in_=xtl_d.ap())
        a1rep_s = singles.tile([F, H, 128], BF16)
        nc.sync.dma_start(out=a1rep_s[:],
                          in_=a1rep_d.ap().rearrange("h f e -> f h e"))
        for jt in range(2, JT):
            nc.sync.dma_start(out=mb_s[:, jt, :], in_=mb_d.ap()[jt])
            nc.sync.dma_start(out=mq_s[:, jt, :], in_=mq_d.ap()[jt])
        wcorr_s = singles.tile([1, C + 2], BF16)
        nc.sync.dma_start(out=wcorr_s[:], in_=wcorr_d.ap())
        ident_s = singles.tile([128, 128], F32)
        nc.sync.dma_start(out=ident_s[:], in_=ident_d.ap())
        identb_s = singles.tile([128, 128], BF16)
        nc.vector.tensor_copy(out=identb_s[:], in_=ident_s[:])
        woext_s = singles.tile([128, KT, C + 2], BF16)
        nc.sync.dma_start(out=woext_s[:],
                          in_=woext_d.ap().rearrange("k f e -> f k e"))

        ones_s = singles.tile([1, 128], BF16)
        nc.gpsimd.memset(ones_s[:], 1.0)
        sel2_s = singles.tile([2, 128], BF16)
        nc.gpsimd.memset(sel2_s[:], 0.0)
        nc.gpsimd.memset(sel2_s[0:1, 0:D], 1.0)
        nc.gpsimd.memset(sel2_s[1:2, D:128], 1.0)

        whbuf = singles.tile([128, H, JT, D + 1], BF16)
        nc.gpsimd.memset(whbuf[:, :, :, D:D + 1], 1.0)
        fcol = singles.tile([128, H, JT, 1], F32)
        fexp1 = singles.tile([128, H, JT, 1], F32)
        fexp2 = singles.tile([128, H, JT, 1], F32)
        hcatT = singles.tile([128, KT, I], BF16)

        # ---------------- phase 0: Wh + f columns for all heads ----------
        with tc.tile_pool(name="ph0", bufs=2) as ph0:
            xt_s = ph0.tile([F, N], BF16, tag="xt", bufs=1)
            nc.sync.dma_start(out=xt_s[:], in_=xt_d.ap())
            wext_s = ph0.tile([F, H, D + 2], BF16, tag="wext", bufs=1)
            nc.sync.dma_start(out=wext_s[:],
                              in_=wext_d.ap().rearrange("h f e -> f h e"))
            g = 0
            for h in range(H):
                for jg in range(JT // 4):
                    whp = psA.tile([128, 4, D + 2], F32, tag="ph")
                    for k in range(4):
                        jt = jg * 4 + k
                        nc.tensor.matmul(whp[:, k, :],
                                         lhsT=xt_s[:, jt * 128:(jt + 1) * 128],
                                         rhs=wext_s[:, h, :])
                    dst = whbuf[:, h, jg * 4:(jg + 1) * 4, 0:D]
                    _copy_engine(nc, ph0_engs[g % len(ph0_engs)],
                                 dst, whp[:, :, 0:D])
                    nc.vector.tensor_copy(
                        out=fcol[:, h, jg * 4:(jg + 1) * 4, :],
                        in_=whp[:, :, D + 1:D + 2])
                    g += 1
                # per-head so head-0 pairs need not wait for full phase 0
                nc.scalar.activation(fexp1[:, h], fcol[:, h], ACT_EXP)
                nc.scalar.activation(fexp2[:, h], fcol[:, h], ACT_EXP,
                                     scale=ALPHA)

        work = ctx.enter_context(tc.tile_pool(name="work", bufs=3))
        workp = ctx.enter_context(tc.tile_pool(name="workp", bufs=3))
        ep1 = ctx.enter_context(tc.tile_pool(name="ep1", bufs=1))
        ep2 = ctx.enter_context(tc.tile_pool(name="ep2", bufs=2))
        epL2 = ctx.enter_context(tc.tile_pool(name="epL2", bufs=1))

        # ---------------- layer 1 ----------------
        hT_prev = None
        for h in range(H):
            head_modes = route[h * PAIRS_PER_HEAD:(h + 1) * PAIRS_PER_HEAD]
            need_f1rep = any(m in (3, 4) for m in head_modes)
            need_B = any(m == 7 for m in head_modes)

            f1p = psB.tile([128, I], F32, tag="rep")
            for hf in range(I // 512):
                sl = slice(hf * 512, (hf + 1) * 512)
                nc.tensor.matmul(f1p[:, sl], lhsT=a1rep_s[:, h, :],
                                 rhs=xtl_s[:, sl])
            f1rep_s = B1_s = B2_s = None
            if need_f1rep:
                f1rep_s = ep2.tile([128, I], BF16, tag="f1rep")
                _copy_engine(nc, f1rep_eng, f1rep_s[:], f1p[:])
            if need_B:
                B1_s = ep2.tile([128, I], BF16, tag="B1")
                nc.scalar.activation(B1_s[:], f1p[:], ACT_EXP)
                B2_s = ep2.tile([128, I], BF16, tag="B2")
                nc.scalar.activation(B2_s[:], f1p[:], ACT_EXP, scale=ALPHA)

            hT = psC.tile([D + 1, I], F32, tag="acc")
            for jp in range(PAIRS_PER_HEAD):
                mode = head_modes[jp]
                subs = []
                for k in range(2):
                    jt = jp * 2 + k
                    subs.append((jt, mb_s[:, jt, :], mq_s[:, jt, :],
                                 fcol[:, h, jt, :], fexp1[:, h, jt, :],
                                 fexp2[:, h, jt, :]))
                p = _emit_pair(nc, work, workp, mode, subs,
                               (f1rep_s, B1_s, B2_s))
                for k in range(2):
                    jt = jp * 2 + k
                    for hf in range(I // 512):
                        sl = slice(hf * 512, (hf + 1) * 512)
                        nc.tensor.matmul(hT[:, sl],
                                         lhsT=whbuf[:, h, jt, :],
                                         rhs=p[:, k, sl],
                                         start=(jt == 0), stop=(jt == JT - 1))

            if h % 2 == 0:
                hT_prev = hT
                continue

            # ---- epilogue for head pair (h-1, h) ----
            hTa, hTb = hT_prev, hT
            lnS = ep1.tile([2, I], F32, tag="lnS")
            nc.scalar.activation(lnS[0:1, :], hTa[D:D + 1, :], ACT_LN)
            nc.scalar.activation(lnS[1:2, :], hTb[D:D + 1, :], ACT_LN)
            r = ep1.tile([2, I], BF16, tag="r")
            nc.scalar.activation(r[:], lnS[:], ACT_EXP, scale=-1.0)
            rbcp = psB.tile([128, I], F32, tag="rep")
            for hf in range(I // 512):
                sl = slice(hf * 512, (hf + 1) * 512)
                nc.tensor.matmul(rbcp[:, sl], lhsT=sel2_s[:], rhs=r[:, sl])
            rbc_s = ep1.tile([128, I], BF16, tag="rbc")
            _copy_engine(nc, rbc_eng, rbc_s[:], rbcp[:])
            v = ep1.tile([128, I], BF16, tag="v")
            if ep_v_gps:
                nc.gpsimd.tensor_mul(v[0:D, :], hTa[0:D, :], rbc_s[0:D, :])
                nc.gpsimd.tensor_mul(v[D:128, :], hTb[0:D, :],
                                     rbc_s[D:128, :])
            else:
                nc.vector.tensor_tensor(out=v[0:D, :], in0=hTa[0:D, :],
                                        in1=rbc_s[0:D, :], op=MULT)
                nc.vector.tensor_tensor(out=v[D:128, :], in0=hTb[0:D, :],
                                        in1=rbc_s[D:128, :], op=MULT)
            t = ep1.tile([128, I], BF16, tag="t")
            nc.vector.tensor_scalar_min(t[:], v[:], 0.0)
            nc.scalar.activation(t[:], t[:], ACT_EXP)
            nc.vector.scalar_tensor_tensor(
                out=hcatT[:, h // 2, :], in0=v[:], scalar=0.0, in1=t[:],
                op0=MAX, op1=ADD)

        # ---------------- layer 2 projection + gather (bf16 payload) ------
        wh2loc = singles.tile([128, IC, C + 2], BF16)
        g1c = singles.tile([128, IC, 1], F32)
        gin = dram.tile([I, C + 2], BF16)
        for ic in range(IC):
            w2p = psA.tile([128, 4, D + 2], F32, tag="ph")
            for kt in range(KT):
                nc.tensor.matmul(
                    w2p[:, 0, 0:C + 2],
                    lhsT=hcatT[:, kt, ic * 128:(ic + 1) * 128],
                    rhs=woext_s[:, kt, :],
                    start=(kt == 0), stop=False)
            nc.tensor.matmul(w2p[:, 0, 0:C + 2], lhsT=ones_s[0:1, :],
                             rhs=wcorr_s[:], start=False, stop=True)
            nc.vector.tensor_copy(out=wh2loc[:, ic, :], in_=w2p[:, 0, 0:C + 2])
            nc.vector.tensor_copy(out=g1c[:, ic, :], in_=w2p[:, 0, 0:1])
            nc.sync.dma_start(out=gin[ic * 128:(ic + 1) * 128, :],
                              in_=wh2loc[:, ic, :])

        gout = dram.tile([N, C + 2], BF16)
        if with_collective:
            nc.gpsimd.collective_compute(
                "AllGather", mybir.AluOpType.bypass,
                replica_groups=REPLICA_GROUPS,
                ins=[gin.opt()], outs=[gout.opt()])
        else:  # timing-model variant: fake the exchange with two local copies
            nc.sync.dma_start(out=gout[0:I, :], in_=gin[:])
            nc.sync.dma_start(out=gout[I:N, :], in_=gin[:])

        # g1 row (local queries) -> replicated [128, I]
        g1rowp = psB.tile([128, I], F32, tag="rep")
        for ic in range(IC):
            nc.tensor.transpose(g1rowp[0:1, ic * 128:(ic + 1) * 128],
                                in_=g1c[:, ic, :], identity=ident_s[:])
        g1row_s = epL2.tile([1, I], BF16, tag="g1row")
        nc.scalar.activation(g1row_s[:], g1rowp[0:1, :], ACT_COPY)
        g1rp = psB.tile([128, I], F32, tag="rep")
        for hf in range(I // 512):
            sl = slice(hf * 512, (hf + 1) * 512)
            nc.tensor.matmul(g1rp[:, sl], lhsT=ones_s[0:1, :],
                             rhs=g1row_s[0:1, sl])
        l2_modes = route[H * PAIRS_PER_HEAD:]
        g1rep_s = B1L2 = B2L2 = None
        if any(m in (2, 3, 4) for m in l2_modes):
            g1rep_s = singles.tile([128, I], BF16)
            nc.vector.tensor_copy(out=g1rep_s[:], in_=g1rp[:])
        if any(m == 7 for m in l2_modes):
            B1L2 = singles.tile([128, I], BF16)
            nc.scalar.activation(B1L2[:], g1rp[:], ACT_EXP)
            B2L2 = singles.tile([128, I], BF16)
            nc.scalar.activation(B2L2[:], g1rp[:], ACT_EXP, scale=ALPHA)

        # gathered rows: [g1, g2, Wh2(32)] bf16 + ones col, direct DMA
        wh2gr = singles.tile([128, JT, C + 3], BF16)
        nc.gpsimd.memset(wh2gr[:, :, C + 2:C + 3], 1.0)
        for jt in range(JT):
            nc.sync.dma_start(out=wh2gr[:, jt, 0:C + 2],
                              in_=gout[jt * 128:(jt + 1) * 128, :])
        # per-4jt g2 cols (f32 for bias/scalar reads) + their exps
        g2c = singles.tile([128, JT, 1], F32)
        its1 = singles.tile([128, JT, 1], F32)
        its2 = singles.tile([128, JT, 1], F32)
        for jg in range(JT // 4):
            s4 = slice(jg * 4, (jg + 1) * 4)
            nc.vector.tensor_copy(out=g2c[:, s4, :], in_=wh2gr[:, s4, 1:2])
            nc.scalar.activation(its1[:, s4, :], g2c[:, s4, :], ACT_EXP)
            nc.scalar.activation(its2[:, s4, :], g2c[:, s4, :], ACT_EXP,
                                 scale=ALPHA)

        # ---------------- layer 2 attention ----------------
        o2T = psC.tile([D + 1, I], F32, tag="acc")
        for jp in range(PAIRS_PER_HEAD):
            mode = l2_modes[jp]
            subs = []
            for k in range(2):
                jt = jp * 2 + k
                subs.append((jt, mb_s[:, jt, :], mq_s[:, jt, :],
                             g2c[:, jt, :], its1[:, jt, :],
                             its2[:, jt, :]))
            p = _emit_pair(nc, work, workp, mode, subs,
                           (g1rep_s, B1L2, B2L2))
            for k in range(2):
                jt = jp * 2 + k
                for hf in range(I // 512):
                    sl = slice(hf * 512, (hf + 1) * 512)
                    nc.tensor.matmul(o2T[0:C + 1, sl],
                                     lhsT=wh2gr[:, jt, 2:C + 3],
                                     rhs=p[:, k, sl],
                                     start=(jt == 0), stop=(jt == JT - 1))

        # ---------------- finalize (transposed: per-query reciprocal) -----
        o2s = epL2.tile([C + 1, I], F32, tag="o2s")
        nc.vector.tensor_copy(out=o2s[:], in_=o2T[0:C + 1, :])
        for k in range(IC):
            ck = slice(k * 128, (k + 1) * 128)
            ofp = psA.tile([128, 4, D + 2], F32, tag="ph")
            nc.tensor.transpose(ofp[:, 0, 0:C], in_=o2s[0:C, ck],
                                identity=ident_s[0:C, 0:C])
            nc.tensor.transpose(ofp[:, 0, C + 2:C + 3], in_=o2s[C:C + 1, ck],
                                identity=ident_s[C:C + 1, C:C + 1])
            s2t = ep2.tile([128, 1], F32, tag="s2t")
            nc.vector.reciprocal(s2t[:], ofp[:, 0, C + 2:C + 3])
            ofs = ep2.tile([128, C], F32, tag="ofs")
            nc.vector.tensor_scalar_mul(ofs[:], ofp[:, 0, 0:C], s2t[:])
            nc.sync.dma_start(out=outp_d.ap()[k * 128:(k + 1) * 128, :],
                              in_=ofs[:])


# --------------------------------------------------------------------------
# host side
# --------------------------------------------------------------------------

def shard_inputs(x, adj, W, a1, a2, Wo, ao1, ao2):
    x = np.asarray(x, np.float32)
    adj = np.asarray(adj)
    W = np.asarray(W, np.float32)
    a1 = np.asarray(a1, np.float32)
    a2 = np.asarray(a2, np.float32)
    Wo = np.asarray(Wo, np.float32)
    ao1 = np.asarray(ao1, np.float32)
    ao2 = np.asarray(ao2, np.float32)
    BF = ml_dtypes.bfloat16

    wvec1 = np.einsum("hfd,hd->hf", W, a1)          # [H, F]
    wvec2 = np.einsum("hfd,hd->hf", W, a2)
    wext = np.concatenate([W, wvec1[:, :, None], wvec2[:, :, None]],
                          axis=2).astype(BF)
    a1rep = np.repeat(wvec1[:, :, None], 128, axis=2).astype(BF)
    wo1 = Wo @ ao1                                   # [512]
    wo2 = Wo @ ao2
    woflat = np.concatenate([wo1[:, None], wo2[:, None], Wo], 1)  # [512, 34]
    woext = woflat.reshape(KT, 128, C + 2).astype(BF)
    wcorr = (-woflat.sum(0))[None, :].astype(BF)
    ident = np.eye(128, dtype=np.float32)

    in_maps = []
    for c in range(N_CORES):
        b, half = c // 2, c % 2
        i0 = half * I
        xt = np.ascontiguousarray(x[b].T).astype(BF)   # [F, N]
        xtl = np.ascontiguousarray(xt[:, i0:i0 + I])
        adjt = adj[b, i0:i0 + I, :].T                # [N, I] = (j, i)
        mb = np.where(adjt > 0, np.float32(0.0), np.float32(-BIG))
        mb = np.ascontiguousarray(mb.reshape(JT, 128, I)).astype(BF)
        mq = np.where(adjt > 0, np.float32(1.0), np.float32(0.0))
        mq = np.ascontiguousarray(mq.reshape(JT, 128, I)).astype(BF)
        in_maps.append({
            "xt": xt, "xtl": xtl, "mb": mb, "mq": mq, "wext": wext,
            "a1rep": a1rep, "woext": woext, "wcorr": wcorr, "ident": ident,
        })
    return in_maps


# Engine routing chosen by cost-model sweep (TimelineSim).
DEFAULT_CFG = {"p4": 24, "p7": 33, "f1rep_act": True, "rbc_act": True}

_CACHE = {}


def _program():
    if "nc" not in _CACHE:
        _CACHE["nc"] = build_program(with_collective=True, cfg=DEFAULT_CFG)
    return _CACHE["nc"]


def kernel(**inputs):
    nc = _program()
    in_maps = shard_inputs(**inputs)
    res = run_bass_kernel_spmd(nc, in_maps, list(range(N_CORES)))
    _CACHE["last_results"] = res
    out = np.empty((B, N, C), np.float32)
    for c in range(N_CORES):
        b, half = c // 2, c % 2
        out[b, half * I:(half + 1) * I, :] = res.results[c]["outp"]
    return out


# revision 18
# speedup vs baseline: 1.0757x; 1.0741x over previous
"""GAT forward (2-layer graph attention, B=4 N=2048 F=128 H=8 D=64 C=32)
as a Bass/Tile SPMD kernel on 8 Trainium2 NeuronCores.

Sharding: core c -> (batch b=c//2, query-row half c%2).  Each core computes
attention for its 1024 query rows over all 2048 keys for all 8 heads
(layer 1) and for the output head (layer 2).  The only cross-core exchange
is a 2-rank AllGather of the layer-2 projections [g1|g2|Wh2] ([1024,34] f32)
within each (2b, 2b+1) pair.

Layout: attention logits are built TRANSPOSED, e^T[j (keys) = partitions,
i (queries) = free], so the PV matmul needs no operand transposes
(lhsT = Wh[j,d] stationary, rhs = p[j,i] moving, out = h^T[d,i]) and
softmax row sums come from a ones-column appended to Wh (PSUM row D).

The N^2 elementwise work (mask+f1+f2, leaky-relu, exp over 144 [128,1024]
logit tiles) is the bottleneck; everything runs in bf16 (2x DVE tensor-
tensor throughput; ACT is dtype-independent) and every logit PAIR of tiles
is routed down one of three pipelines to saturate ACT+DVE+GPSIMD jointly:

  P3 : u = mb + f1rep       (DVE tensor_tensor, bf16 2x)
       z = Prelu(u + f2col) (ACT, per-sub bias)     p = Exp(z)  (ACT)
  P4 : same but the combine runs on GPSIMD tensor_add
  P7 : exp(prelu(u)) == max(exp(u), exp(0.2u)) and exp(u) factors rank-1:
       q1 = (mb01 * exp(f2_j)) * exp(f1_i)   (DVE STT mult,mult)
       q2 = (mb01 * exp(.2 f2)) * exp(.2 f1) (DVE STT)
       p  = max(q1, q2)                      (GPSIMD tensor_max; no ACT!)

Softmax division is deferred to the per-head-pair epilogue (1/S via Ln/Exp
of the row-sum), fused with ELU via elu(v)+1 = relu(v) + exp(min(v,0)),
the +1 folded into a rank-1 correction matmul of the layer-2 projection.
"""

import numpy as np
import ml_dtypes

import concourse.bass as bass
import concourse.tile as tile
from concourse import mybir
from concourse.bass_utils import run_bass_kernel_spmd

F32 = mybir.dt.float32
BF16 = mybir.dt.bfloat16

B, N, F, H, D, C = 4, 2048, 128, 8, 64, 32
I = N // 2          # query rows per core
JT = N // 128       # key tiles
IC = I // 128       # query-row 128-chunks per core
KT = (H * D) // 128 # hidden-dim 128-chunks
ALPHA = 0.2
BIG = 512.0         # mask bias; exp(lrelu(-BIG+eps)) underflows to 0
N_CORES = 8
REPLICA_GROUPS = [[0, 1], [2, 3], [4, 5], [6, 7]]

ADD = mybir.AluOpType.add
MAX = mybir.AluOpType.max
MULT = mybir.AluOpType.mult
# NOTE: hardware "Lrelu" has a fixed 0.01 slope and ignores alpha;
# "Prelu" honors alpha (verified on HW) — it is the configurable leaky relu.
ACT_LRELU = mybir.ActivationFunctionType.Prelu
ACT_EXP = mybir.ActivationFunctionType.Exp
ACT_LN = mybir.ActivationFunctionType.Ln
ACT_COPY = mybir.ActivationFunctionType.Copy

N_HEADS_ALL = H + 1          # 8 layer-1 heads + the layer-2 output head
PAIRS_PER_HEAD = JT // 2     # 8
N_PAIRS = N_HEADS_ALL * PAIRS_PER_HEAD  # 72


def _split_multiwaits(nc):
    """Pinned walrus accepts only one sync-wait per instruction; Tile's exit
    drain (and occasionally others) carries several.  Hoist extras onto
    single-wait Drains on the same engine immediately before the owner."""
    n_fixed = 0
    for fn in nc.m.functions:
        for bb in fn.blocks:
            for name in [i.name for i in bb.instructions]:
                idx = [i.name for i in bb.instructions].index(name)
                inst = bb.instructions[idx]
                si = inst.sync_info
                if si is None or len(si.on_wait) <= 1:
                    continue
                waits = list(si.on_wait)
                for k, w in enumerate(waits[:-1]):
                    nd = mybir.InstDrain(
                        name=f"waitfix-{inst.name}-{k}", ins=[], outs=[])
                    nd.engine = inst.engine
                    nd.sync_info = mybir.SyncInfo(on_wait=[w], on_update=[])
                    nc.register_instruction(nd, overwrite=True)
                    bb.instructions.insert(idx + k, nd)
                inst.sync_info = mybir.SyncInfo(
                    on_wait=waits[-1:], on_update=list(si.on_update))
                n_fixed += 1
    return n_fixed


def _make_route(cfg):
    """Per-pair pipeline assignment:
      3 (DVE-add + ACT prelu/exp), 4 (GPS-add + ACT prelu/exp),
      7 (factored rank-1 exps on DVE + GPS max, no ACT),
      2 (DVE STT-add + DVE prelu + ACT exp only).
    Counts spread across the 9 'heads' (8 L1 + L2); p7 optionally
    concentrated into few heads (fewer exp(f1)-prep ops on ACT)."""
    if "route" in cfg:                      # explicit per-pair override
        route = list(cfg["route"])
        assert len(route) == N_PAIRS
        return route
    n4 = int(cfg.get("p4", 24))
    n7 = int(cfg.get("p7", 33))
    n2 = int(cfg.get("p2", 0))
    n3 = N_PAIRS - n4 - n7 - n2
    assert n3 >= 0
    conc = int(cfg.get("p7_conc", 0))  # 0 = spread; 1 = concentrate P7
    per_head = [[0, 0, 0, 0] for _ in range(N_HEADS_ALL)]  # [n3, n4, n7, n2]
    if conc:
        # fill P7 into heads round-robin starting at head 1, PAIRS_PER_HEAD
        # per head; distribute the rest evenly over remaining slots
        order = [1, 3, 5, 7, 8, 0, 2, 4, 6]
        left7 = n7
        for hh in order:
            take = min(left7, PAIRS_PER_HEAD)
            per_head[hh][2] = take
            left7 -= take
            if left7 == 0:
                break
        slots = [PAIRS_PER_HEAD - per_head[hh][2] for hh in range(N_HEADS_ALL)]
        for idx, cnt in ((1, n4), (3, n2), (0, n3)):
            left = cnt
            while left > 0:
                done = True
                for hh in range(N_HEADS_ALL):
                    used = sum(per_head[hh])
                    if used < PAIRS_PER_HEAD and left > 0:
                        per_head[hh][idx] += 1
                        left -= 1
                        done = False
                if done:
                    break
    else:
        for idx, cnt in enumerate((n3, n4, n7, n2)):
            base, rem = divmod(cnt, N_HEADS_ALL)
            for hh in range(N_HEADS_ALL):
                per_head[hh][idx] = base + (1 if hh < rem else 0)
        for hh in range(N_HEADS_ALL):
            while sum(per_head[hh]) > PAIRS_PER_HEAD:
                per_head[hh][int(np.argmax(per_head[hh]))] -= 1
            while sum(per_head[hh]) < PAIRS_PER_HEAD:
                per_head[hh][int(np.argmin(per_head[hh]))] += 1
    route = []
    for hh in range(N_HEADS_ALL):
        c3, c4, c7, c2 = per_head[hh]
        pool = [7] * c7 + [4] * c4 + [3] * c3 + [2] * c2
        mix, lo, hi = [], 0, len(pool) - 1
        toggle = True
        while lo <= hi:
            if toggle:
                mix.append(pool[lo]); lo += 1
            else:
                mix.append(pool[hi]); hi -= 1
            toggle = not toggle
        route.extend(mix)
    return route


def build_program(with_collective=True, cfg=None, repeat=1):
    cfg = dict(cfg or {})
    route = _make_route(cfg)

    nc = bass.Bass("TRN2", target_bir_lowering=False, debug=False,
                   enable_asserts=False, num_devices=N_CORES)

    xt_d = nc.dram_tensor("xt", [F, N], BF16, kind="ExternalInput")
    xtl_d = nc.dram_tensor("xtl", [F, I], BF16, kind="ExternalInput")
    mb_d = nc.dram_tensor("mb", [JT, 128, I], BF16, kind="ExternalInput")
    mq_d = nc.dram_tensor("mq", [JT, 128, I], BF16, kind="ExternalInput")
    wext_d = nc.dram_tensor("wext", [H, F, D + 2], BF16, kind="ExternalInput")
    a1rep_d = nc.dram_tensor("a1rep", [H, F, 128], BF16, kind="ExternalInput")
    woext_d = nc.dram_tensor("woext", [KT, 128, C + 2], BF16,
                             kind="ExternalInput")
    wcorr_d = nc.dram_tensor("wcorr", [1, C + 2], BF16, kind="ExternalInput")
    ident_d = nc.dram_tensor("ident", [128, 128], F32, kind="ExternalInput")
    outp_d = nc.dram_tensor("outp", [I, C], F32, kind="ExternalOutput")

    with tile.TileContext(nc) as tc:
        if repeat > 1:
            def body(iv, unroll=None):
                _build_body(nc, tc, xt_d, xtl_d, mb_d, mq_d, wext_d, a1rep_d,
                            woext_d, wcorr_d, ident_d, outp_d,
                            with_collective, route, cfg)
            with tc.For_i(0, repeat, 1) as iv:
                body(iv)
        else:
            _build_body(nc, tc, xt_d, xtl_d, mb_d, mq_d, wext_d, a1rep_d,
                        woext_d, wcorr_d, ident_d, outp_d,
                        with_collective, route, cfg)
    _split_multiwaits(nc)
    return nc


def _emit_pair(nc, work, workp, mode, subs, pair_args):
    """Emit one logit pair.  subs = [(jt, mb_ap, mq_ap, f2col_ap, e1col_ap,
    e2col_ap)]; pair_args = (f1rep, B1, B2)."""
    f1rep, B1, B2 = pair_args
    if mode == 7:
        q1 = work.tile([128, 2, I], BF16, tag="q1", bufs=2)
        q2 = work.tile([128, 2, I], BF16, tag="q2", bufs=2)
        for k, (jt, mb_ap, mq_ap, f2c, e1c, e2c) in enumerate(subs):
            nc.vector.scalar_tensor_tensor(
                out=q1[:, k, :], in0=mq_ap, scalar=e1c, in1=B1[:],
                op0=MULT, op1=MULT)
        for k, (jt, mb_ap, mq_ap, f2c, e1c, e2c) in enumerate(subs):
            nc.vector.scalar_tensor_tensor(
                out=q2[:, k, :], in0=mq_ap, scalar=e2c, in1=B2[:],
                op0=MULT, op1=MULT)
        p = workp.tile([128, 2, I], BF16, tag="p")
        nc.gpsimd.tensor_max(p[:], q1[:], q2[:])
        return p
    if mode == 2:
        # combine WITH f2 (STT) so prelu can run on DVE; exp is the only ACT op
        u = work.tile([128, 2, I], BF16, tag="u")
        for k, (jt, mb_ap, mq_ap, f2c, e1c, e2c) in enumerate(subs):
            nc.vector.scalar_tensor_tensor(
                out=u[:, k, :], in0=mb_ap, scalar=f2c, in1=f1rep[:],
                op0=ADD, op1=ADD)
        w = work.tile([128, 2, I], BF16, tag="q1", bufs=2)
        nc.vector.tensor_scalar_mul(w[:], u[:], ALPHA)     # bf16 4x
        nc.vector.tensor_tensor(out=u[:], in0=u[:], in1=w[:], op=MAX)
        p = workp.tile([128, 2, I], BF16, tag="p")
        nc.scalar.activation(p[:], u[:], ACT_EXP)
        return p
    u = work.tile([128, 2, I], BF16, tag="u")
    for k, (jt, mb_ap, mq_ap, f2c, e1c, e2c) in enumerate(subs):
        if mode == 4:
            nc.gpsimd.tensor_add(u[:, k, :], mb_ap, f1rep[:])
        else:
            nc.vector.tensor_tensor(out=u[:, k, :], in0=mb_ap, in1=f1rep[:],
                                    op=ADD)
    for k, (jt, mb_ap, mq_ap, f2c, e1c, e2c) in enumerate(subs):
        nc.scalar.activation(u[:, k, :], u[:, k, :], ACT_LRELU,
                             bias=f2c, alpha=ALPHA)
    p = workp.tile([128, 2, I], BF16, tag="p")
    nc.scalar.activation(p[:], u[:], ACT_EXP)
    return p


def _copy_engine(nc, eng, out, in_):
    if eng == "act":
        nc.scalar.activation(out, in_, ACT_COPY)
    elif eng == "gps":
        nc.gpsimd.tensor_copy(out=out, in_=in_)
    else:
        nc.vector.tensor_copy(out=out, in_=in_)


def _build_body(nc, tc, xt_d, xtl_d, mb_d, mq_d, wext_d, a1rep_d, woext_d,
                wcorr_d, ident_d, outp_d, with_collective, route, cfg):
    from contextlib import ExitStack
    ctx = ExitStack()
    f1rep_eng = cfg.get("f1rep_eng", "gps")
    rbc_eng = cfg.get("rbc_eng", "gps")
    ph0_engs = cfg.get("ph0_engs", ("act", "dve"))
    ep_v_gps = bool(cfg.get("ep_v_gps", False))
    with ctx:
        singles = ctx.enter_context(tc.tile_pool(name="singles", bufs=1))
        psA = ctx.enter_context(tc.tile_pool(name="psA", bufs=2, space="PSUM"))
        psB = ctx.enter_context(tc.tile_pool(name="psB", bufs=1, space="PSUM"))
        psC = ctx.enter_context(tc.tile_pool(name="psC", bufs=2, space="PSUM"))
        dram = ctx.enter_context(tc.tile_pool(name="dram", bufs=1,
                                              space="DRAM"))

        # ---------------- persistent loads ----------------
        mb_s = singles.tile([128, JT, I], BF16)
        mq_s = singles.tile([128, JT, I], BF16)
        for jt in range(2):
            nc.sync.dma_start(out=mb_s[:, jt, :], in_=mb_d.ap()[jt])
            nc.sync.dma_start(out=mq_s[:, jt, :], in_=mq_d.ap()[jt])
        xtl_s = singles.tile([F, I], BF16)
        nc.sync.dma_start(out=xtl_s[:], in_=xtl_d.ap())
        a1rep_s = singles.tile([F, H, 128], BF16)
        nc.sync.dma_start(out=a1rep_s[:],
                          in_=a1rep_d.ap().rearrange("h f e -> f h e"))
        for jt in range(2, JT):
            nc.sync.dma_start(out=mb_s[:, jt, :], in_=mb_d.ap()[jt])
            nc.sync.dma_start(out=mq_s[:, jt, :], in_=mq_d.ap()[jt])
        wcorr_s = singles.tile([1, C + 2], BF16)
        nc.sync.dma_start(out=wcorr_s[:], in_=wcorr_d.ap())
        ident_s = singles.tile([128, 128], F32)
        nc.sync.dma_start(out=ident_s[:], in_=ident_d.ap())
        identb_s = singles.tile([128, 128], BF16)
        nc.vector.tensor_copy(out=identb_s[:], in_=ident_s[:])
        woext_s = singles.tile([128, KT, C + 2], BF16)
        nc.sync.dma_start(out=woext_s[:],
                          in_=woext_d.ap().rearrange("k f e -> f k e"))

        ones_s = singles.tile([1, 128], BF16)
        nc.gpsimd.memset(ones_s[:], 1.0)
        sel2_s = singles.tile([2, 128], BF16)
        nc.gpsimd.memset(sel2_s[:], 0.0)
        nc.gpsimd.memset(sel2_s[0:1, 0:D], 1.0)
        nc.gpsimd.memset(sel2_s[1:2, D:128], 1.0)

        whbuf = singles.tile([128, H, JT, D + 1], BF16)
        nc.gpsimd.memset(whbuf[:, :, :, D:D + 1], 1.0)
        fcol = singles.tile([128, H, JT, 1], F32)
        fexp1 = singles.tile([128, H, JT, 1], F32)
        fexp2 = singles.tile([128, H, JT, 1], F32)
        hcatT = singles.tile([128, KT, I], BF16)

        xt_s = singles.tile([F, N], BF16)
        nc.sync.dma_start(out=xt_s[:], in_=xt_d.ap())
        wext_s = singles.tile([F, H, D + 2], BF16)
        nc.sync.dma_start(out=wext_s[:],
                          in_=wext_d.ap().rearrange("h f e -> f h e"))

        work = ctx.enter_context(tc.tile_pool(name="work", bufs=3))
        workp = ctx.enter_context(tc.tile_pool(name="workp", bufs=3))
        ep1 = ctx.enter_context(tc.tile_pool(name="ep1", bufs=1))
        ep2 = ctx.enter_context(tc.tile_pool(name="ep2", bufs=2))
        epL2 = ctx.enter_context(tc.tile_pool(name="epL2", bufs=1))

        def emit_phase0_head(h):
            # Wh tiles + f columns for head h (emitted per-head so the
            # copies overlap the previous head's logit work)
            for jg in range(JT // 4):
                whp = psA.tile([128, 4, D + 2], F32, tag="ph")
                for k in range(4):
                    jt = jg * 4 + k
                    nc.tensor.matmul(whp[:, k, :],
                                     lhsT=xt_s[:, jt * 128:(jt + 1) * 128],
                                     rhs=wext_s[:, h, :])
                dst = whbuf[:, h, jg * 4:(jg + 1) * 4, 0:D]
                _copy_engine(nc, ph0_engs[jg % len(ph0_engs)],
                             dst, whp[:, :, 0:D])
                nc.vector.tensor_copy(
                    out=fcol[:, h, jg * 4:(jg + 1) * 4, :],
                    in_=whp[:, :, D + 1:D + 2])
            nc.scalar.activation(fexp1[:, h], fcol[:, h], ACT_EXP)
            nc.scalar.activation(fexp2[:, h], fcol[:, h], ACT_EXP,
                                 scale=ALPHA)

        ep_state = {}

        def emit_half_ep(hT, h):
            # per-head half-epilogue: r = 1/S via Ln/Exp, SWDGE partition
            # broadcast, v-half = hT*r.  The odd half finishes the pair:
            # hcat = elu(v)+1 = relu(v)+exp(min(v,0)).
            lnS = ep1.tile([1, I], F32, tag=f"ln{h % 2}", bufs=2)
            nc.scalar.activation(lnS[:], hT[D:D + 1, :], ACT_LN)
            r = ep1.tile([1, I], BF16, tag=f"r{h % 2}", bufs=2)
            nc.scalar.activation(r[:], lnS[:], ACT_EXP, scale=-1.0)
            if h % 2 == 0:
                rbc = ep1.tile([128, I], BF16, tag="rbc", bufs=2)
                v = ep1.tile([128, I], BF16, tag="v", bufs=2)
                ep_state["rbc"], ep_state["v"] = rbc, v
                half = slice(0, D)
            else:
                rbc, v = ep_state["rbc"], ep_state["v"]
                half = slice(D, 128)
            nc.gpsimd.partition_broadcast(rbc[half, :], r[:])
            if ep_v_gps:
                nc.gpsimd.tensor_mul(v[half, :], hT[0:D, :], rbc[half, :])
            else:
                nc.vector.tensor_tensor(out=v[half, :], in0=hT[0:D, :],
                                        in1=rbc[half, :], op=MULT)
            if h % 2 == 1:
                t = ep1.tile([128, I], BF16, tag="t", bufs=2)
                nc.vector.tensor_scalar_min(t[:], v[:], 0.0)
                nc.scalar.activation(t[:], t[:], ACT_EXP)
                nc.vector.scalar_tensor_tensor(
                    out=hcatT[:, h // 2, :], in0=v[:], scalar=0.0, in1=t[:],
                    op0=MAX, op1=ADD)

        # ---------------- layer 1 ----------------
        pending_ep = None   # deferred half-epilogue (software pipelining)
        for h in range(H):
            emit_phase0_head(h)
            head_modes = route[h * PAIRS_PER_HEAD:(h + 1) * PAIRS_PER_HEAD]
            need_f1rep = any(m in (2, 3, 4) for m in head_modes)
            need_B = any(m == 7 for m in head_modes)

            f1p = psB.tile([128, I], F32, tag="rep")
            for hf in range(I // 512):
                sl = slice(hf * 512, (hf + 1) * 512)
                nc.tensor.matmul(f1p[:, sl], lhsT=a1rep_s[:, h, :],
                                 rhs=xtl_s[:, sl])
            f1rep_s = B1_s = B2_s = None
            if need_f1rep:
                f1rep_s = ep2.tile([128, I], BF16, tag="f1rep")
                _copy_engine(nc, f1rep_eng, f1rep_s[:], f1p[:])
            if need_B:
                B1_s = ep2.tile([128, I], BF16, tag="B1")
                nc.scalar.activation(B1_s[:], f1p[:], ACT_EXP)
                B2_s = ep2.tile([128, I], BF16, tag="B2")
                nc.scalar.activation(B2_s[:], f1p[:], ACT_EXP, scale=ALPHA)

            hT = psC.tile([D + 1, I], F32, tag="acc")
            for jp in range(PAIRS_PER_HEAD):
                if jp == 2 and pending_ep is not None:
                    pending_ep()
                    pending_ep = None
                mode = head_modes[jp]
                subs = []
                for k in range(2):
                    jt = jp * 2 + k
                    subs.append((jt, mb_s[:, jt, :], mq_s[:, jt, :],
                                 fcol[:, h, jt, :], fexp1[:, h, jt, :],
                                 fexp2[:, h, jt, :]))
                p = _emit_pair(nc, work, workp, mode, subs,
                               (f1rep_s, B1_s, B2_s))
                for k in range(2):
                    jt = jp * 2 + k
                    for hf in range(I // 512):
                        sl = slice(hf * 512, (hf + 1) * 512)
                        nc.tensor.matmul(hT[:, sl],
                                         lhsT=whbuf[:, h, jt, :],
                                         rhs=p[:, k, sl],
                                         start=(jt == 0), stop=(jt == JT - 1))

            pending_ep = (lambda t_=hT, h_=h: emit_half_ep(t_, h_))
        if pending_ep is not None:
            pending_ep()
            pending_ep = None

        # ---------------- layer 2 projection + gather (bf16 payload) ------
        wh2loc = singles.tile([128, IC, C + 2], BF16)
        g1c = singles.tile([128, IC, 1], F32)
        gin = dram.tile([I, C + 2], BF16)
        for ic in range(IC):
            w2p = psA.tile([128, 4, D + 2], F32, tag="ph")
            for kt in range(KT):
                nc.tensor.matmul(
                    w2p[:, 0, 0:C + 2],
                    lhsT=hcatT[:, kt, ic * 128:(ic + 1) * 128],
                    rhs=woext_s[:, kt, :],
                    start=(kt == 0), stop=False)
            nc.tensor.matmul(w2p[:, 0, 0:C + 2], lhsT=ones_s[0:1, :],
                             rhs=wcorr_s[:], start=False, stop=True)
            nc.vector.tensor_copy(out=wh2loc[:, ic, :], in_=w2p[:, 0, 0:C + 2])
            nc.vector.tensor_copy(out=g1c[:, ic, :], in_=w2p[:, 0, 0:1])
            nc.sync.dma_start(out=gin[ic * 128:(ic + 1) * 128, :],
                              in_=wh2loc[:, ic, :])

        gout = dram.tile([N, C + 2], BF16)
        if with_collective:
            nc.gpsimd.collective_compute(
                "AllGather", mybir.AluOpType.bypass,
                replica_groups=REPLICA_GROUPS,
                ins=[gin.opt()], outs=[gout.opt()])
        else:  # timing-model variant: fake the exchange with two local copies
            nc.sync.dma_start(out=gout[0:I, :], in_=gin[:])
            nc.sync.dma_start(out=gout[I:N, :], in_=gin[:])

        # g1 row (local queries) -> replicated [128, I]
        g1rowp = psB.tile([128, I], F32, tag="rep")
        for ic in range(IC):
            nc.tensor.transpose(g1rowp[0:1, ic * 128:(ic + 1) * 128],
                                in_=g1c[:, ic, :], identity=ident_s[:])
        g1row_s = epL2.tile([1, I], BF16, tag="g1row")
        nc.scalar.activation(g1row_s[:], g1rowp[0:1, :], ACT_COPY)
        g1rp = psB.tile([128, I], F32, tag="rep")
        for hf in range(I // 512):
            sl = slice(hf * 512, (hf + 1) * 512)
            nc.tensor.matmul(g1rp[:, sl], lhsT=ones_s[0:1, :],
                             rhs=g1row_s[0:1, sl])
        l2_modes = route[H * PAIRS_PER_HEAD:]
        g1rep_s = B1L2 = B2L2 = None
        if any(m in (2, 3, 4) for m in l2_modes):
            g1rep_s = singles.tile([128, I], BF16)
            nc.vector.tensor_copy(out=g1rep_s[:], in_=g1rp[:])
        if any(m == 7 for m in l2_modes):
            B1L2 = singles.tile([128, I], BF16)
            nc.scalar.activation(B1L2[:], g1rp[:], ACT_EXP)
            B2L2 = singles.tile([128, I], BF16)
            nc.scalar.activation(B2L2[:], g1rp[:], ACT_EXP, scale=ALPHA)

        # gathered rows: [g1, g2, Wh2(32)] bf16 + ones col, direct DMA
        wh2gr = singles.tile([128, JT, C + 3], BF16)
        nc.gpsimd.memset(wh2gr[:, :, C + 2:C + 3], 1.0)
        for jt in range(JT):
            nc.sync.dma_start(out=wh2gr[:, jt, 0:C + 2],
                              in_=gout[jt * 128:(jt + 1) * 128, :])
        # per-4jt g2 cols (f32 for bias/scalar reads) + their exps
        g2c = singles.tile([128, JT, 1], F32)
        its1 = singles.tile([128, JT, 1], F32)
        its2 = singles.tile([128, JT, 1], F32)
        for jg in range(JT // 4):
            s4 = slice(jg * 4, (jg + 1) * 4)
            nc.vector.tensor_copy(out=g2c[:, s4, :], in_=wh2gr[:, s4, 1:2])
            nc.scalar.activation(its1[:, s4, :], g2c[:, s4, :], ACT_EXP)
            nc.scalar.activation(its2[:, s4, :], g2c[:, s4, :], ACT_EXP,
                                 scale=ALPHA)

        # ---------------- layer 2 attention ----------------
        o2T = psC.tile([D + 1, I], F32, tag="acc")
        for jp in range(PAIRS_PER_HEAD):
            mode = l2_modes[jp]
            subs = []
            for k in range(2):
                jt = jp * 2 + k
                subs.append((jt, mb_s[:, jt, :], mq_s[:, jt, :],
                             g2c[:, jt, :], its1[:, jt, :],
                             its2[:, jt, :]))
            p = _emit_pair(nc, work, workp, mode, subs,
                           (g1rep_s, B1L2, B2L2))
            for k in range(2):
                jt = jp * 2 + k
                for hf in range(I // 512):
                    sl = slice(hf * 512, (hf + 1) * 512)
                    nc.tensor.matmul(o2T[0:C + 1, sl],
                                     lhsT=wh2gr[:, jt, 2:C + 3],
                                     rhs=p[:, k, sl],
                                     start=(jt == 0), stop=(jt == JT - 1))

        # ---------------- finalize (transposed: per-query reciprocal) -----
        o2s = epL2.tile([C + 1, I], F32, tag="o2s")
        nc.vector.tensor_copy(out=o2s[:], in_=o2T[0:C + 1, :])
        for k in range(IC):
            ck = slice(k * 128, (k + 1) * 128)
            ofp = psA.tile([128, 4, D + 2], F32, tag="ph")
            nc.tensor.transpose(ofp[:, 0, 0:C], in_=o2s[0:C, ck],
                                identity=ident_s[0:C, 0:C])
            nc.tensor.transpose(ofp[:, 0, C + 2:C + 3], in_=o2s[C:C + 1, ck],
                                identity=ident_s[C:C + 1, C:C + 1])
            s2t = ep2.tile([128, 1], F32, tag="s2t")
            nc.vector.reciprocal(s2t[:], ofp[:, 0, C + 2:C + 3])
            ofs = ep2.tile([128, C], F32, tag="ofs")
            nc.vector.tensor_scalar_mul(ofs[:], ofp[:, 0, 0:C], s2t[:])
            nc.sync.dma_start(out=outp_d.ap()[k * 128:(k + 1) * 128, :],
                              in_=ofs[:])


# --------------------------------------------------------------------------
# host side
# --------------------------------------------------------------------------

def shard_inputs(x, adj, W, a1, a2, Wo, ao1, ao2):
    x = np.asarray(x, np.float32)
    adj = np.asarray(adj)
    W = np.asarray(W, np.float32)
    a1 = np.asarray(a1, np.float32)
    a2 = np.asarray(a2, np.float32)
    Wo = np.asarray(Wo, np.float32)
    ao1 = np.asarray(ao1, np.float32)
    ao2 = np.asarray(ao2, np.float32)
    BF = ml_dtypes.bfloat16

    wvec1 = np.einsum("hfd,hd->hf", W, a1)          # [H, F]
    wvec2 = np.einsum("hfd,hd->hf", W, a2)
    wext = np.concatenate([W, wvec1[:, :, None], wvec2[:, :, None]],
                          axis=2).astype(BF)
    a1rep = np.repeat(wvec1[:, :, None], 128, axis=2).astype(BF)
    wo1 = Wo @ ao1                                   # [512]
    wo2 = Wo @ ao2
    woflat = np.concatenate([wo1[:, None], wo2[:, None], Wo], 1)  # [512, 34]
    woext = woflat.reshape(KT, 128, C + 2).astype(BF)
    wcorr = (-woflat.sum(0))[None, :].astype(BF)
    ident = np.eye(128, dtype=np.float32)

    in_maps = []
    for c in range(N_CORES):
        b, half = c // 2, c % 2
        i0 = half * I
        xt = np.ascontiguousarray(x[b].T).astype(BF)   # [F, N]
        xtl = np.ascontiguousarray(xt[:, i0:i0 + I])
        adjt = adj[b, i0:i0 + I, :].T                # [N, I] = (j, i)
        mb = np.where(adjt > 0, np.float32(0.0), np.float32(-BIG))
        mb = np.ascontiguousarray(mb.reshape(JT, 128, I)).astype(BF)
        mq = np.where(adjt > 0, np.float32(1.0), np.float32(0.0))
        mq = np.ascontiguousarray(mq.reshape(JT, 128, I)).astype(BF)
        in_maps.append({
            "xt": xt, "xtl": xtl, "mb": mb, "mq": mq, "wext": wext,
            "a1rep": a1rep, "woext": woext, "wcorr": wcorr, "ident": ident,
        })
    return in_maps


# Engine routing chosen by cost-model sweep (TimelineSim).
DEFAULT_CFG = {"p4": 24, "p7": 33, "f1rep_act": True, "rbc_act": True}

_CACHE = {}


def _program():
    if "nc" not in _CACHE:
        _CACHE["nc"] = build_program(with_collective=True, cfg=DEFAULT_CFG)
    return _CACHE["nc"]


def kernel(**inputs):
    nc = _program()
    in_maps = shard_inputs(**inputs)
    res = run_bass_kernel_spmd(nc, in_maps, list(range(N_CORES)))
    _CACHE["last_results"] = res
    out = np.empty((B, N, C), np.float32)
    for c in range(N_CORES):
        b, half = c // 2, c % 2
        out[b, half * I:(half + 1) * I, :] = res.results[c]["outp"]
    return out


# revision 35
# speedup vs baseline: 1.3048x; 1.2130x over previous
"""GAT forward (2-layer graph attention, B=4 N=2048 F=128 H=8 D=64 C=32)
as a Bass/Tile SPMD kernel on 8 Trainium2 NeuronCores.

Sharding: core c -> (batch b=c//2, query-row half c%2).  Each core computes
attention for its 1024 query rows over all 2048 keys for all 8 heads
(layer 1) and for the output head (layer 2).  The only cross-core exchange
is a 2-rank AllGather of the layer-2 projections [g1|g2|Wh2] ([1024,34] f32)
within each (2b, 2b+1) pair.

Layout: attention logits are built TRANSPOSED, e^T[j (keys) = partitions,
i (queries) = free], so the PV matmul needs no operand transposes
(lhsT = Wh[j,d] stationary, rhs = p[j,i] moving, out = h^T[d,i]) and
softmax row sums come from a ones-column appended to Wh (PSUM row D).

The N^2 elementwise work (mask+f1+f2, leaky-relu, exp over 144 [128,1024]
logit tiles) is the bottleneck; everything runs in bf16 (2x DVE tensor-
tensor throughput; ACT is dtype-independent) and every logit PAIR of tiles
is routed down one of three pipelines to saturate ACT+DVE+GPSIMD jointly:

  P3 : u = mb + f1rep       (DVE tensor_tensor, bf16 2x)
       z = Prelu(u + f2col) (ACT, per-sub bias)     p = Exp(z)  (ACT)
  P4 : same but the combine runs on GPSIMD tensor_add
  P7 : exp(prelu(u)) == max(exp(u), exp(0.2u)) and exp(u) factors rank-1:
       q1 = (mb01 * exp(f2_j)) * exp(f1_i)   (DVE STT mult,mult)
       q2 = (mb01 * exp(.2 f2)) * exp(.2 f1) (DVE STT)
       p  = max(q1, q2)                      (GPSIMD tensor_max; no ACT!)

Softmax division is deferred to the per-head-pair epilogue (1/S via Ln/Exp
of the row-sum), fused with ELU via elu(v)+1 = relu(v) + exp(min(v,0)),
the +1 folded into a rank-1 correction matmul of the layer-2 projection.
"""

import numpy as np
import ml_dtypes

import concourse.bass as bass
import concourse.tile as tile
from concourse import mybir
from concourse.bass_utils import run_bass_kernel_spmd

F32 = mybir.dt.float32
F32R = mybir.dt.float32r
BF16 = mybir.dt.bfloat16

B, N, F, H, D, C = 4, 2048, 128, 8, 64, 32
I = N // 2          # query rows per core
JT = N // 128       # key tiles
IC = I // 128       # query-row 128-chunks per core
KT = (H * D) // 128 # hidden-dim 128-chunks
ALPHA = 0.2
BIG = 1e15          # mask bias; also dominates A*B in the factored path
N_CORES = 8
REPLICA_GROUPS = [[0, 1], [2, 3], [4, 5], [6, 7]]

ADD = mybir.AluOpType.add
MAX = mybir.AluOpType.max
MULT = mybir.AluOpType.mult
# NOTE: hardware "Lrelu" has a fixed 0.01 slope and ignores alpha;
# "Prelu" honors alpha (verified on HW) — it is the configurable leaky relu.
ACT_LRELU = mybir.ActivationFunctionType.Prelu
ACT_EXP = mybir.ActivationFunctionType.Exp
ACT_LN = mybir.ActivationFunctionType.Ln
ACT_COPY = mybir.ActivationFunctionType.Copy

N_HEADS_ALL = H + 1          # 8 layer-1 heads + the layer-2 output head
PAIRS_PER_HEAD = JT // 2     # 8
N_PAIRS = N_HEADS_ALL * PAIRS_PER_HEAD  # 72


def _split_multiwaits(nc):
    """Pinned walrus accepts only one sync-wait per instruction; Tile's exit
    drain (and occasionally others) carries several.  Hoist extras onto
    single-wait Drains on the same engine immediately before the owner."""
    n_fixed = 0
    for fn in nc.m.functions:
        for bb in fn.blocks:
            for name in [i.name for i in bb.instructions]:
                idx = [i.name for i in bb.instructions].index(name)
                inst = bb.instructions[idx]
                si = inst.sync_info
                if si is None or len(si.on_wait) <= 1:
                    continue
                waits = list(si.on_wait)
                for k, w in enumerate(waits[:-1]):
                    nd = mybir.InstDrain(
                        name=f"waitfix-{inst.name}-{k}", ins=[], outs=[])
                    nd.engine = inst.engine
                    nd.sync_info = mybir.SyncInfo(on_wait=[w], on_update=[])
                    nc.register_instruction(nd, overwrite=True)
                    bb.instructions.insert(idx + k, nd)
                inst.sync_info = mybir.SyncInfo(
                    on_wait=waits[-1:], on_update=list(si.on_update))
                n_fixed += 1
    return n_fixed


def _make_route(cfg):
    """Per-pair pipeline assignment:
      3 (DVE-add + ACT prelu/exp), 4 (GPS-add + ACT prelu/exp),
      7 (factored rank-1 exps on DVE + GPS max, no ACT),
      2 (DVE STT-add + DVE prelu + ACT exp only).
    Counts spread across the 9 'heads' (8 L1 + L2); p7 optionally
    concentrated into few heads (fewer exp(f1)-prep ops on ACT)."""
    if "route" in cfg:                      # explicit per-pair override
        route = list(cfg["route"])
        assert len(route) == N_PAIRS
        return route
    n4 = int(cfg.get("p4", 24))
    n7 = int(cfg.get("p7", 33))
    n2 = int(cfg.get("p2", 0))
    n3 = N_PAIRS - n4 - n7 - n2
    assert n3 >= 0
    conc = int(cfg.get("p7_conc", 0))  # 0 = spread; 1 = concentrate P7
    per_head = [[0, 0, 0, 0] for _ in range(N_HEADS_ALL)]  # [n3, n4, n7, n2]
    if conc:
        # fill P7 into heads round-robin starting at head 1, PAIRS_PER_HEAD
        # per head; distribute the rest evenly over remaining slots
        order = [1, 3, 5, 7, 8, 0, 2, 4, 6]
        left7 = n7
        for hh in order:
            take = min(left7, PAIRS_PER_HEAD)
            per_head[hh][2] = take
            left7 -= take
            if left7 == 0:
                break
        slots = [PAIRS_PER_HEAD - per_head[hh][2] for hh in range(N_HEADS_ALL)]
        for idx, cnt in ((1, n4), (3, n2), (0, n3)):
            left = cnt
            while left > 0:
                done = True
                for hh in range(N_HEADS_ALL):
                    used = sum(per_head[hh])
                    if used < PAIRS_PER_HEAD and left > 0:
                        per_head[hh][idx] += 1
                        left -= 1
                        done = False
                if done:
                    break
    else:
        for idx, cnt in enumerate((n3, n4, n7, n2)):
            base, rem = divmod(cnt, N_HEADS_ALL)
            for hh in range(N_HEADS_ALL):
                per_head[hh][idx] = base + (1 if hh < rem else 0)
        for hh in range(N_HEADS_ALL):
            while sum(per_head[hh]) > PAIRS_PER_HEAD:
                per_head[hh][int(np.argmax(per_head[hh]))] -= 1
            while sum(per_head[hh]) < PAIRS_PER_HEAD:
                per_head[hh][int(np.argmin(per_head[hh]))] += 1
    route = []
    for hh in range(N_HEADS_ALL):
        c3, c4, c7, c2 = per_head[hh]
        pool = [7] * c7 + [4] * c4 + [3] * c3 + [2] * c2
        mix, lo, hi = [], 0, len(pool) - 1
        toggle = True
        while lo <= hi:
            if toggle:
                mix.append(pool[lo]); lo += 1
            else:
                mix.append(pool[hi]); hi -= 1
            toggle = not toggle
        route.extend(mix)
    return route


def build_program(with_collective=True, cfg=None, repeat=1):
    cfg = dict(cfg or {})
    route = _make_route(cfg)

    nc = bass.Bass("TRN2", target_bir_lowering=False, debug=False,
                   enable_asserts=False, num_devices=N_CORES)

    xt_d = nc.dram_tensor("xt", [F, N], BF16, kind="ExternalInput")
    xtl_d = nc.dram_tensor("xtl", [F, I], BF16, kind="ExternalInput")
    mb_d = nc.dram_tensor("mb", [JT, 128, I], BF16, kind="ExternalInput")
    wext_d = nc.dram_tensor("wext", [H, F, D + 2], BF16, kind="ExternalInput")
    a1rep_d = nc.dram_tensor("a1rep", [H, F, 128], BF16, kind="ExternalInput")
    woext_d = nc.dram_tensor("woext", [KT, 128, C + 2], F32,
                             kind="ExternalInput")
    wcorr_d = nc.dram_tensor("wcorr", [1, C + 2], F32, kind="ExternalInput")
    ident_d = nc.dram_tensor("ident", [128, 128], F32, kind="ExternalInput")
    outp_d = nc.dram_tensor("outp", [I, C], F32, kind="ExternalOutput")

    with tile.TileContext(nc) as tc:
        if repeat > 1:
            def body(iv, unroll=None):
                _build_body(nc, tc, xt_d, xtl_d, mb_d, wext_d, a1rep_d,
                            woext_d, wcorr_d, ident_d, outp_d,
                            with_collective, route, cfg)
            with tc.For_i(0, repeat, 1) as iv:
                body(iv)
        else:
            _build_body(nc, tc, xt_d, xtl_d, mb_d, wext_d, a1rep_d,
                        woext_d, wcorr_d, ident_d, outp_d,
                        with_collective, route, cfg)
    _split_multiwaits(nc)
    return nc


def _emit_pair(nc, work, workp, mode, subs, pair_args):
    """Emit one logit pair.  subs = [(jt, mb_ap, mq_ap, f2col_ap, e1col_ap,
    e2col_ap)]; pair_args = (f1rep, B1, B2)."""
    f1rep, B1, B2 = pair_args
    if mode == 7:
        q1 = work.tile([128, 2, I], BF16, tag="q1", bufs=3)
        q2 = work.tile([128, 2, I], BF16, tag="q2", bufs=3)
        for k, (jt, mb_ap, f2c, e1c, e2c) in enumerate(subs):
            nc.vector.scalar_tensor_tensor(
                out=q1[:, k, :], in0=B1[:], scalar=e1c, in1=mb_ap,
                op0=MULT, op1=ADD)
            nc.vector.scalar_tensor_tensor(
                out=q2[:, k, :], in0=B2[:], scalar=e2c, in1=mb_ap,
                op0=MULT, op1=ADD)
        p = workp.tile([128, 2, I], BF16, tag="p")
        nc.vector.tensor_tensor(out=p[:], in0=q1[:], in1=q2[:], op=MAX)
        nc.vector.tensor_scalar_max(p[:], p[:], 0.0)
        return p
    if mode == 2:
        # combine WITH f2 (STT) so prelu can run on DVE; exp is the only ACT op
        u = work.tile([128, 2, I], BF16, tag="u")
        for k, (jt, mb_ap, f2c, e1c, e2c) in enumerate(subs):
            nc.vector.scalar_tensor_tensor(
                out=u[:, k, :], in0=mb_ap, scalar=f2c, in1=f1rep[:],
                op0=ADD, op1=ADD)
        w = work.tile([128, 2, I], BF16, tag="q1", bufs=3)
        nc.vector.tensor_scalar_mul(w[:], u[:], ALPHA)     # bf16 4x
        nc.vector.tensor_tensor(out=u[:], in0=u[:], in1=w[:], op=MAX)
        p = workp.tile([128, 2, I], BF16, tag="p")
        nc.scalar.activation(p[:], u[:], ACT_EXP)
        return p
    u = work.tile([128, 2, I], BF16, tag="u")
    for k, (jt, mb_ap, f2c, e1c, e2c) in enumerate(subs):
        if mode == 4:
            nc.gpsimd.tensor_add(u[:, k, :], mb_ap, f1rep[:])
        else:
            nc.vector.tensor_tensor(out=u[:, k, :], in0=mb_ap, in1=f1rep[:],
                                    op=ADD)
    for k, (jt, mb_ap, f2c, e1c, e2c) in enumerate(subs):
        nc.scalar.activation(u[:, k, :], u[:, k, :], ACT_LRELU,
                             bias=f2c, alpha=ALPHA)
    p = workp.tile([128, 2, I], BF16, tag="p")
    nc.scalar.activation(p[:], u[:], ACT_EXP)
    return p


def _copy_engine(nc, eng, out, in_):
    if eng == "act":
        nc.scalar.activation(out, in_, ACT_COPY)
    elif eng == "gps":
        nc.gpsimd.tensor_copy(out=out, in_=in_)
    else:
        nc.vector.tensor_copy(out=out, in_=in_)


def _build_body(nc, tc, xt_d, xtl_d, mb_d, wext_d, a1rep_d, woext_d,
                wcorr_d, ident_d, outp_d, with_collective, route, cfg):
    from contextlib import ExitStack
    ctx = ExitStack()
    f1rep_eng = cfg.get("f1rep_eng", "dve")
    rbc_eng = cfg.get("rbc_eng", "act")
    assert f1rep_eng != "gps" and rbc_eng != "gps"  # GPSIMD cannot read PSUM
    
    ph0_engs = cfg.get("ph0_engs", ("act", "dve"))
    ep_v_gps = False  # GPSIMD cannot read PSUM (hT)
    with ctx:
        singles = ctx.enter_context(tc.tile_pool(name="singles", bufs=1))
        psA = ctx.enter_context(tc.tile_pool(name="psA", bufs=2, space="PSUM"))
        psB = ctx.enter_context(tc.tile_pool(name="psB", bufs=1, space="PSUM"))
        psC = ctx.enter_context(tc.tile_pool(name="psC", bufs=2, space="PSUM"))
        dram = ctx.enter_context(tc.tile_pool(name="dram", bufs=1,
                                              space="DRAM"))

        # ---------------- persistent loads ----------------
        mb_s = singles.tile([128, JT, I], BF16)
        nc.sync.dma_start(out=mb_s[:, 0:2, :],
                          in_=mb_d.ap()[0:2].rearrange("jt p i -> p jt i"))
        xtl_s = singles.tile([F, I], BF16)
        nc.sync.dma_start(out=xtl_s[:], in_=xtl_d.ap())
        a1rep_s = singles.tile([F, H, 128], BF16)
        nc.sync.dma_start(out=a1rep_s[:],
                          in_=a1rep_d.ap().rearrange("h f e -> f h e"))
        wcorr_s = singles.tile([1, C + 2], F32)
        nc.sync.dma_start(out=wcorr_s[:], in_=wcorr_d.ap())
        ident_s = singles.tile([128, 128], F32)
        nc.sync.dma_start(out=ident_s[:], in_=ident_d.ap())
        woext_raw = singles.tile([128, KT, C + 2], F32)
        nc.sync.dma_start(out=woext_raw[:],
                          in_=woext_d.ap().rearrange("k f e -> f k e"))
        woext_s = singles.tile([128, KT, C + 2], F32R)
        nc.vector.tensor_copy(out=woext_s[:], in_=woext_raw[:])

        ones_s = singles.tile([1, 128], BF16)
        nc.gpsimd.memset(ones_s[:], 1.0)
        onesf_s = singles.tile([1, 128], F32)
        nc.gpsimd.memset(onesf_s[:], 1.0)

        whbuf = singles.tile([128, H, JT, D + 1], BF16)
        nc.gpsimd.memset(whbuf[:, :, :, D:D + 1], 1.0)
        fcol = singles.tile([128, H, JT, 1], F32)
        fexp1 = singles.tile([128, H, JT, 1], F32)
        fexp2 = singles.tile([128, H, JT, 1], F32)
        hcatT = singles.tile([128, KT, I], F32R)

        xt_s = singles.tile([F, N], BF16)
        nc.sync.dma_start(out=xt_s[:], in_=xt_d.ap())
        wext_s = singles.tile([F, H, D + 2], BF16)
        nc.sync.dma_start(out=wext_s[:],
                          in_=wext_d.ap().rearrange("h f e -> f h e"))
        for j0 in (2, 6, 10):
            j1 = j0 + 4 if j0 < 10 else JT
            nc.sync.dma_start(
                out=mb_s[:, j0:j1, :],
                in_=mb_d.ap()[j0:j1].rearrange("jt p i -> p jt i"))

        work = ctx.enter_context(tc.tile_pool(name="work", bufs=4))
        workp = ctx.enter_context(tc.tile_pool(name="workp", bufs=4))
        ep1 = ctx.enter_context(tc.tile_pool(name="ep1", bufs=1))
        ep2 = ctx.enter_context(tc.tile_pool(name="ep2", bufs=2))
        epL2 = ctx.enter_context(tc.tile_pool(name="epL2", bufs=1))

        def emit_phase0_head(h):
            # Wh tiles + f columns for head h (emitted per-head so the
            # copies overlap the previous head's logit work)
            for jg in range(JT // 4):
                whp = psA.tile([128, 4, D + 2], F32, tag="ph")
                for k in range(4):
                    jt = jg * 4 + k
                    nc.tensor.matmul(whp[:, k, :],
                                     lhsT=xt_s[:, jt * 128:(jt + 1) * 128],
                                     rhs=wext_s[:, h, :])
                dst = whbuf[:, h, jg * 4:(jg + 1) * 4, 0:D]
                _copy_engine(nc, ph0_engs[jg % len(ph0_engs)],
                             dst, whp[:, :, 0:D])
                nc.vector.tensor_copy(
                    out=fcol[:, h, jg * 4:(jg + 1) * 4, :],
                    in_=whp[:, :, D + 1:D + 2])
            nc.scalar.activation(fexp1[:, h], fcol[:, h], ACT_EXP)
            nc.scalar.activation(fexp2[:, h], fcol[:, h], ACT_EXP,
                                 scale=ALPHA)

        ep_state = {}

        def emit_half_ep(hT, h):
            # per-head half-epilogue: rinv = 1/S via DVE reciprocal, SWDGE
            # partition broadcast, v-half = hT*rinv.  The odd half finishes:
            # hcat = elu(v)+1 = relu(v)+exp(min(v,0)).
            rinv = ep1.tile([1, I], F32, tag=f"ri{h % 2}", bufs=1)
            if cfg.get("dbg_ep_lnexp"):
                lnS_ = ep1.tile([1, I], F32, tag=f"ln{h % 2}", bufs=2)
                nc.scalar.activation(lnS_[:], hT[D:D + 1, :], ACT_LN)
                nc.scalar.activation(rinv[:], lnS_[:], ACT_EXP, scale=-1.0)
            else:
                nc.vector.reciprocal(rinv[:], hT[D:D + 1, :])
            if h % 2 == 0:
                v = ep1.tile([128, I], BF16, tag="v", bufs=2)
                ep_state["v"] = v
                half = slice(0, D)
            else:
                v = ep_state["v"]
                half = slice(D, 128)
            rbp = psB.tile([128, I], F32, tag="rep")
            for hf_ in range(I // 512):
                sl_ = slice(hf_ * 512, (hf_ + 1) * 512)
                nc.tensor.matmul(rbp[0:D, sl_], lhsT=onesf_s[0:1, 0:D],
                                 rhs=rinv[0:1, sl_])
            # rbc half kept at base partition 0: DVE inputs must share bases
            rbc = ep1.tile([D, I], F32, tag=f"rb{h % 2}", bufs=1)
            _copy_engine(nc, rbc_eng, rbc[:], rbp[0:D, :])
            nc.vector.tensor_tensor(out=v[half, :], in0=hT[0:D, :],
                                    in1=rbc[:], op=MULT)
            if h % 2 == 1:
                t = ep1.tile([128, I], BF16, tag="t", bufs=2)
                nc.vector.tensor_scalar_min(t[:], v[:], 0.0)
                nc.scalar.activation(t[:], t[:], ACT_EXP)
                nc.vector.scalar_tensor_tensor(
                    out=hcatT[:, h // 2, :], in0=v[:], scalar=0.0, in1=t[:],
                    op0=MAX, op1=ADD)

        def emit_head_prep(h):
            emit_phase0_head(h)
            head_modes = route[h * PAIRS_PER_HEAD:(h + 1) * PAIRS_PER_HEAD]
            need_f1rep = any(m in (2, 3, 4) for m in head_modes)
            need_B = any(m == 7 for m in head_modes)
            f1p = psB.tile([128, I], F32, tag="rep")
            for hf in range(I // 512):
                sl = slice(hf * 512, (hf + 1) * 512)
                nc.tensor.matmul(f1p[:, sl], lhsT=a1rep_s[:, h, :],
                                 rhs=xtl_s[:, sl])
            f1rep_s = B1_s = B2_s = None
            if need_f1rep:
                f1rep_s = ep2.tile([128, I], BF16, tag="f1rep")
                _copy_engine(nc, f1rep_eng, f1rep_s[:], f1p[:])
            if need_B:
                B1_s = ep2.tile([128, I], BF16, tag="B1")
                nc.scalar.activation(B1_s[:], f1p[:], ACT_EXP)
                B2_s = ep2.tile([128, I], BF16, tag="B2")
                nc.scalar.activation(B2_s[:], f1p[:], ACT_EXP, scale=ALPHA)
            return f1rep_s, B1_s, B2_s

        # ---------------- layer 1 ----------------
        pending_ep = None   # deferred half-epilogue (software pipelining)
        preps = emit_head_prep(0)
        for h in range(H):
            head_modes = route[h * PAIRS_PER_HEAD:(h + 1) * PAIRS_PER_HEAD]
            cur = preps
            hT = psC.tile([D + 1, I], F32, tag="acc")
            for jp in range(PAIRS_PER_HEAD):
                if jp == int(cfg.get("ep_defer", 2)) and pending_ep is not None:
                    pending_ep()
                    pending_ep = None
                if jp == 4 and h + 1 < H:
                    preps = emit_head_prep(h + 1)
                mode = head_modes[jp]
                subs = []
                for k in range(2):
                    jt = jp * 2 + k
                    subs.append((jt, mb_s[:, jt, :],
                                 fcol[:, h, jt, :], fexp1[:, h, jt, :],
                                 fexp2[:, h, jt, :]))
                p = _emit_pair(nc, work, workp, mode, subs, cur)
                for k in range(2):
                    jt = jp * 2 + k
                    for hf in range(I // 512):
                        sl = slice(hf * 512, (hf + 1) * 512)
                        nc.tensor.matmul(hT[:, sl],
                                         lhsT=whbuf[:, h, jt, :],
                                         rhs=p[:, k, sl],
                                         start=(jt == 0), stop=(jt == JT - 1))

            pending_ep = (lambda t_=hT, h_=h: emit_half_ep(t_, h_))
        if pending_ep is not None:
            pending_ep()
            pending_ep = None

        # ---------------- layer 2 projection + gather (bf16 payload) ------
        wh2loc = singles.tile([128, IC, C + 2], F32)
        gin = dram.tile([I, C + 2], F32)
        for ic in range(IC):
            w2p = psA.tile([128, 4, D + 2], F32, tag="ph")
            for kt in range(KT):
                nc.tensor.matmul(
                    w2p[:, 0, 0:C + 2],
                    lhsT=hcatT[:, kt, ic * 128:(ic + 1) * 128],
                    rhs=woext_s[:, kt, :],
                    start=(kt == 0), stop=False)
            nc.tensor.matmul(w2p[:, 0, 0:C + 2], lhsT=onesf_s[0:1, :],
                             rhs=wcorr_s[:], start=False, stop=True)
            nc.vector.tensor_copy(out=wh2loc[:, ic, :], in_=w2p[:, 0, 0:C + 2])
        nc.sync.dma_start(
            out=gin.rearrange("(ic p) c -> p ic c", p=128),
            in_=wh2loc[:])

        gout = dram.tile([N, C + 2], F32)
        if with_collective:
            nc.gpsimd.collective_compute(
                "AllGather", mybir.AluOpType.bypass,
                replica_groups=REPLICA_GROUPS,
                ins=[gin.opt()], outs=[gout.opt()])
        else:  # timing-model variant: fake the exchange with two local copies
            nc.sync.dma_start(out=gout[0:I, :], in_=gin[:])
            nc.sync.dma_start(out=gout[I:N, :], in_=gin[:])

        # g1 row (local queries) -> replicated [128, I]
        g1rowp = psB.tile([128, I], F32, tag="rep")
        for ic in range(IC):
            nc.tensor.transpose(g1rowp[0:1, ic * 128:(ic + 1) * 128],
                                in_=wh2loc[:, ic, 0:1], identity=ident_s[:])
        g1row_s = epL2.tile([1, I], BF16, tag="g1row")
        nc.scalar.activation(g1row_s[:], g1rowp[0:1, :], ACT_COPY)
        g1rp = psB.tile([128, I], F32, tag="rep")
        for hf in range(I // 512):
            sl = slice(hf * 512, (hf + 1) * 512)
            nc.tensor.matmul(g1rp[:, sl], lhsT=ones_s[0:1, :],
                             rhs=g1row_s[0:1, sl])
        l2_modes = route[H * PAIRS_PER_HEAD:]
        g1rep_s = B1L2 = B2L2 = None
        if any(m in (2, 3, 4) for m in l2_modes):
            g1rep_s = singles.tile([128, I], BF16)
            nc.vector.tensor_copy(out=g1rep_s[:], in_=g1rp[:])
        if any(m == 7 for m in l2_modes):
            B1L2 = singles.tile([128, I], BF16)
            nc.scalar.activation(B1L2[:], g1rp[:], ACT_EXP)
            B2L2 = singles.tile([128, I], BF16)
            nc.scalar.activation(B2L2[:], g1rp[:], ACT_EXP, scale=ALPHA)

        # gathered rows: [g1, g2, Wh2(32)] f32 staged, bf16 for the PV lhsT
        wh2tmp = singles.tile([128, JT, C + 2], F32)
        nc.sync.dma_start(
            out=wh2tmp[:],
            in_=gout.rearrange("(jt p) c -> p jt c", p=128))
        wh2gr = singles.tile([128, JT, C + 3], BF16)
        nc.gpsimd.memset(wh2gr[:, :, C + 2:C + 3], 1.0)
        its1 = singles.tile([128, JT, 1], F32)
        its2 = singles.tile([128, JT, 1], F32)
        for jg in range(JT // 4):
            s4 = slice(jg * 4, (jg + 1) * 4)
            nc.vector.tensor_copy(out=wh2gr[:, s4, 0:C + 2],
                                  in_=wh2tmp[:, s4, :])
            nc.scalar.activation(its1[:, s4, :], wh2tmp[:, s4, 1:2], ACT_EXP)
            nc.scalar.activation(its2[:, s4, :], wh2tmp[:, s4, 1:2], ACT_EXP,
                                 scale=ALPHA)

        # ---------------- layer 2 attention ----------------
        o2T = psC.tile([D + 1, I], F32, tag="acc")
        for jp in range(PAIRS_PER_HEAD):
            mode = l2_modes[jp]
            subs = []
            for k in range(2):
                jt = jp * 2 + k
                subs.append((jt, mb_s[:, jt, :],
                             wh2tmp[:, jt, 1:2], its1[:, jt, :],
                             its2[:, jt, :]))
            p = _emit_pair(nc, work, workp, mode, subs,
                           (g1rep_s, B1L2, B2L2))
            for k in range(2):
                jt = jp * 2 + k
                for hf in range(I // 512):
                    sl = slice(hf * 512, (hf + 1) * 512)
                    nc.tensor.matmul(o2T[0:C + 1, sl],
                                     lhsT=wh2gr[:, jt, 2:C + 3],
                                     rhs=p[:, k, sl],
                                     start=(jt == 0), stop=(jt == JT - 1))

        # ---------------- finalize (transposed: per-query reciprocal) -----
        if cfg.get("dbg_simple_fin"):
            r2ln = epL2.tile([1, I], F32, tag="lnS2")
            nc.scalar.activation(r2ln[:], o2T[C:C + 1, :], ACT_LN)
            r2 = epL2.tile([1, I], BF16, tag="r2")
            nc.scalar.activation(r2[:], r2ln[:], ACT_EXP, scale=-1.0)
            rbc2p = psB.tile([128, I], F32, tag="rep")
            for hf in range(I // 512):
                sl = slice(hf * 512, (hf + 1) * 512)
                nc.tensor.matmul(rbc2p[0:C, sl], lhsT=ones_s[0:1, 0:C],
                                 rhs=r2[0:1, sl])
            rbc2_s = epL2.tile([C, I], F32, tag="rbc2")
            nc.vector.tensor_copy(out=rbc2_s[:], in_=rbc2p[0:C, :])
            oT_s = epL2.tile([C, I], F32, tag="oT")
            nc.vector.tensor_tensor(out=oT_s[:], in0=o2T[0:C, :],
                                    in1=rbc2_s[:], op=MULT)
            for k in range(IC):
                ofp = psA.tile([128, 4, D + 2], F32, tag="ph")
                nc.tensor.transpose(ofp[:, 0, 0:C],
                                    in_=oT_s[:, k * 128:(k + 1) * 128],
                                    identity=ident_s[0:C, 0:C])
                ofs = ep2.tile([128, C], F32, tag="ofs")
                nc.vector.tensor_copy(out=ofs[:], in_=ofp[:, 0, 0:C])
                nc.sync.dma_start(out=outp_d.ap()[k * 128:(k + 1) * 128, :],
                                  in_=ofs[:])
        else:
            o2s = epL2.tile([C, I], F32, tag="o2s")
            nc.vector.tensor_copy(out=o2s[:], in_=o2T[0:C, :])
            o2r = epL2.tile([1, I], F32, tag="o2r")
            nc.vector.tensor_copy(out=o2r[:], in_=o2T[C:C + 1, :])
            for k in range(IC):
                ck = slice(k * 128, (k + 1) * 128)
                ofp = psA.tile([128, 4, D + 2], F32, tag="ph")
                nc.tensor.transpose(ofp[:, 0, 0:C], in_=o2s[:, ck],
                                    identity=ident_s[0:C, 0:C])
                ofq = psA.tile([128, 4, D + 2], F32, tag="ph")
                nc.tensor.transpose(ofq[:, 0, 0:1], in_=o2r[:, ck],
                                    identity=ident_s[0:1, 0:1])
                s2t = ep2.tile([128, 1], F32, tag="s2t")
                nc.vector.reciprocal(s2t[:], ofq[:, 0, 0:1])
                ofs = ep2.tile([128, C], F32, tag="ofs")
                nc.vector.tensor_scalar_mul(ofs[:], ofp[:, 0, 0:C], s2t[:])
                nc.sync.dma_start(out=outp_d.ap()[k * 128:(k + 1) * 128, :],
                                  in_=ofs[:])


# --------------------------------------------------------------------------
# host side
# --------------------------------------------------------------------------

def shard_inputs(x, adj, W, a1, a2, Wo, ao1, ao2):
    x = np.asarray(x, np.float32)
    adj = np.asarray(adj)
    W = np.asarray(W, np.float32)
    a1 = np.asarray(a1, np.float32)
    a2 = np.asarray(a2, np.float32)
    Wo = np.asarray(Wo, np.float32)
    ao1 = np.asarray(ao1, np.float32)
    ao2 = np.asarray(ao2, np.float32)
    BF = ml_dtypes.bfloat16

    wvec1 = np.einsum("hfd,hd->hf", W, a1)          # [H, F]
    wvec2 = np.einsum("hfd,hd->hf", W, a2)
    wext = np.concatenate([W, wvec1[:, :, None], wvec2[:, :, None]],
                          axis=2).astype(BF)
    a1rep = np.repeat(wvec1[:, :, None], 128, axis=2).astype(BF)
    wo1 = Wo @ ao1                                   # [512]
    wo2 = Wo @ ao2
    woflat = np.concatenate([wo1[:, None], wo2[:, None], Wo], 1)  # [512, 34]
    woext = woflat.reshape(KT, 128, C + 2).astype(np.float32)
    wcorr = (-woflat.sum(0))[None, :].astype(np.float32)
    ident = np.eye(128, dtype=np.float32)

    in_maps = []
    for c in range(N_CORES):
        b, half = c // 2, c % 2
        i0 = half * I
        xt = np.ascontiguousarray(x[b].T).astype(BF)   # [F, N]
        xtl = np.ascontiguousarray(xt[:, i0:i0 + I])
        adjt = adj[b, i0:i0 + I, :].T                # [N, I] = (j, i)
        mb = np.where(adjt > 0, np.float32(0.0), np.float32(-BIG))
        mb = np.ascontiguousarray(mb.reshape(JT, 128, I)).astype(BF)
        in_maps.append({
            "xt": xt, "xtl": xtl, "mb": mb, "wext": wext,
            "a1rep": a1rep, "woext": woext, "wcorr": wcorr, "ident": ident,
        })
    return in_maps


# Engine routing chosen by cost-model sweep (TimelineSim).
DEFAULT_CFG = {"p4": 43, "p7": 27, "ep_defer": 2, "f1rep_eng": "dve"}

_CACHE = {}


def _program():
    if "nc" not in _CACHE:
        _CACHE["nc"] = build_program(with_collective=True, cfg=DEFAULT_CFG)
    return _CACHE["nc"]


def kernel(**inputs):
    nc = _program()
    in_maps = shard_inputs(**inputs)
    res = run_bass_kernel_spmd(nc, in_maps, list(range(N_CORES)))
    _CACHE["last_results"] = res
    out = np.empty((B, N, C), np.float32)
    for c in range(N_CORES):
        b, half = c // 2, c % 2
        out[b, half * I:(half + 1) * I, :] = res.results[c]["outp"]
    return out


# revision 38
# speedup vs baseline: 1.4803x; 1.1345x over previous
"""GAT forward (2-layer graph attention, B=4 N=2048 F=128 H=8 D=64 C=32)
as a Bass/Tile SPMD kernel on 8 Trainium2 NeuronCores.

Sharding: core c -> (batch b=c//2, query-row half c%2).  Each core computes
attention for its 1024 query rows over all 2048 keys for all 8 heads
(layer 1) and for the output head (layer 2).  The only cross-core exchange
is a 2-rank AllGather of the layer-2 projections [g1|g2|Wh2] ([1024,34] f32)
within each (2b, 2b+1) pair.

Layout: attention logits are built TRANSPOSED, e^T[j (keys) = partitions,
i (queries) = free], so the PV matmul needs no operand transposes
(lhsT = Wh[j,d] stationary, rhs = p[j,i] moving, out = h^T[d,i]) and
softmax row sums come from a ones-column appended to Wh (PSUM row D).

The N^2 elementwise work (mask+f1+f2, leaky-relu, exp over 144 [128,1024]
logit tiles) is the bottleneck; everything runs in bf16 (2x DVE tensor-
tensor throughput; ACT is dtype-independent) and every logit PAIR of tiles
is routed down one of three pipelines to saturate ACT+DVE+GPSIMD jointly:

  P3 : u = mb + f1rep       (DVE tensor_tensor, bf16 2x)
       z = Prelu(u + f2col) (ACT, per-sub bias)     p = Exp(z)  (ACT)
  P4 : same but the combine runs on GPSIMD tensor_add
  P7 : exp(prelu(u)) == max(exp(u), exp(0.2u)) and exp(u) factors rank-1:
       q1 = (mb01 * exp(f2_j)) * exp(f1_i)   (DVE STT mult,mult)
       q2 = (mb01 * exp(.2 f2)) * exp(.2 f1) (DVE STT)
       p  = max(q1, q2)                      (GPSIMD tensor_max; no ACT!)

Softmax division is deferred to the per-head-pair epilogue (1/S via Ln/Exp
of the row-sum), fused with ELU via elu(v)+1 = relu(v) + exp(min(v,0)),
the +1 folded into a rank-1 correction matmul of the layer-2 projection.
"""

import numpy as np
import ml_dtypes

import concourse.bass as bass
import concourse.tile as tile
from concourse import mybir
from concourse.bass_utils import run_bass_kernel_spmd

F32 = mybir.dt.float32
F32R = mybir.dt.float32r
BF16 = mybir.dt.bfloat16

B, N, F, H, D, C = 4, 2048, 128, 8, 64, 32
I = N // 2          # query rows per core
JT = N // 128       # key tiles
IC = I // 128       # query-row 128-chunks per core
KT = (H * D) // 128 # hidden-dim 128-chunks
ALPHA = 0.2
BIG = 1e15          # mask bias; also dominates A*B in the factored path
N_CORES = 8
REPLICA_GROUPS = [[0, 1], [2, 3], [4, 5], [6, 7]]

ADD = mybir.AluOpType.add
MAX = mybir.AluOpType.max
MULT = mybir.AluOpType.mult
# NOTE: hardware "Lrelu" has a fixed 0.01 slope and ignores alpha;
# "Prelu" honors alpha (verified on HW) — it is the configurable leaky relu.
ACT_LRELU = mybir.ActivationFunctionType.Prelu
ACT_EXP = mybir.ActivationFunctionType.Exp
ACT_LN = mybir.ActivationFunctionType.Ln
ACT_COPY = mybir.ActivationFunctionType.Copy

N_HEADS_ALL = H + 1          # 8 layer-1 heads + the layer-2 output head
PAIRS_PER_HEAD = JT // 2     # 8
N_PAIRS = N_HEADS_ALL * PAIRS_PER_HEAD  # 72


def _split_multiwaits(nc):
    """Pinned walrus accepts only one sync-wait per instruction; Tile's exit
    drain (and occasionally others) carries several.  Hoist extras onto
    single-wait Drains on the same engine immediately before the owner."""
    n_fixed = 0
    for fn in nc.m.functions:
        for bb in fn.blocks:
            for name in [i.name for i in bb.instructions]:
                idx = [i.name for i in bb.instructions].index(name)
                inst = bb.instructions[idx]
                si = inst.sync_info
                if si is None or len(si.on_wait) <= 1:
                    continue
                waits = list(si.on_wait)
                for k, w in enumerate(waits[:-1]):
                    nd = mybir.InstDrain(
                        name=f"waitfix-{inst.name}-{k}", ins=[], outs=[])
                    nd.engine = inst.engine
                    nd.sync_info = mybir.SyncInfo(on_wait=[w], on_update=[])
                    nc.register_instruction(nd, overwrite=True)
                    bb.instructions.insert(idx + k, nd)
                inst.sync_info = mybir.SyncInfo(
                    on_wait=waits[-1:], on_update=list(si.on_update))
                n_fixed += 1
    return n_fixed


def _make_route(cfg):
    """Per-pair pipeline assignment:
      3 (DVE-add + ACT prelu/exp), 4 (GPS-add + ACT prelu/exp),
      7 (factored rank-1 exps on DVE + GPS max, no ACT),
      2 (DVE STT-add + DVE prelu + ACT exp only).
    Counts spread across the 9 'heads' (8 L1 + L2); p7 optionally
    concentrated into few heads (fewer exp(f1)-prep ops on ACT)."""
    if "route" in cfg:                      # explicit per-pair override
        route = list(cfg["route"])
        assert len(route) == N_PAIRS
        return route
    n4 = int(cfg.get("p4", 24))
    n7 = int(cfg.get("p7", 33))
    n2 = int(cfg.get("p2", 0))
    n3 = N_PAIRS - n4 - n7 - n2
    assert n3 >= 0
    conc = int(cfg.get("p7_conc", 0))  # 0 = spread; 1 = concentrate P7
    per_head = [[0, 0, 0, 0] for _ in range(N_HEADS_ALL)]  # [n3, n4, n7, n2]
    if conc:
        # fill P7 into heads round-robin starting at head 1, PAIRS_PER_HEAD
        # per head; distribute the rest evenly over remaining slots
        order = [1, 3, 5, 7, 8, 0, 2, 4, 6]
        left7 = n7
        for hh in order:
            take = min(left7, PAIRS_PER_HEAD)
            per_head[hh][2] = take
            left7 -= take
            if left7 == 0:
                break
        slots = [PAIRS_PER_HEAD - per_head[hh][2] for hh in range(N_HEADS_ALL)]
        for idx, cnt in ((1, n4), (3, n2), (0, n3)):
            left = cnt
            while left > 0:
                done = True
                for hh in range(N_HEADS_ALL):
                    used = sum(per_head[hh])
                    if used < PAIRS_PER_HEAD and left > 0:
                        per_head[hh][idx] += 1
                        left -= 1
                        done = False
                if done:
                    break
    else:
        for idx, cnt in enumerate((n3, n4, n7, n2)):
            base, rem = divmod(cnt, N_HEADS_ALL)
            for hh in range(N_HEADS_ALL):
                per_head[hh][idx] = base + (1 if hh < rem else 0)
        for hh in range(N_HEADS_ALL):
            while sum(per_head[hh]) > PAIRS_PER_HEAD:
                per_head[hh][int(np.argmax(per_head[hh]))] -= 1
            while sum(per_head[hh]) < PAIRS_PER_HEAD:
                per_head[hh][int(np.argmin(per_head[hh]))] += 1
    if "l2route" in cfg:
        l2r = list(cfg["l2route"])
        assert len(l2r) == PAIRS_PER_HEAD
    else:
        l2r = None
    route = []
    for hh in range(N_HEADS_ALL):
        if hh == H and l2r is not None:
            route.extend(l2r)
            continue
        c3, c4, c7, c2 = per_head[hh]
        pool = [7] * c7 + [4] * c4 + [3] * c3 + [2] * c2
        mix, lo, hi = [], 0, len(pool) - 1
        toggle = True
        while lo <= hi:
            if toggle:
                mix.append(pool[lo]); lo += 1
            else:
                mix.append(pool[hi]); hi -= 1
            toggle = not toggle
        route.extend(mix)
    return route


def build_program(with_collective=True, cfg=None, repeat=1):
    cfg = dict(cfg or {})
    route = _make_route(cfg)

    nc = bass.Bass("TRN2", target_bir_lowering=False, debug=False,
                   enable_asserts=False, num_devices=N_CORES)

    xt_d = nc.dram_tensor("xt", [F, N], BF16, kind="ExternalInput")
    xtl_d = nc.dram_tensor("xtl", [F, I], BF16, kind="ExternalInput")
    mb_d = nc.dram_tensor("mb", [JT, 128, I], BF16, kind="ExternalInput")
    wext_d = nc.dram_tensor("wext", [H, F, D + 2], BF16, kind="ExternalInput")
    a1rep_d = nc.dram_tensor("a1rep", [H, F, 128], BF16, kind="ExternalInput")
    woext_d = nc.dram_tensor("woext", [KT, 128, C + 2], F32,
                             kind="ExternalInput")
    wcorr_d = nc.dram_tensor("wcorr", [1, C + 2], F32, kind="ExternalInput")
    ident_d = nc.dram_tensor("ident", [128, 128], F32, kind="ExternalInput")
    outp_d = nc.dram_tensor("outp", [I, C], F32, kind="ExternalOutput")

    with tile.TileContext(nc) as tc:
        if repeat > 1:
            def body(iv, unroll=None):
                _build_body(nc, tc, xt_d, xtl_d, mb_d, wext_d, a1rep_d,
                            woext_d, wcorr_d, ident_d, outp_d,
                            with_collective, route, cfg)
            with tc.For_i(0, repeat, 1) as iv:
                body(iv)
        else:
            _build_body(nc, tc, xt_d, xtl_d, mb_d, wext_d, a1rep_d,
                        woext_d, wcorr_d, ident_d, outp_d,
                        with_collective, route, cfg)
    _split_multiwaits(nc)
    return nc


def _emit_pair(nc, work, workp, mode, subs, pair_args):
    """Emit one logit pair.  subs = [(jt, mb_ap, mq_ap, f2col_ap, e1col_ap,
    e2col_ap)]; pair_args = (f1rep, B1, B2)."""
    f1rep, B1, B2 = pair_args
    if mode == 7:
        q1 = work.tile([128, 2, I], BF16, tag="q1", bufs=3)
        q2 = work.tile([128, 2, I], BF16, tag="q2", bufs=3)
        for k, (jt, mb_ap, f2c, e1c, e2c) in enumerate(subs):
            nc.vector.scalar_tensor_tensor(
                out=q1[:, k, :], in0=B1[:], scalar=e1c, in1=mb_ap,
                op0=MULT, op1=ADD)
            nc.vector.tensor_scalar_add(q2[:, k, :], mb_ap, e2c)
        p = workp.tile([128, 2, I], BF16, tag="p")
        nc.vector.tensor_tensor(out=p[:], in0=q1[:], in1=q2[:], op=MAX)
        nc.vector.tensor_scalar_max(p[:], p[:], 0.0)
        return p
    if mode == 2:
        # combine WITH f2 (STT) so prelu can run on DVE; exp is the only ACT op
        u = work.tile([128, 2, I], BF16, tag="u")
        for k, (jt, mb_ap, f2c, e1c, e2c) in enumerate(subs):
            nc.vector.scalar_tensor_tensor(
                out=u[:, k, :], in0=mb_ap, scalar=f2c, in1=f1rep[:],
                op0=ADD, op1=ADD)
        w = work.tile([128, 2, I], BF16, tag="q1", bufs=3)
        nc.vector.tensor_scalar_mul(w[:], u[:], ALPHA)     # bf16 4x
        nc.vector.tensor_tensor(out=u[:], in0=u[:], in1=w[:], op=MAX)
        p = workp.tile([128, 2, I], BF16, tag="p")
        nc.scalar.activation(p[:], u[:], ACT_EXP)
        return p
    u = work.tile([128, 2, I], BF16, tag="u")
    for k, (jt, mb_ap, f2c, e1c, e2c) in enumerate(subs):
        if mode == 4:
            nc.gpsimd.tensor_add(u[:, k, :], mb_ap, f1rep[:])
        else:
            nc.vector.tensor_tensor(out=u[:, k, :], in0=mb_ap, in1=f1rep[:],
                                    op=ADD)
    for k, (jt, mb_ap, f2c, e1c, e2c) in enumerate(subs):
        nc.scalar.activation(u[:, k, :], u[:, k, :], ACT_LRELU,
                             bias=f2c, alpha=ALPHA)
    p = workp.tile([128, 2, I], BF16, tag="p")
    nc.scalar.activation(p[:], u[:], ACT_EXP)
    return p


def _copy_engine(nc, eng, out, in_):
    if eng == "act":
        nc.scalar.activation(out, in_, ACT_COPY)
    elif eng == "gps":
        nc.gpsimd.tensor_copy(out=out, in_=in_)
    else:
        nc.vector.tensor_copy(out=out, in_=in_)


def _build_body(nc, tc, xt_d, xtl_d, mb_d, wext_d, a1rep_d, woext_d,
                wcorr_d, ident_d, outp_d, with_collective, route, cfg):
    from contextlib import ExitStack
    ctx = ExitStack()
    f1rep_eng = cfg.get("f1rep_eng", "dve")
    rbc_eng = cfg.get("rbc_eng", "act")
    assert f1rep_eng != "gps" and rbc_eng != "gps"  # GPSIMD cannot read PSUM
    
    ph0_engs = cfg.get("ph0_engs", ("act", "dve"))
    ep_v_gps = False  # GPSIMD cannot read PSUM (hT)
    with ctx:
        singles = ctx.enter_context(tc.tile_pool(name="singles", bufs=1))
        psA = ctx.enter_context(tc.tile_pool(name="psA", bufs=2, space="PSUM"))
        psB = ctx.enter_context(tc.tile_pool(name="psB", bufs=1, space="PSUM"))
        psC = ctx.enter_context(tc.tile_pool(name="psC", bufs=2, space="PSUM"))
        dram = ctx.enter_context(tc.tile_pool(name="dram", bufs=1,
                                              space="DRAM"))

        # ---------------- persistent loads ----------------
        mb_s = singles.tile([128, JT, I], BF16)
        nc.sync.dma_start(out=mb_s[:, 0:2, :],
                          in_=mb_d.ap()[0:2].rearrange("jt p i -> p jt i"))
        xtl_s = singles.tile([F, I], BF16)
        nc.sync.dma_start(out=xtl_s[:], in_=xtl_d.ap())
        a1rep_s = singles.tile([F, H, 128], BF16)
        nc.sync.dma_start(out=a1rep_s[:],
                          in_=a1rep_d.ap().rearrange("h f e -> f h e"))
        wcorr_s = singles.tile([1, C + 2], F32)
        nc.sync.dma_start(out=wcorr_s[:], in_=wcorr_d.ap())
        ident_s = singles.tile([128, 128], F32)
        nc.sync.dma_start(out=ident_s[:], in_=ident_d.ap())
        woext_raw = singles.tile([128, KT, C + 2], F32)
        nc.sync.dma_start(out=woext_raw[:],
                          in_=woext_d.ap().rearrange("k f e -> f k e"))
        woext_s = singles.tile([128, KT, C + 2], F32R)
        nc.vector.tensor_copy(out=woext_s[:], in_=woext_raw[:])

        ones_s = singles.tile([1, 128], BF16)
        nc.gpsimd.memset(ones_s[:], 1.0)
        onesf_s = singles.tile([1, 128], F32)
        nc.gpsimd.memset(onesf_s[:], 1.0)

        whbuf = singles.tile([128, H, JT, D + 1], BF16)
        nc.gpsimd.memset(whbuf[:, :, :, D:D + 1], 1.0)
        fcol = singles.tile([128, H, JT, 1], F32)
        fexp1 = singles.tile([128, H, JT, 1], F32)
        fexp2 = singles.tile([128, H, JT, 1], F32)
        hcatT = singles.tile([128, KT, I], F32R)

        xt_s = singles.tile([F, N], BF16)
        nc.sync.dma_start(out=xt_s[:], in_=xt_d.ap())
        wext_s = singles.tile([F, H, D + 2], BF16)
        nc.sync.dma_start(out=wext_s[:],
                          in_=wext_d.ap().rearrange("h f e -> f h e"))
        for j0 in (2, 6, 10):
            j1 = j0 + 4 if j0 < 10 else JT
            nc.sync.dma_start(
                out=mb_s[:, j0:j1, :],
                in_=mb_d.ap()[j0:j1].rearrange("jt p i -> p jt i"))

        work = ctx.enter_context(tc.tile_pool(name="work", bufs=4))
        workp = ctx.enter_context(tc.tile_pool(name="workp", bufs=4))
        ep1 = ctx.enter_context(tc.tile_pool(name="ep1", bufs=1))
        ep2 = ctx.enter_context(tc.tile_pool(name="ep2", bufs=2))
        epL2 = ctx.enter_context(tc.tile_pool(name="epL2", bufs=1))

        def emit_phase0_head(h):
            # Wh tiles + f columns for head h (emitted per-head so the
            # copies overlap the previous head's logit work)
            for jg in range(JT // 4):
                whp = psA.tile([128, 4, D + 2], F32, tag="ph")
                for k in range(4):
                    jt = jg * 4 + k
                    nc.tensor.matmul(whp[:, k, :],
                                     lhsT=xt_s[:, jt * 128:(jt + 1) * 128],
                                     rhs=wext_s[:, h, :])
                dst = whbuf[:, h, jg * 4:(jg + 1) * 4, 0:D]
                _copy_engine(nc, ph0_engs[jg % len(ph0_engs)],
                             dst, whp[:, :, 0:D])
                nc.vector.tensor_copy(
                    out=fcol[:, h, jg * 4:(jg + 1) * 4, :],
                    in_=whp[:, :, D + 1:D + 2])
            nc.scalar.activation(fexp1[:, h], fcol[:, h], ACT_EXP)
            nc.scalar.activation(fexp2[:, h], fcol[:, h], ACT_EXP,
                                 scale=ALPHA)

        ep_state = {}

        def emit_half_ep(hT, h):
            # per-head half-epilogue: rinv = 1/S via DVE reciprocal, SWDGE
            # partition broadcast, v-half = hT*rinv.  The odd half finishes:
            # hcat = elu(v)+1 = relu(v)+exp(min(v,0)).
            rinv = ep1.tile([1, I], F32, tag=f"ri{h % 2}", bufs=1)
            if cfg.get("dbg_ep_lnexp"):
                lnS_ = ep1.tile([1, I], F32, tag=f"ln{h % 2}", bufs=2)
                nc.scalar.activation(lnS_[:], hT[D:D + 1, :], ACT_LN)
                nc.scalar.activation(rinv[:], lnS_[:], ACT_EXP, scale=-1.0)
            else:
                nc.vector.reciprocal(rinv[:], hT[D:D + 1, :])
            if h % 2 == 0:
                v = ep1.tile([128, I], BF16, tag="v", bufs=2)
                ep_state["v"] = v
                half = slice(0, D)
            else:
                v = ep_state["v"]
                half = slice(D, 128)
            rbp = psB.tile([128, I], F32, tag="rep")
            for hf_ in range(I // 512):
                sl_ = slice(hf_ * 512, (hf_ + 1) * 512)
                nc.tensor.matmul(rbp[0:D, sl_], lhsT=onesf_s[0:1, 0:D],
                                 rhs=rinv[0:1, sl_])
            # rbc half kept at base partition 0: DVE inputs must share bases
            rbc = ep1.tile([D, I], F32, tag=f"rb{h % 2}", bufs=1)
            _copy_engine(nc, rbc_eng, rbc[:], rbp[0:D, :])
            nc.vector.tensor_tensor(out=v[half, :], in0=hT[0:D, :],
                                    in1=rbc[:], op=MULT)
            if h % 2 == 1:
                t = ep1.tile([128, I], BF16, tag="t", bufs=2)
                nc.vector.tensor_scalar_min(t[:], v[:], 0.0)
                nc.scalar.activation(t[:], t[:], ACT_EXP)
                nc.vector.scalar_tensor_tensor(
                    out=hcatT[:, h // 2, :], in0=v[:], scalar=0.0, in1=t[:],
                    op0=MAX, op1=ADD)

        def emit_head_prep(h):
            emit_phase0_head(h)
            head_modes = route[h * PAIRS_PER_HEAD:(h + 1) * PAIRS_PER_HEAD]
            need_f1rep = any(m in (2, 3, 4) for m in head_modes)
            need_B = any(m == 7 for m in head_modes)
            f1p = psB.tile([128, I], F32, tag="rep")
            for hf in range(I // 512):
                sl = slice(hf * 512, (hf + 1) * 512)
                nc.tensor.matmul(f1p[:, sl], lhsT=a1rep_s[:, h, :],
                                 rhs=xtl_s[:, sl])
            f1rep_s = B1_s = None
            B2_s = True  # unused (q2 needs only the per-partition scalar)
            if need_f1rep:
                f1rep_s = ep2.tile([128, I], BF16, tag="f1rep")
                _copy_engine(nc, f1rep_eng, f1rep_s[:], f1p[:])
            if need_B:
                B1_s = ep2.tile([128, I], BF16, tag="B1")
                nc.scalar.activation(B1_s[:], f1p[:], ACT_EXP, scale=1.0 - ALPHA)
            return f1rep_s, B1_s, B2_s

        # ---------------- layer 1 ----------------
        pending_ep = None   # deferred half-epilogue (software pipelining)
        preps = emit_head_prep(0)
        for h in range(H):
            head_modes = route[h * PAIRS_PER_HEAD:(h + 1) * PAIRS_PER_HEAD]
            cur = preps
            hT = psC.tile([D + 1, I], F32, tag="acc")
            for jp in range(PAIRS_PER_HEAD):
                if jp == int(cfg.get("ep_defer", 2)) and pending_ep is not None:
                    pending_ep()
                    pending_ep = None
                if jp == int(cfg.get("prep_at", 4)) and h + 1 < H:
                    preps = emit_head_prep(h + 1)
                mode = head_modes[jp]
                subs = []
                for k in range(2):
                    jt = jp * 2 + k
                    subs.append((jt, mb_s[:, jt, :],
                                 fcol[:, h, jt, :], fexp1[:, h, jt, :],
                                 fexp2[:, h, jt, :]))
                p = _emit_pair(nc, work, workp, mode, subs, cur)
                for k in range(2):
                    jt = jp * 2 + k
                    for hf in range(I // 512):
                        sl = slice(hf * 512, (hf + 1) * 512)
                        nc.tensor.matmul(hT[:, sl],
                                         lhsT=whbuf[:, h, jt, :],
                                         rhs=p[:, k, sl],
                                         start=(jt == 0), stop=(jt == JT - 1))

            pending_ep = (lambda t_=hT, h_=h: emit_half_ep(t_, h_))
        if pending_ep is not None:
            pending_ep()
            pending_ep = None

        # ---------------- layer 2 projection + gather (bf16 payload) ------
        wh2loc = singles.tile([128, IC, C + 2], F32)
        gin = dram.tile([I, C + 2], F32)
        for ic in range(IC):
            w2p = psA.tile([128, 4, D + 2], F32, tag="ph")
            for kt in range(KT):
                nc.tensor.matmul(
                    w2p[:, 0, 0:C + 2],
                    lhsT=hcatT[:, kt, ic * 128:(ic + 1) * 128],
                    rhs=woext_s[:, kt, :],
                    start=(kt == 0), stop=False)
            nc.tensor.matmul(w2p[:, 0, 0:C + 2], lhsT=onesf_s[0:1, :],
                             rhs=wcorr_s[:], start=False, stop=True)
            nc.vector.tensor_copy(out=wh2loc[:, ic, :], in_=w2p[:, 0, 0:C + 2])
        nc.sync.dma_start(
            out=gin.rearrange("(ic p) c -> p ic c", p=128),
            in_=wh2loc[:])

        gout = dram.tile([N, C + 2], F32)
        if with_collective:
            nc.gpsimd.collective_compute(
                "AllGather", mybir.AluOpType.bypass,
                replica_groups=REPLICA_GROUPS,
                ins=[gin.opt()], outs=[gout.opt()])
        else:  # timing-model variant: fake the exchange with two local copies
            nc.sync.dma_start(out=gout[0:I, :], in_=gin[:])
            nc.sync.dma_start(out=gout[I:N, :], in_=gin[:])

        # g1 row (local queries) -> replicated [128, I]
        g1rowp = psB.tile([128, I], F32, tag="rep")
        for ic in range(IC):
            nc.tensor.transpose(g1rowp[0:1, ic * 128:(ic + 1) * 128],
                                in_=wh2loc[:, ic, 0:1], identity=ident_s[:])
        g1row_s = epL2.tile([1, I], BF16, tag="g1row")
        nc.scalar.activation(g1row_s[:], g1rowp[0:1, :], ACT_COPY)
        g1rp = psB.tile([128, I], F32, tag="rep")
        for hf in range(I // 512):
            sl = slice(hf * 512, (hf + 1) * 512)
            nc.tensor.matmul(g1rp[:, sl], lhsT=ones_s[0:1, :],
                             rhs=g1row_s[0:1, sl])
        l2_modes = route[H * PAIRS_PER_HEAD:]
        g1rep_s = B1L2 = None
        B2L2 = True
        if any(m in (2, 3, 4) for m in l2_modes):
            g1rep_s = singles.tile([128, I], BF16)
            nc.vector.tensor_copy(out=g1rep_s[:], in_=g1rp[:])
        if any(m == 7 for m in l2_modes):
            B1L2 = singles.tile([128, I], BF16)
            nc.scalar.activation(B1L2[:], g1rp[:], ACT_EXP, scale=1.0 - ALPHA)

        # gathered rows: [g1, g2, Wh2(32)] f32 staged, bf16 for the PV lhsT
        wh2tmp = singles.tile([128, JT, C + 2], F32)
        nc.sync.dma_start(
            out=wh2tmp[:],
            in_=gout.rearrange("(jt p) c -> p jt c", p=128))
        wh2gr = singles.tile([128, JT, C + 3], BF16)
        nc.gpsimd.memset(wh2gr[:, :, C + 2:C + 3], 1.0)
        its1 = singles.tile([128, JT, 1], F32)
        its2 = singles.tile([128, JT, 1], F32)
        for jg in range(JT // 4):
            s4 = slice(jg * 4, (jg + 1) * 4)
            nc.gpsimd.tensor_copy(out=wh2gr[:, s4, 0:C + 2],
                                  in_=wh2tmp[:, s4, :])
            nc.scalar.activation(its1[:, s4, :], wh2tmp[:, s4, 1:2], ACT_EXP)
            nc.scalar.activation(its2[:, s4, :], wh2tmp[:, s4, 1:2], ACT_EXP,
                                 scale=ALPHA)

        # ---------------- layer 2 attention ----------------
        o2T = psC.tile([D + 1, I], F32, tag="acc")
        for jp in range(PAIRS_PER_HEAD):
            mode = l2_modes[jp]
            subs = []
            for k in range(2):
                jt = jp * 2 + k
                subs.append((jt, mb_s[:, jt, :],
                             wh2tmp[:, jt, 1:2], its1[:, jt, :],
                             its2[:, jt, :]))
            p = _emit_pair(nc, work, workp, mode, subs,
                           (g1rep_s, B1L2, B2L2))
            for k in range(2):
                jt = jp * 2 + k
                for hf in range(I // 512):
                    sl = slice(hf * 512, (hf + 1) * 512)
                    nc.tensor.matmul(o2T[0:C + 1, sl],
                                     lhsT=wh2gr[:, jt, 2:C + 3],
                                     rhs=p[:, k, sl],
                                     start=(jt == 0), stop=(jt == JT - 1))

        # ---------------- finalize (transposed: per-query reciprocal) -----
        if cfg.get("dbg_simple_fin"):
            r2ln = epL2.tile([1, I], F32, tag="lnS2")
            nc.scalar.activation(r2ln[:], o2T[C:C + 1, :], ACT_LN)
            r2 = epL2.tile([1, I], BF16, tag="r2")
            nc.scalar.activation(r2[:], r2ln[:], ACT_EXP, scale=-1.0)
            rbc2p = psB.tile([128, I], F32, tag="rep")
            for hf in range(I // 512):
                sl = slice(hf * 512, (hf + 1) * 512)
                nc.tensor.matmul(rbc2p[0:C, sl], lhsT=ones_s[0:1, 0:C],
                                 rhs=r2[0:1, sl])
            rbc2_s = epL2.tile([C, I], F32, tag="rbc2")
            nc.vector.tensor_copy(out=rbc2_s[:], in_=rbc2p[0:C, :])
            oT_s = epL2.tile([C, I], F32, tag="oT")
            nc.vector.tensor_tensor(out=oT_s[:], in0=o2T[0:C, :],
                                    in1=rbc2_s[:], op=MULT)
            for k in range(IC):
                ofp = psA.tile([128, 4, D + 2], F32, tag="ph")
                nc.tensor.transpose(ofp[:, 0, 0:C],
                                    in_=oT_s[:, k * 128:(k + 1) * 128],
                                    identity=ident_s[0:C, 0:C])
                ofs = ep2.tile([128, C], F32, tag="ofs")
                nc.vector.tensor_copy(out=ofs[:], in_=ofp[:, 0, 0:C])
                nc.sync.dma_start(out=outp_d.ap()[k * 128:(k + 1) * 128, :],
                                  in_=ofs[:])
        else:
            o2s = epL2.tile([C, I], F32, tag="o2s")
            nc.vector.tensor_copy(out=o2s[:], in_=o2T[0:C, :])
            o2r = epL2.tile([1, I], F32, tag="o2r")
            nc.vector.tensor_copy(out=o2r[:], in_=o2T[C:C + 1, :])
            for k in range(IC):
                ck = slice(k * 128, (k + 1) * 128)
                ofp = psA.tile([128, 4, D + 2], F32, tag="ph")
                nc.tensor.transpose(ofp[:, 0, 0:C], in_=o2s[:, ck],
                                    identity=ident_s[0:C, 0:C])
                ofq = psA.tile([128, 4, D + 2], F32, tag="ph")
                nc.tensor.transpose(ofq[:, 0, 0:1], in_=o2r[:, ck],
                                    identity=ident_s[0:1, 0:1])
                s2t = ep2.tile([128, 1], F32, tag="s2t")
                nc.vector.reciprocal(s2t[:], ofq[:, 0, 0:1])
                ofs = ep2.tile([128, C], F32, tag="ofs")
                nc.vector.tensor_scalar_mul(ofs[:], ofp[:, 0, 0:C], s2t[:])
                nc.sync.dma_start(out=outp_d.ap()[k * 128:(k + 1) * 128, :],
                                  in_=ofs[:])


# --------------------------------------------------------------------------
# host side
# --------------------------------------------------------------------------

def shard_inputs(x, adj, W, a1, a2, Wo, ao1, ao2):
    x = np.asarray(x, np.float32)
    adj = np.asarray(adj)
    W = np.asarray(W, np.float32)
    a1 = np.asarray(a1, np.float32)
    a2 = np.asarray(a2, np.float32)
    Wo = np.asarray(Wo, np.float32)
    ao1 = np.asarray(ao1, np.float32)
    ao2 = np.asarray(ao2, np.float32)
    BF = ml_dtypes.bfloat16

    wvec1 = np.einsum("hfd,hd->hf", W, a1)          # [H, F]
    wvec2 = np.einsum("hfd,hd->hf", W, a2)
    wext = np.concatenate([W, wvec1[:, :, None], wvec2[:, :, None]],
                          axis=2).astype(BF)
    a1rep = np.repeat(wvec1[:, :, None], 128, axis=2).astype(BF)
    wo1 = Wo @ ao1                                   # [512]
    wo2 = Wo @ ao2
    woflat = np.concatenate([wo1[:, None], wo2[:, None], Wo], 1)  # [512, 34]
    woext = woflat.reshape(KT, 128, C + 2).astype(np.float32)
    wcorr = (-woflat.sum(0))[None, :].astype(np.float32)
    ident = np.eye(128, dtype=np.float32)

    in_maps = []
    for c in range(N_CORES):
        b, half = c // 2, c % 2
        i0 = half * I
        xt = np.ascontiguousarray(x[b].T).astype(BF)   # [F, N]
        xtl = np.ascontiguousarray(xt[:, i0:i0 + I])
        adjt = adj[b, i0:i0 + I, :].T                # [N, I] = (j, i)
        mb = np.where(adjt > 0, np.float32(0.0), np.float32(-BIG))
        mb = np.ascontiguousarray(mb.reshape(JT, 128, I)).astype(BF)
        in_maps.append({
            "xt": xt, "xtl": xtl, "mb": mb, "wext": wext,
            "a1rep": a1rep, "woext": woext, "wcorr": wcorr, "ident": ident,
        })
    return in_maps


# Engine routing chosen by cost-model sweep (TimelineSim).
DEFAULT_CFG = {"p4": 43, "p7": 27, "ep_defer": 2, "f1rep_eng": "dve",
               "rbc_eng": "dve", "prep_at": 3,
               "l2route": [4, 2, 7, 4, 2, 7, 4, 4]}

_CACHE = {}


def _program():
    if "nc" not in _CACHE:
        _CACHE["nc"] = build_program(with_collective=True, cfg=DEFAULT_CFG)
    return _CACHE["nc"]


def kernel(**inputs):
    nc = _program()
    in_maps = shard_inputs(**inputs)
    res = run_bass_kernel_spmd(nc, in_maps, list(range(N_CORES)))
    _CACHE["last_results"] = res
    out = np.empty((B, N, C), np.float32)
    for c in range(N_CORES):
        b, half = c // 2, c % 2
        out[b, half * I:(half + 1) * I, :] = res.results[c]["outp"]
    return out


# revision 41
# speedup vs baseline: 1.4821x; 1.0012x over previous
"""GAT forward (2-layer graph attention, B=4 N=2048 F=128 H=8 D=64 C=32)
as a Bass/Tile SPMD kernel on 8 Trainium2 NeuronCores.

Sharding: core c -> (batch b=c//2, query-row half c%2).  Each core computes
attention for its 1024 query rows over all 2048 keys for all 8 heads
(layer 1) and for the output head (layer 2).  The only cross-core exchange
is a 2-rank AllGather of the layer-2 projections [g1|g2|Wh2] ([1024,34] f32)
within each (2b, 2b+1) pair.

Layout: attention logits are built TRANSPOSED, e^T[j (keys) = partitions,
i (queries) = free], so the PV matmul needs no operand transposes
(lhsT = Wh[j,d] stationary, rhs = p[j,i] moving, out = h^T[d,i]) and
softmax row sums come from a ones-column appended to Wh (PSUM row D).

The N^2 elementwise work (mask+f1+f2, leaky-relu, exp over 144 [128,1024]
logit tiles) is the bottleneck; everything runs in bf16 (2x DVE tensor-
tensor throughput; ACT is dtype-independent) and every logit PAIR of tiles
is routed down one of three pipelines to saturate ACT+DVE+GPSIMD jointly:

  P3 : u = mb + f1rep       (DVE tensor_tensor, bf16 2x)
       z = Prelu(u + f2col) (ACT, per-sub bias)     p = Exp(z)  (ACT)
  P4 : same but the combine runs on GPSIMD tensor_add
  P2 : STT combine (incl f2) + DVE prelu; exp is the only ACT op
  P7 : exp(prelu(u)) == max(exp(u), exp(0.2u)); exps factor rank-1 and,
       after a softmax-invariant per-query rescale by exp(-0.2 f1):
       q1 = exp(.8 f1_i)*exp(f2_j) + mb     (DVE STT mult,add)
       q2 = exp(.2 f2_j) + mb               (DVE tensor-scalar add)
       p  = relu(max(q1, q2))               (DVE bf16 2x ops; no ACT!)
       (mb = -1e15 masked / 0 unmasked dominates both branches exactly)

Softmax division is deferred to the per-head-pair epilogue (1/S via Ln/Exp
of the row-sum), fused with ELU via elu(v)+1 = relu(v) + exp(min(v,0)),
the +1 folded into a rank-1 correction matmul of the layer-2 projection.
"""

import numpy as np
import ml_dtypes

import concourse.bass as bass
import concourse.tile as tile
from concourse import mybir
from concourse.bass_utils import run_bass_kernel_spmd

F32 = mybir.dt.float32
F32R = mybir.dt.float32r
BF16 = mybir.dt.bfloat16

B, N, F, H, D, C = 4, 2048, 128, 8, 64, 32
I = N // 2          # query rows per core
JT = N // 128       # key tiles
IC = I // 128       # query-row 128-chunks per core
KT = (H * D) // 128 # hidden-dim 128-chunks
ALPHA = 0.2
BIG = 1e15          # mask bias; also dominates A*B in the factored path
N_CORES = 8
REPLICA_GROUPS = [[0, 1], [2, 3], [4, 5], [6, 7]]

ADD = mybir.AluOpType.add
MAX = mybir.AluOpType.max
MULT = mybir.AluOpType.mult
# NOTE: hardware "Lrelu" has a fixed 0.01 slope and ignores alpha;
# "Prelu" honors alpha (verified on HW) — it is the configurable leaky relu.
ACT_LRELU = mybir.ActivationFunctionType.Prelu
ACT_EXP = mybir.ActivationFunctionType.Exp
ACT_LN = mybir.ActivationFunctionType.Ln
ACT_COPY = mybir.ActivationFunctionType.Copy

N_HEADS_ALL = H + 1          # 8 layer-1 heads + the layer-2 output head
PAIRS_PER_HEAD = JT // 2     # 8
N_PAIRS = N_HEADS_ALL * PAIRS_PER_HEAD  # 72


def _split_multiwaits(nc):
    """Pinned walrus accepts only one sync-wait per instruction; Tile's exit
    drain (and occasionally others) carries several.  Hoist extras onto
    single-wait Drains on the same engine immediately before the owner."""
    n_fixed = 0
    for fn in nc.m.functions:
        for bb in fn.blocks:
            for name in [i.name for i in bb.instructions]:
                idx = [i.name for i in bb.instructions].index(name)
                inst = bb.instructions[idx]
                si = inst.sync_info
                if si is None or len(si.on_wait) <= 1:
                    continue
                waits = list(si.on_wait)
                for k, w in enumerate(waits[:-1]):
                    nd = mybir.InstDrain(
                        name=f"waitfix-{inst.name}-{k}", ins=[], outs=[])
                    nd.engine = inst.engine
                    nd.sync_info = mybir.SyncInfo(on_wait=[w], on_update=[])
                    nc.register_instruction(nd, overwrite=True)
                    bb.instructions.insert(idx + k, nd)
                inst.sync_info = mybir.SyncInfo(
                    on_wait=waits[-1:], on_update=list(si.on_update))
                n_fixed += 1
    return n_fixed


def _make_route(cfg):
    """Per-pair pipeline assignment:
      3 (DVE-add + ACT prelu/exp), 4 (GPS-add + ACT prelu/exp),
      7 (factored rank-1 exps on DVE + GPS max, no ACT),
      2 (DVE STT-add + DVE prelu + ACT exp only).
    Counts spread across the 9 'heads' (8 L1 + L2); p7 optionally
    concentrated into few heads (fewer exp(f1)-prep ops on ACT)."""
    if "route" in cfg:                      # explicit per-pair override
        route = list(cfg["route"])
        assert len(route) == N_PAIRS
        return route
    n4 = int(cfg.get("p4", 24))
    n7 = int(cfg.get("p7", 33))
    n2 = int(cfg.get("p2", 0))
    n3 = N_PAIRS - n4 - n7 - n2
    assert n3 >= 0
    conc = int(cfg.get("p7_conc", 0))  # 0 = spread; 1 = concentrate P7
    per_head = [[0, 0, 0, 0] for _ in range(N_HEADS_ALL)]  # [n3, n4, n7, n2]
    if conc:
        # fill P7 into heads round-robin starting at head 1, PAIRS_PER_HEAD
        # per head; distribute the rest evenly over remaining slots
        order = [1, 3, 5, 7, 8, 0, 2, 4, 6]
        left7 = n7
        for hh in order:
            take = min(left7, PAIRS_PER_HEAD)
            per_head[hh][2] = take
            left7 -= take
            if left7 == 0:
                break
        slots = [PAIRS_PER_HEAD - per_head[hh][2] for hh in range(N_HEADS_ALL)]
        for idx, cnt in ((1, n4), (3, n2), (0, n3)):
            left = cnt
            while left > 0:
                done = True
                for hh in range(N_HEADS_ALL):
                    used = sum(per_head[hh])
                    if used < PAIRS_PER_HEAD and left > 0:
                        per_head[hh][idx] += 1
                        left -= 1
                        done = False
                if done:
                    break
    else:
        for idx, cnt in enumerate((n3, n4, n7, n2)):
            base, rem = divmod(cnt, N_HEADS_ALL)
            for hh in range(N_HEADS_ALL):
                per_head[hh][idx] = base + (1 if hh < rem else 0)
        for hh in range(N_HEADS_ALL):
            while sum(per_head[hh]) > PAIRS_PER_HEAD:
                per_head[hh][int(np.argmax(per_head[hh]))] -= 1
            while sum(per_head[hh]) < PAIRS_PER_HEAD:
                per_head[hh][int(np.argmin(per_head[hh]))] += 1
    if "l2route" in cfg:
        l2r = list(cfg["l2route"])
        assert len(l2r) == PAIRS_PER_HEAD
    else:
        l2r = None
    route = []
    for hh in range(N_HEADS_ALL):
        if hh == H and l2r is not None:
            route.extend(l2r)
            continue
        c3, c4, c7, c2 = per_head[hh]
        pool = [7] * c7 + [4] * c4 + [3] * c3 + [2] * c2
        mix, lo, hi = [], 0, len(pool) - 1
        toggle = True
        while lo <= hi:
            if toggle:
                mix.append(pool[lo]); lo += 1
            else:
                mix.append(pool[hi]); hi -= 1
            toggle = not toggle
        route.extend(mix)
    return route


def build_program(with_collective=True, cfg=None, repeat=1):
    cfg = dict(cfg or {})
    route = _make_route(cfg)

    nc = bass.Bass("TRN2", target_bir_lowering=False, debug=False,
                   enable_asserts=False, num_devices=N_CORES)

    xt_d = nc.dram_tensor("xt", [F, N], BF16, kind="ExternalInput")
    xtl_d = nc.dram_tensor("xtl", [F, I], BF16, kind="ExternalInput")
    mb_d = nc.dram_tensor("mb", [JT, 128, I], BF16, kind="ExternalInput")
    wext_d = nc.dram_tensor("wext", [H, F, D + 2], BF16, kind="ExternalInput")
    a1rep_d = nc.dram_tensor("a1rep", [H, F, 128], BF16, kind="ExternalInput")
    woext_d = nc.dram_tensor("woext", [KT, 128, C + 2], F32,
                             kind="ExternalInput")
    wcorr_d = nc.dram_tensor("wcorr", [1, C + 2], F32, kind="ExternalInput")
    ident_d = nc.dram_tensor("ident", [128, 128], F32, kind="ExternalInput")
    outp_d = nc.dram_tensor("outp", [I, C], F32, kind="ExternalOutput")

    with tile.TileContext(nc) as tc:
        if repeat > 1:
            def body(iv, unroll=None):
                _build_body(nc, tc, xt_d, xtl_d, mb_d, wext_d, a1rep_d,
                            woext_d, wcorr_d, ident_d, outp_d,
                            with_collective, route, cfg)
            with tc.For_i(0, repeat, 1) as iv:
                body(iv)
        else:
            _build_body(nc, tc, xt_d, xtl_d, mb_d, wext_d, a1rep_d,
                        woext_d, wcorr_d, ident_d, outp_d,
                        with_collective, route, cfg)
    _split_multiwaits(nc)
    return nc


def _emit_pair(nc, work, workp, mode, subs, pair_args):
    """Emit one logit pair.  subs = [(jt, mb_ap, mq_ap, f2col_ap, e1col_ap,
    e2col_ap)]; pair_args = (f1rep, B1, B2)."""
    f1rep, B1, B2 = pair_args
    if mode == 7:
        q1 = work.tile([128, 2, I], BF16, tag="q1", bufs=3)
        q2 = work.tile([128, 2, I], BF16, tag="q2", bufs=3)
        for k, (jt, mb_ap, f2c, e1c, e2c) in enumerate(subs):
            nc.vector.scalar_tensor_tensor(
                out=q1[:, k, :], in0=B1[:], scalar=e1c, in1=mb_ap,
                op0=MULT, op1=ADD)
            nc.vector.tensor_scalar_add(q2[:, k, :], mb_ap, e2c)
        p = workp.tile([128, 2, I], BF16, tag="p")
        nc.vector.tensor_tensor(out=p[:], in0=q1[:], in1=q2[:], op=MAX)
        nc.vector.tensor_scalar_max(p[:], p[:], 0.0)
        return p
    if mode == 2:
        # combine WITH f2 (STT) so prelu can run on DVE; exp is the only ACT op
        u = work.tile([128, 2, I], BF16, tag="u")
        for k, (jt, mb_ap, f2c, e1c, e2c) in enumerate(subs):
            nc.vector.scalar_tensor_tensor(
                out=u[:, k, :], in0=mb_ap, scalar=f2c, in1=f1rep[:],
                op0=ADD, op1=ADD)
        w = work.tile([128, 2, I], BF16, tag="q1", bufs=3)
        nc.vector.tensor_scalar_mul(w[:], u[:], ALPHA)     # bf16 4x
        nc.vector.tensor_tensor(out=u[:], in0=u[:], in1=w[:], op=MAX)
        p = workp.tile([128, 2, I], BF16, tag="p")
        nc.scalar.activation(p[:], u[:], ACT_EXP)
        return p
    u = work.tile([128, 2, I], BF16, tag="u")
    for k, (jt, mb_ap, f2c, e1c, e2c) in enumerate(subs):
        if mode == 4:
            nc.gpsimd.tensor_add(u[:, k, :], mb_ap, f1rep[:])
        else:
            nc.vector.tensor_tensor(out=u[:, k, :], in0=mb_ap, in1=f1rep[:],
                                    op=ADD)
    for k, (jt, mb_ap, f2c, e1c, e2c) in enumerate(subs):
        nc.scalar.activation(u[:, k, :], u[:, k, :], ACT_LRELU,
                             bias=f2c, alpha=ALPHA)
    p = workp.tile([128, 2, I], BF16, tag="p")
    nc.scalar.activation(p[:], u[:], ACT_EXP)
    return p


def _copy_engine(nc, eng, out, in_):
    if eng == "act":
        nc.scalar.activation(out, in_, ACT_COPY)
    elif eng == "gps":
        nc.gpsimd.tensor_copy(out=out, in_=in_)
    else:
        nc.vector.tensor_copy(out=out, in_=in_)


def _build_body(nc, tc, xt_d, xtl_d, mb_d, wext_d, a1rep_d, woext_d,
                wcorr_d, ident_d, outp_d, with_collective, route, cfg):
    from contextlib import ExitStack
    ctx = ExitStack()
    f1rep_eng = cfg.get("f1rep_eng", "dve")
    rbc_eng = cfg.get("rbc_eng", "act")
    assert f1rep_eng != "gps" and rbc_eng != "gps"  # GPSIMD cannot read PSUM
    
    ph0_engs = cfg.get("ph0_engs", ("act", "dve"))
    ep_v_gps = False  # GPSIMD cannot read PSUM (hT)
    with ctx:
        singles = ctx.enter_context(tc.tile_pool(name="singles", bufs=1))
        psA = ctx.enter_context(tc.tile_pool(name="psA", bufs=2, space="PSUM"))
        psB = ctx.enter_context(tc.tile_pool(name="psB", bufs=1, space="PSUM"))
        psC = ctx.enter_context(tc.tile_pool(name="psC", bufs=2, space="PSUM"))
        dram = ctx.enter_context(tc.tile_pool(name="dram", bufs=1,
                                              space="DRAM"))

        # ---------------- persistent loads ----------------
        mb_s = singles.tile([128, JT, I], BF16)
        nc.sync.dma_start(out=mb_s[:, 0:2, :],
                          in_=mb_d.ap()[0:2].rearrange("jt p i -> p jt i"))
        xtl_s = singles.tile([F, I], BF16)
        nc.sync.dma_start(out=xtl_s[:], in_=xtl_d.ap())
        a1rep_s = singles.tile([F, H, 128], BF16)
        nc.sync.dma_start(out=a1rep_s[:],
                          in_=a1rep_d.ap().rearrange("h f e -> f h e"))
        wcorr_s = singles.tile([1, C + 2], F32)
        nc.sync.dma_start(out=wcorr_s[:], in_=wcorr_d.ap())
        ident_s = singles.tile([128, 128], F32)
        nc.sync.dma_start(out=ident_s[:], in_=ident_d.ap())
        woext_raw = singles.tile([128, KT, C + 2], F32)
        nc.sync.dma_start(out=woext_raw[:],
                          in_=woext_d.ap().rearrange("k f e -> f k e"))
        woext_s = singles.tile([128, KT, C + 2], F32R)
        nc.vector.tensor_copy(out=woext_s[:], in_=woext_raw[:])

        ones_s = singles.tile([1, 128], BF16)
        nc.gpsimd.memset(ones_s[:], 1.0)
        onesf_s = singles.tile([1, 128], F32)
        nc.gpsimd.memset(onesf_s[:], 1.0)

        whbuf = singles.tile([128, H, JT, D + 1], BF16)
        nc.gpsimd.memset(whbuf[:, :, :, D:D + 1], 1.0)
        fcol = singles.tile([128, H, JT, 1], F32)
        fexp1 = singles.tile([128, H, JT, 1], F32)
        fexp2 = singles.tile([128, H, JT, 1], F32)
        hcatT = singles.tile([128, KT, I], F32R)

        xt_s = singles.tile([F, N], BF16)
        nc.sync.dma_start(out=xt_s[:], in_=xt_d.ap())
        wext_s = singles.tile([F, H, D + 2], BF16)
        nc.sync.dma_start(out=wext_s[:],
                          in_=wext_d.ap().rearrange("h f e -> f h e"))
        for j0 in (2, 6, 10):
            j1 = j0 + 4 if j0 < 10 else JT
            nc.sync.dma_start(
                out=mb_s[:, j0:j1, :],
                in_=mb_d.ap()[j0:j1].rearrange("jt p i -> p jt i"))

        work = ctx.enter_context(tc.tile_pool(name="work", bufs=4))
        workp = ctx.enter_context(tc.tile_pool(name="workp", bufs=4))
        ep1 = ctx.enter_context(tc.tile_pool(name="ep1", bufs=1))
        ep2 = ctx.enter_context(tc.tile_pool(name="ep2", bufs=2))
        epL2 = ctx.enter_context(tc.tile_pool(name="epL2", bufs=1))

        def emit_phase0_head(h):
            # Wh tiles + f columns for head h (emitted per-head so the
            # copies overlap the previous head's logit work)
            for jg in range(JT // 4):
                whp = psA.tile([128, 4, D + 2], F32, tag="ph")
                for k in range(4):
                    jt = jg * 4 + k
                    nc.tensor.matmul(whp[:, k, :],
                                     lhsT=xt_s[:, jt * 128:(jt + 1) * 128],
                                     rhs=wext_s[:, h, :])
                dst = whbuf[:, h, jg * 4:(jg + 1) * 4, 0:D]
                _copy_engine(nc, ph0_engs[jg % len(ph0_engs)],
                             dst, whp[:, :, 0:D])
                nc.vector.tensor_copy(
                    out=fcol[:, h, jg * 4:(jg + 1) * 4, :],
                    in_=whp[:, :, D + 1:D + 2])
            nc.scalar.activation(fexp1[:, h], fcol[:, h], ACT_EXP)
            nc.scalar.activation(fexp2[:, h], fcol[:, h], ACT_EXP,
                                 scale=ALPHA)

        ep_state = {}

        def emit_half_ep(hT, h):
            # per-head half-epilogue: rinv = 1/S via DVE reciprocal, SWDGE
            # partition broadcast, v-half = hT*rinv.  The odd half finishes:
            # hcat = elu(v)+1 = relu(v)+exp(min(v,0)).
            rinv = ep1.tile([1, I], F32, tag=f"ri{h % 2}", bufs=1)
            if cfg.get("dbg_ep_lnexp"):
                lnS_ = ep1.tile([1, I], F32, tag=f"ln{h % 2}", bufs=2)
                nc.scalar.activation(lnS_[:], hT[D:D + 1, :], ACT_LN)
                nc.scalar.activation(rinv[:], lnS_[:], ACT_EXP, scale=-1.0)
            else:
                nc.vector.reciprocal(rinv[:], hT[D:D + 1, :])
            if h % 2 == 0:
                v = ep1.tile([128, I], BF16, tag="v", bufs=2)
                ep_state["v"] = v
                half = slice(0, D)
            else:
                v = ep_state["v"]
                half = slice(D, 128)
            rbp = psB.tile([128, I], F32, tag="rep")
            for hf_ in range(I // 512):
                sl_ = slice(hf_ * 512, (hf_ + 1) * 512)
                nc.tensor.matmul(rbp[0:D, sl_], lhsT=onesf_s[0:1, 0:D],
                                 rhs=rinv[0:1, sl_])
            # rbc half kept at base partition 0: DVE inputs must share bases
            rbc = ep1.tile([D, I], F32, tag=f"rb{h % 2}", bufs=1)
            _copy_engine(nc, rbc_eng, rbc[:], rbp[0:D, :])
            nc.vector.tensor_tensor(out=v[half, :], in0=hT[0:D, :],
                                    in1=rbc[:], op=MULT)
            if h % 2 == 1:
                t = ep1.tile([128, I], BF16, tag="t", bufs=2)
                nc.vector.tensor_scalar_min(t[:], v[:], 0.0)
                nc.scalar.activation(t[:], t[:], ACT_EXP)
                nc.vector.scalar_tensor_tensor(
                    out=hcatT[:, h // 2, :], in0=v[:], scalar=0.0, in1=t[:],
                    op0=MAX, op1=ADD)

        def emit_head_prep(h):
            emit_phase0_head(h)
            head_modes = route[h * PAIRS_PER_HEAD:(h + 1) * PAIRS_PER_HEAD]
            need_f1rep = any(m in (2, 3, 4) for m in head_modes)
            need_B = any(m == 7 for m in head_modes)
            f1p = psB.tile([128, I], F32, tag="rep")
            for hf in range(I // 512):
                sl = slice(hf * 512, (hf + 1) * 512)
                nc.tensor.matmul(f1p[:, sl], lhsT=a1rep_s[:, h, :],
                                 rhs=xtl_s[:, sl])
            f1rep_s = B1_s = None
            B2_s = True  # unused (q2 needs only the per-partition scalar)
            if need_f1rep:
                f1rep_s = ep2.tile([128, I], BF16, tag="f1rep")
                _copy_engine(nc, f1rep_eng, f1rep_s[:], f1p[:])
            if need_B:
                B1_s = ep2.tile([128, I], BF16, tag="B1")
                nc.scalar.activation(B1_s[:], f1p[:], ACT_EXP, scale=1.0 - ALPHA)
            return f1rep_s, B1_s, B2_s

        # ---------------- layer 1 ----------------
        pending_ep = None   # deferred half-epilogue (software pipelining)
        preps = emit_head_prep(0)
        for h in range(H):
            head_modes = route[h * PAIRS_PER_HEAD:(h + 1) * PAIRS_PER_HEAD]
            cur = preps
            hT = psC.tile([D + 1, I], F32, tag="acc")
            for jp in range(PAIRS_PER_HEAD):
                if jp == int(cfg.get("ep_defer", 2)) and pending_ep is not None:
                    pending_ep()
                    pending_ep = None
                if jp == int(cfg.get("prep_at", 4)) and h + 1 < H:
                    preps = emit_head_prep(h + 1)
                mode = head_modes[jp]
                subs = []
                for k in range(2):
                    jt = jp * 2 + k
                    subs.append((jt, mb_s[:, jt, :],
                                 fcol[:, h, jt, :], fexp1[:, h, jt, :],
                                 fexp2[:, h, jt, :]))
                p = _emit_pair(nc, work, workp, mode, subs, cur)
                for k in range(2):
                    jt = jp * 2 + k
                    for hf in range(I // 512):
                        sl = slice(hf * 512, (hf + 1) * 512)
                        nc.tensor.matmul(hT[:, sl],
                                         lhsT=whbuf[:, h, jt, :],
                                         rhs=p[:, k, sl],
                                         start=(jt == 0), stop=(jt == JT - 1))

            pending_ep = (lambda t_=hT, h_=h: emit_half_ep(t_, h_))
        if pending_ep is not None:
            pending_ep()
            pending_ep = None

        # ---------------- layer 2 projection + gather (bf16 payload) ------
        wh2loc = singles.tile([128, IC, C + 2], F32)
        gin = dram.tile([I, C + 2], F32)
        for ic in range(IC):
            w2p = psA.tile([128, 4, D + 2], F32, tag="ph")
            for kt in range(KT):
                nc.tensor.matmul(
                    w2p[:, 0, 0:C + 2],
                    lhsT=hcatT[:, kt, ic * 128:(ic + 1) * 128],
                    rhs=woext_s[:, kt, :],
                    start=(kt == 0), stop=False)
            nc.tensor.matmul(w2p[:, 0, 0:C + 2], lhsT=onesf_s[0:1, :],
                             rhs=wcorr_s[:], start=False, stop=True)
            nc.vector.tensor_copy(out=wh2loc[:, ic, :], in_=w2p[:, 0, 0:C + 2])
        nc.sync.dma_start(
            out=gin.rearrange("(ic p) c -> p ic c", p=128),
            in_=wh2loc[:])

        gout = dram.tile([N, C + 2], F32)
        if with_collective:
            nc.gpsimd.collective_compute(
                "AllGather", mybir.AluOpType.bypass,
                replica_groups=REPLICA_GROUPS,
                ins=[gin.opt()], outs=[gout.opt()])
        else:  # timing-model variant: fake the exchange with two local copies
            nc.sync.dma_start(out=gout[0:I, :], in_=gin[:])
            nc.sync.dma_start(out=gout[I:N, :], in_=gin[:])

        # g1 row (local queries) -> replicated [128, I]
        g1rowp = psB.tile([128, I], F32, tag="rep")
        for ic in range(IC):
            nc.tensor.transpose(g1rowp[0:1, ic * 128:(ic + 1) * 128],
                                in_=wh2loc[:, ic, 0:1], identity=ident_s[:])
        g1row_s = epL2.tile([1, I], BF16, tag="g1row")
        nc.scalar.activation(g1row_s[:], g1rowp[0:1, :], ACT_COPY)
        g1rp = psB.tile([128, I], F32, tag="rep")
        for hf in range(I // 512):
            sl = slice(hf * 512, (hf + 1) * 512)
            nc.tensor.matmul(g1rp[:, sl], lhsT=ones_s[0:1, :],
                             rhs=g1row_s[0:1, sl])
        l2_modes = route[H * PAIRS_PER_HEAD:]
        g1rep_s = B1L2 = None
        B2L2 = True
        if any(m in (2, 3, 4) for m in l2_modes):
            g1rep_s = singles.tile([128, I], BF16)
            nc.vector.tensor_copy(out=g1rep_s[:], in_=g1rp[:])
        if any(m == 7 for m in l2_modes):
            B1L2 = singles.tile([128, I], BF16)
            nc.scalar.activation(B1L2[:], g1rp[:], ACT_EXP, scale=1.0 - ALPHA)

        # gathered rows: [g1, g2, Wh2(32)] f32 staged, bf16 for the PV lhsT
        wh2tmp = singles.tile([128, JT, C + 2], F32)
        nc.sync.dma_start(
            out=wh2tmp[:],
            in_=gout.rearrange("(jt p) c -> p jt c", p=128))
        wh2gr = singles.tile([128, JT, C + 3], BF16)
        nc.gpsimd.memset(wh2gr[:, :, C + 2:C + 3], 1.0)
        its1 = singles.tile([128, JT, 1], F32)
        its2 = singles.tile([128, JT, 1], F32)
        for jg in range(JT // 4):
            s4 = slice(jg * 4, (jg + 1) * 4)
            nc.gpsimd.tensor_copy(out=wh2gr[:, s4, 0:C + 2],
                                  in_=wh2tmp[:, s4, :])
            nc.scalar.activation(its1[:, s4, :], wh2tmp[:, s4, 1:2], ACT_EXP)
            nc.scalar.activation(its2[:, s4, :], wh2tmp[:, s4, 1:2], ACT_EXP,
                                 scale=ALPHA)

        # ---------------- layer 2 attention ----------------
        # hoist the gather-independent combines (mask + g1rep) so DVE/GPS
        # work while the AllGather is still in flight; reuse the idle q1/q2
        # rings so the main u-ring keeps flowing
        l2_u = {}
        hoist_tags = ["q1", "q1", "q2", "q2"]
        for jp in range(PAIRS_PER_HEAD):
            if l2_modes[jp] not in (3, 4) or not hoist_tags:
                continue
            u = work.tile([128, 2, I], BF16, tag=hoist_tags.pop(0), bufs=3)
            for k in range(2):
                jt = jp * 2 + k
                if l2_modes[jp] == 4:
                    nc.gpsimd.tensor_add(u[:, k, :], mb_s[:, jt, :],
                                         g1rep_s[:])
                else:
                    nc.vector.tensor_tensor(out=u[:, k, :],
                                            in0=mb_s[:, jt, :],
                                            in1=g1rep_s[:], op=ADD)
            l2_u[jp] = u

        o2T = psC.tile([D + 1, I], F32, tag="acc")
        for jp in range(PAIRS_PER_HEAD):
            mode = l2_modes[jp]
            subs = []
            for k in range(2):
                jt = jp * 2 + k
                subs.append((jt, mb_s[:, jt, :],
                             wh2tmp[:, jt, 1:2], its1[:, jt, :],
                             its2[:, jt, :]))
            if jp in l2_u:
                u = l2_u[jp]
                for k, (jt, mb_ap, f2c, e1c, e2c) in enumerate(subs):
                    nc.scalar.activation(u[:, k, :], u[:, k, :], ACT_LRELU,
                                         bias=f2c, alpha=ALPHA)
                p = workp.tile([128, 2, I], BF16, tag="p")
                nc.scalar.activation(p[:], u[:], ACT_EXP)
            else:
                p = _emit_pair(nc, work, workp, mode, subs,
                               (g1rep_s, B1L2, B2L2))
            for k in range(2):
                jt = jp * 2 + k
                for hf in range(I // 512):
                    sl = slice(hf * 512, (hf + 1) * 512)
                    nc.tensor.matmul(o2T[0:C + 1, sl],
                                     lhsT=wh2gr[:, jt, 2:C + 3],
                                     rhs=p[:, k, sl],
                                     start=(jt == 0), stop=(jt == JT - 1))

        # ---------------- finalize (transposed: per-query reciprocal) -----
        if cfg.get("dbg_simple_fin"):
            r2ln = epL2.tile([1, I], F32, tag="lnS2")
            nc.scalar.activation(r2ln[:], o2T[C:C + 1, :], ACT_LN)
            r2 = epL2.tile([1, I], BF16, tag="r2")
            nc.scalar.activation(r2[:], r2ln[:], ACT_EXP, scale=-1.0)
            rbc2p = psB.tile([128, I], F32, tag="rep")
            for hf in range(I // 512):
                sl = slice(hf * 512, (hf + 1) * 512)
                nc.tensor.matmul(rbc2p[0:C, sl], lhsT=ones_s[0:1, 0:C],
                                 rhs=r2[0:1, sl])
            rbc2_s = epL2.tile([C, I], F32, tag="rbc2")
            nc.vector.tensor_copy(out=rbc2_s[:], in_=rbc2p[0:C, :])
            oT_s = epL2.tile([C, I], F32, tag="oT")
            nc.vector.tensor_tensor(out=oT_s[:], in0=o2T[0:C, :],
                                    in1=rbc2_s[:], op=MULT)
            for k in range(IC):
                ofp = psA.tile([128, 4, D + 2], F32, tag="ph")
                nc.tensor.transpose(ofp[:, 0, 0:C],
                                    in_=oT_s[:, k * 128:(k + 1) * 128],
                                    identity=ident_s[0:C, 0:C])
                ofs = ep2.tile([128, C], F32, tag="ofs")
                nc.vector.tensor_copy(out=ofs[:], in_=ofp[:, 0, 0:C])
                nc.sync.dma_start(out=outp_d.ap()[k * 128:(k + 1) * 128, :],
                                  in_=ofs[:])
        else:
            o2s = epL2.tile([C, I], F32, tag="o2s")
            nc.vector.tensor_copy(out=o2s[:], in_=o2T[0:C, :])
            o2r = epL2.tile([1, I], F32, tag="o2r")
            nc.scalar.activation(o2r[:], o2T[C:C + 1, :], ACT_COPY)
            for k in range(IC):
                ck = slice(k * 128, (k + 1) * 128)
                ofp = psA.tile([128, 4, D + 2], F32, tag="ph")
                nc.tensor.transpose(ofp[:, 0, 0:C], in_=o2s[:, ck],
                                    identity=ident_s[0:C, 0:C])
                ofq = psA.tile([128, 4, D + 2], F32, tag="ph")
                nc.tensor.transpose(ofq[:, 0, 0:1], in_=o2r[:, ck],
                                    identity=ident_s[0:1, 0:1])
                s2t = ep2.tile([128, 1], F32, tag="s2t")
                nc.vector.reciprocal(s2t[:], ofq[:, 0, 0:1])
                ofs = ep2.tile([128, C], F32, tag="ofs")
                nc.vector.tensor_scalar_mul(ofs[:], ofp[:, 0, 0:C], s2t[:])
                nc.sync.dma_start(out=outp_d.ap()[k * 128:(k + 1) * 128, :],
                                  in_=ofs[:])


# --------------------------------------------------------------------------
# host side
# --------------------------------------------------------------------------

def shard_inputs(x, adj, W, a1, a2, Wo, ao1, ao2):
    x = np.asarray(x, np.float32)
    adj = np.asarray(adj)
    W = np.asarray(W, np.float32)
    a1 = np.asarray(a1, np.float32)
    a2 = np.asarray(a2, np.float32)
    Wo = np.asarray(Wo, np.float32)
    ao1 = np.asarray(ao1, np.float32)
    ao2 = np.asarray(ao2, np.float32)
    BF = ml_dtypes.bfloat16

    wvec1 = np.einsum("hfd,hd->hf", W, a1)          # [H, F]
    wvec2 = np.einsum("hfd,hd->hf", W, a2)
    wext = np.concatenate([W, wvec1[:, :, None], wvec2[:, :, None]],
                          axis=2).astype(BF)
    a1rep = np.repeat(wvec1[:, :, None], 128, axis=2).astype(BF)
    wo1 = Wo @ ao1                                   # [512]
    wo2 = Wo @ ao2
    woflat = np.concatenate([wo1[:, None], wo2[:, None], Wo], 1)  # [512, 34]
    woext = woflat.reshape(KT, 128, C + 2).astype(np.float32)
    wcorr = (-woflat.sum(0))[None, :].astype(np.float32)
    ident = np.eye(128, dtype=np.float32)

    in_maps = []
    for c in range(N_CORES):
        b, half = c // 2, c % 2
        i0 = half * I
        xt = np.ascontiguousarray(x[b].T).astype(BF)   # [F, N]
        xtl = np.ascontiguousarray(xt[:, i0:i0 + I])
        adjt = adj[b, i0:i0 + I, :].T                # [N, I] = (j, i)
        mb = np.where(adjt > 0, np.float32(0.0), np.float32(-BIG))
        mb = np.ascontiguousarray(mb.reshape(JT, 128, I)).astype(BF)
        in_maps.append({
            "xt": xt, "xtl": xtl, "mb": mb, "wext": wext,
            "a1rep": a1rep, "woext": woext, "wcorr": wcorr, "ident": ident,
        })
    return in_maps


# Engine routing chosen by cost-model sweep (TimelineSim).
DEFAULT_CFG = {"p4": 43, "p7": 27, "ep_defer": 2, "f1rep_eng": "dve",
               "rbc_eng": "dve", "prep_at": 3,
               "l2route": [4, 4, 7, 7, 4, 4, 2, 2]}

_CACHE = {}


def _program():
    if "nc" not in _CACHE:
        _CACHE["nc"] = build_program(with_collective=True, cfg=DEFAULT_CFG)
    return _CACHE["nc"]


def kernel(**inputs):
    nc = _program()
    in_maps = shard_inputs(**inputs)
    res = run_bass_kernel_spmd(nc, in_maps, list(range(N_CORES)))
    _CACHE["last_results"] = res
    out = np.empty((B, N, C), np.float32)
    for c in range(N_CORES):
        b, half = c // 2, c % 2
        out[b, half * I:(half + 1) * I, :] = res.results[c]["outp"]
    return out


# revision 44
# speedup vs baseline: 1.4823x; 1.0002x over previous
"""GAT forward (2-layer graph attention, B=4 N=2048 F=128 H=8 D=64 C=32)
as a Bass/Tile SPMD kernel on 8 Trainium2 NeuronCores.

Sharding: core c -> (batch b=c//2, query-row half c%2).  Each core computes
attention for its 1024 query rows over all 2048 keys for all 8 heads
(layer 1) and for the output head (layer 2).  The only cross-core exchange
is a 2-rank AllGather of the layer-2 projections [g1|g2|Wh2] ([1024,34] f32)
within each (2b, 2b+1) pair.

Layout: attention logits are built TRANSPOSED, e^T[j (keys) = partitions,
i (queries) = free], so the PV matmul needs no operand transposes
(lhsT = Wh[j,d] stationary, rhs = p[j,i] moving, out = h^T[d,i]) and
softmax row sums come from a ones-column appended to Wh (PSUM row D).

The N^2 elementwise work (mask+f1+f2, leaky-relu, exp over 144 [128,1024]
logit tiles) is the bottleneck; everything runs in bf16 (2x DVE tensor-
tensor throughput; ACT is dtype-independent) and every logit PAIR of tiles
is routed down one of three pipelines to saturate ACT+DVE+GPSIMD jointly:

  P3 : u = mb + f1rep       (DVE tensor_tensor, bf16 2x)
       z = Prelu(u + f2col) (ACT, per-sub bias)     p = Exp(z)  (ACT)
  P4 : same but the combine runs on GPSIMD tensor_add
  P2 : STT combine (incl f2) + DVE prelu; exp is the only ACT op
  P7 : exp(prelu(u)) == max(exp(u), exp(0.2u)); exps factor rank-1 and,
       after a softmax-invariant per-query rescale by exp(-0.2 f1):
       q1 = exp(.8 f1_i)*exp(f2_j) + mb     (DVE STT mult,add)
       q2 = exp(.2 f2_j) + mb               (DVE tensor-scalar add)
       p  = relu(max(q1, q2))               (DVE bf16 2x ops; no ACT!)
       (mb = -1e15 masked / 0 unmasked dominates both branches exactly)

Softmax division is deferred to the per-head-pair epilogue (1/S via Ln/Exp
of the row-sum), fused with ELU via elu(v)+1 = relu(v) + exp(min(v,0)),
the +1 folded into a rank-1 correction matmul of the layer-2 projection.
"""

import numpy as np
import ml_dtypes

import concourse.bass as bass
import concourse.tile as tile
from concourse import mybir
from concourse.bass_utils import run_bass_kernel_spmd

F32 = mybir.dt.float32
F32R = mybir.dt.float32r
BF16 = mybir.dt.bfloat16

B, N, F, H, D, C = 4, 2048, 128, 8, 64, 32
I = N // 2          # query rows per core
JT = N // 128       # key tiles
IC = I // 128       # query-row 128-chunks per core
KT = (H * D) // 128 # hidden-dim 128-chunks
ALPHA = 0.2
BIG = 1e15          # mask bias; also dominates A*B in the factored path
N_CORES = 8
REPLICA_GROUPS = [[0, 1], [2, 3], [4, 5], [6, 7]]

ADD = mybir.AluOpType.add
MAX = mybir.AluOpType.max
MULT = mybir.AluOpType.mult
# NOTE: hardware "Lrelu" has a fixed 0.01 slope and ignores alpha;
# "Prelu" honors alpha (verified on HW) — it is the configurable leaky relu.
ACT_LRELU = mybir.ActivationFunctionType.Prelu
ACT_EXP = mybir.ActivationFunctionType.Exp
ACT_LN = mybir.ActivationFunctionType.Ln
ACT_COPY = mybir.ActivationFunctionType.Copy

N_HEADS_ALL = H + 1          # 8 layer-1 heads + the layer-2 output head
PAIRS_PER_HEAD = JT // 2     # 8
N_PAIRS = N_HEADS_ALL * PAIRS_PER_HEAD  # 72


def _split_multiwaits(nc):
    """Pinned walrus accepts only one sync-wait per instruction; Tile's exit
    drain (and occasionally others) carries several.  Hoist extras onto
    single-wait Drains on the same engine immediately before the owner."""
    n_fixed = 0
    for fn in nc.m.functions:
        for bb in fn.blocks:
            for name in [i.name for i in bb.instructions]:
                idx = [i.name for i in bb.instructions].index(name)
                inst = bb.instructions[idx]
                si = inst.sync_info
                if si is None or len(si.on_wait) <= 1:
                    continue
                waits = list(si.on_wait)
                for k, w in enumerate(waits[:-1]):
                    nd = mybir.InstDrain(
                        name=f"waitfix-{inst.name}-{k}", ins=[], outs=[])
                    nd.engine = inst.engine
                    nd.sync_info = mybir.SyncInfo(on_wait=[w], on_update=[])
                    nc.register_instruction(nd, overwrite=True)
                    bb.instructions.insert(idx + k, nd)
                inst.sync_info = mybir.SyncInfo(
                    on_wait=waits[-1:], on_update=list(si.on_update))
                n_fixed += 1
    return n_fixed


def _make_route(cfg):
    """Per-pair pipeline assignment:
      3 (DVE-add + ACT prelu/exp), 4 (GPS-add + ACT prelu/exp),
      7 (factored rank-1 exps on DVE + GPS max, no ACT),
      2 (DVE STT-add + DVE prelu + ACT exp only).
    Counts spread across the 9 'heads' (8 L1 + L2); p7 optionally
    concentrated into few heads (fewer exp(f1)-prep ops on ACT)."""
    if "route" in cfg:                      # explicit per-pair override
        route = list(cfg["route"])
        assert len(route) == N_PAIRS
        return route
    n4 = int(cfg.get("p4", 24))
    n7 = int(cfg.get("p7", 33))
    n2 = int(cfg.get("p2", 0))
    n3 = N_PAIRS - n4 - n7 - n2
    assert n3 >= 0
    conc = int(cfg.get("p7_conc", 0))  # 0 = spread; 1 = concentrate P7
    per_head = [[0, 0, 0, 0] for _ in range(N_HEADS_ALL)]  # [n3, n4, n7, n2]
    if conc:
        # fill P7 into heads round-robin starting at head 1, PAIRS_PER_HEAD
        # per head; distribute the rest evenly over remaining slots
        order = [1, 3, 5, 7, 8, 0, 2, 4, 6]
        left7 = n7
        for hh in order:
            take = min(left7, PAIRS_PER_HEAD)
            per_head[hh][2] = take
            left7 -= take
            if left7 == 0:
                break
        slots = [PAIRS_PER_HEAD - per_head[hh][2] for hh in range(N_HEADS_ALL)]
        for idx, cnt in ((1, n4), (3, n2), (0, n3)):
            left = cnt
            while left > 0:
                done = True
                for hh in range(N_HEADS_ALL):
                    used = sum(per_head[hh])
                    if used < PAIRS_PER_HEAD and left > 0:
                        per_head[hh][idx] += 1
                        left -= 1
                        done = False
                if done:
                    break
    else:
        for idx, cnt in enumerate((n3, n4, n7, n2)):
            base, rem = divmod(cnt, N_HEADS_ALL)
            for hh in range(N_HEADS_ALL):
                per_head[hh][idx] = base + (1 if hh < rem else 0)
        for hh in range(N_HEADS_ALL):
            while sum(per_head[hh]) > PAIRS_PER_HEAD:
                per_head[hh][int(np.argmax(per_head[hh]))] -= 1
            while sum(per_head[hh]) < PAIRS_PER_HEAD:
                per_head[hh][int(np.argmin(per_head[hh]))] += 1
    if "l2route" in cfg:
        l2r = list(cfg["l2route"])
        assert len(l2r) == PAIRS_PER_HEAD
    else:
        l2r = None
    route = []
    for hh in range(N_HEADS_ALL):
        if hh == H and l2r is not None:
            route.extend(l2r)
            continue
        c3, c4, c7, c2 = per_head[hh]
        pool = [7] * c7 + [4] * c4 + [3] * c3 + [2] * c2
        mix, lo, hi = [], 0, len(pool) - 1
        toggle = True
        while lo <= hi:
            if toggle:
                mix.append(pool[lo]); lo += 1
            else:
                mix.append(pool[hi]); hi -= 1
            toggle = not toggle
        route.extend(mix)
    return route


def build_program(with_collective=True, cfg=None, repeat=1):
    cfg = dict(cfg or {})
    QB[0] = int(cfg.get("qbufs", 3))
    route = _make_route(cfg)

    nc = bass.Bass("TRN2", target_bir_lowering=False, debug=False,
                   enable_asserts=False, num_devices=N_CORES)

    xt_d = nc.dram_tensor("xt", [F, N], BF16, kind="ExternalInput")
    xtl_d = nc.dram_tensor("xtl", [F, I], BF16, kind="ExternalInput")
    mb_d = nc.dram_tensor("mb", [JT, 128, I], BF16, kind="ExternalInput")
    wext_d = nc.dram_tensor("wext", [H, F, D + 2], BF16, kind="ExternalInput")
    a1rep_d = nc.dram_tensor("a1rep", [H, F, 128], BF16, kind="ExternalInput")
    woext_d = nc.dram_tensor("woext", [KT, 128, C + 2], F32,
                             kind="ExternalInput")
    wcorr_d = nc.dram_tensor("wcorr", [1, C + 2], F32, kind="ExternalInput")
    ident_d = nc.dram_tensor("ident", [128, 128], F32, kind="ExternalInput")
    outp_d = nc.dram_tensor("outp", [I, C], F32, kind="ExternalOutput")

    with tile.TileContext(nc) as tc:
        if repeat > 1:
            def body(iv, unroll=None):
                _build_body(nc, tc, xt_d, xtl_d, mb_d, wext_d, a1rep_d,
                            woext_d, wcorr_d, ident_d, outp_d,
                            with_collective, route, cfg)
            with tc.For_i(0, repeat, 1) as iv:
                body(iv)
        else:
            _build_body(nc, tc, xt_d, xtl_d, mb_d, wext_d, a1rep_d,
                        woext_d, wcorr_d, ident_d, outp_d,
                        with_collective, route, cfg)
    _split_multiwaits(nc)
    return nc


QB = [3]


def _emit_pair(nc, work, workp, mode, subs, pair_args):
    """Emit one logit pair.  subs = [(jt, mb_ap, mq_ap, f2col_ap, e1col_ap,
    e2col_ap)]; pair_args = (f1rep, B1, B2)."""
    f1rep, B1, B2 = pair_args
    if mode == 7:
        q1 = work.tile([128, 2, I], BF16, tag="q1", bufs=QB[0])
        q2 = work.tile([128, 2, I], BF16, tag="q2", bufs=QB[0])
        for k, (jt, mb_ap, f2c, e1c, e2c) in enumerate(subs):
            nc.vector.scalar_tensor_tensor(
                out=q1[:, k, :], in0=B1[:], scalar=e1c, in1=mb_ap,
                op0=MULT, op1=ADD)
            nc.vector.tensor_scalar_add(q2[:, k, :], mb_ap, e2c)
        p = workp.tile([128, 2, I], BF16, tag="p")
        nc.vector.tensor_tensor(out=p[:], in0=q1[:], in1=q2[:], op=MAX)
        nc.vector.tensor_scalar_max(p[:], p[:], 0.0)
        return p
    if mode == 2:
        # combine WITH f2 (STT) so prelu can run on DVE; exp is the only ACT op
        u = work.tile([128, 2, I], BF16, tag="u")
        for k, (jt, mb_ap, f2c, e1c, e2c) in enumerate(subs):
            nc.vector.scalar_tensor_tensor(
                out=u[:, k, :], in0=mb_ap, scalar=f2c, in1=f1rep[:],
                op0=ADD, op1=ADD)
        w = work.tile([128, 2, I], BF16, tag="q1", bufs=QB[0])
        nc.vector.tensor_scalar_mul(w[:], u[:], ALPHA)     # bf16 4x
        nc.vector.tensor_tensor(out=u[:], in0=u[:], in1=w[:], op=MAX)
        p = workp.tile([128, 2, I], BF16, tag="p")
        nc.scalar.activation(p[:], u[:], ACT_EXP)
        return p
    u = work.tile([128, 2, I], BF16, tag="u")
    for k, (jt, mb_ap, f2c, e1c, e2c) in enumerate(subs):
        if mode == 4:
            nc.gpsimd.tensor_add(u[:, k, :], mb_ap, f1rep[:])
        else:
            nc.vector.tensor_tensor(out=u[:, k, :], in0=mb_ap, in1=f1rep[:],
                                    op=ADD)
    for k, (jt, mb_ap, f2c, e1c, e2c) in enumerate(subs):
        nc.scalar.activation(u[:, k, :], u[:, k, :], ACT_LRELU,
                             bias=f2c, alpha=ALPHA)
    p = workp.tile([128, 2, I], BF16, tag="p")
    nc.scalar.activation(p[:], u[:], ACT_EXP)
    return p


def _copy_engine(nc, eng, out, in_):
    if eng == "act":
        nc.scalar.activation(out, in_, ACT_COPY)
    elif eng == "gps":
        nc.gpsimd.tensor_copy(out=out, in_=in_)
    else:
        nc.vector.tensor_copy(out=out, in_=in_)


def _build_body(nc, tc, xt_d, xtl_d, mb_d, wext_d, a1rep_d, woext_d,
                wcorr_d, ident_d, outp_d, with_collective, route, cfg):
    from contextlib import ExitStack
    ctx = ExitStack()
    f1rep_eng = cfg.get("f1rep_eng", "dve")
    rbc_eng = cfg.get("rbc_eng", "act")
    assert f1rep_eng != "gps" and rbc_eng != "gps"  # GPSIMD cannot read PSUM
    
    ph0_engs = cfg.get("ph0_engs", ("act", "dve"))
    ep_v_gps = False  # GPSIMD cannot read PSUM (hT)
    with ctx:
        singles = ctx.enter_context(tc.tile_pool(name="singles", bufs=1))
        psA = ctx.enter_context(tc.tile_pool(name="psA", bufs=2, space="PSUM"))
        psB = ctx.enter_context(tc.tile_pool(name="psB", bufs=1, space="PSUM"))
        psC = ctx.enter_context(tc.tile_pool(name="psC", bufs=2, space="PSUM"))
        dram = ctx.enter_context(tc.tile_pool(name="dram", bufs=1,
                                              space="DRAM"))

        # ---------------- persistent loads ----------------
        mb_s = singles.tile([128, JT, I], BF16)
        nc.sync.dma_start(out=mb_s[:, 0:2, :],
                          in_=mb_d.ap()[0:2].rearrange("jt p i -> p jt i"))
        xtl_s = singles.tile([F, I], BF16)
        nc.sync.dma_start(out=xtl_s[:], in_=xtl_d.ap())
        a1rep_s = singles.tile([F, H, 128], BF16)
        nc.sync.dma_start(out=a1rep_s[:],
                          in_=a1rep_d.ap().rearrange("h f e -> f h e"))
        wcorr_s = singles.tile([1, C + 2], F32)
        nc.sync.dma_start(out=wcorr_s[:], in_=wcorr_d.ap())
        ident_s = singles.tile([128, 128], F32)
        nc.sync.dma_start(out=ident_s[:], in_=ident_d.ap())
        woext_raw = singles.tile([128, KT, C + 2], F32)
        nc.sync.dma_start(out=woext_raw[:],
                          in_=woext_d.ap().rearrange("k f e -> f k e"))
        woext_s = singles.tile([128, KT, C + 2], F32R)
        nc.vector.tensor_copy(out=woext_s[:], in_=woext_raw[:])

        ones_s = singles.tile([1, 128], BF16)
        nc.gpsimd.memset(ones_s[:], 1.0)
        onesf_s = singles.tile([1, 128], F32)
        nc.gpsimd.memset(onesf_s[:], 1.0)

        whbuf = singles.tile([128, H, JT, D + 1], BF16)
        nc.gpsimd.memset(whbuf[:, :, :, D:D + 1], 1.0)
        fcol = singles.tile([128, H, JT, 1], F32)
        fexp1 = singles.tile([128, H, JT, 1], F32)
        fexp2 = singles.tile([128, H, JT, 1], F32)
        hcatT = singles.tile([128, KT, I], F32R)

        xt_s = singles.tile([F, N], BF16)
        nc.sync.dma_start(out=xt_s[:], in_=xt_d.ap())
        wext_s = singles.tile([F, H, D + 2], BF16)
        nc.sync.dma_start(out=wext_s[:],
                          in_=wext_d.ap().rearrange("h f e -> f h e"))
        for j0 in (2, 6, 10):
            j1 = j0 + 4 if j0 < 10 else JT
            nc.sync.dma_start(
                out=mb_s[:, j0:j1, :],
                in_=mb_d.ap()[j0:j1].rearrange("jt p i -> p jt i"))

        work = ctx.enter_context(tc.tile_pool(name="work", bufs=4))
        workp = ctx.enter_context(tc.tile_pool(name="workp", bufs=4))
        ep1 = ctx.enter_context(tc.tile_pool(name="ep1", bufs=1))
        ep2 = ctx.enter_context(tc.tile_pool(name="ep2", bufs=2))
        epL2 = ctx.enter_context(tc.tile_pool(name="epL2", bufs=1))

        def emit_phase0_head(h):
            # Wh tiles + f columns for head h (emitted per-head so the
            # copies overlap the previous head's logit work)
            for jg in range(JT // 4):
                whp = psA.tile([128, 4, D + 2], F32, tag="ph")
                for k in range(4):
                    jt = jg * 4 + k
                    nc.tensor.matmul(whp[:, k, :],
                                     lhsT=xt_s[:, jt * 128:(jt + 1) * 128],
                                     rhs=wext_s[:, h, :])
                dst = whbuf[:, h, jg * 4:(jg + 1) * 4, 0:D]
                _copy_engine(nc, ph0_engs[jg % len(ph0_engs)],
                             dst, whp[:, :, 0:D])
                nc.vector.tensor_copy(
                    out=fcol[:, h, jg * 4:(jg + 1) * 4, :],
                    in_=whp[:, :, D + 1:D + 2])
            nc.scalar.activation(fexp1[:, h], fcol[:, h], ACT_EXP)
            nc.scalar.activation(fexp2[:, h], fcol[:, h], ACT_EXP,
                                 scale=ALPHA)

        ep_state = {}

        def emit_half_ep(hT, h):
            # per-head half-epilogue: rinv = 1/S via DVE reciprocal, SWDGE
            # partition broadcast, v-half = hT*rinv.  The odd half finishes:
            # hcat = elu(v)+1 = relu(v)+exp(min(v,0)).
            rinv = ep1.tile([1, I], F32, tag=f"ri{h % 2}", bufs=1)
            if cfg.get("dbg_ep_lnexp"):
                lnS_ = ep1.tile([1, I], F32, tag=f"ln{h % 2}", bufs=2)
                nc.scalar.activation(lnS_[:], hT[D:D + 1, :], ACT_LN)
                nc.scalar.activation(rinv[:], lnS_[:], ACT_EXP, scale=-1.0)
            else:
                nc.vector.reciprocal(rinv[:], hT[D:D + 1, :])
            if h % 2 == 0:
                v = ep1.tile([128, I], BF16, tag="v", bufs=2)
                ep_state["v"] = v
                half = slice(0, D)
            else:
                v = ep_state["v"]
                half = slice(D, 128)
            rbp = psB.tile([128, I], F32, tag="rep")
            for hf_ in range(I // 512):
                sl_ = slice(hf_ * 512, (hf_ + 1) * 512)
                nc.tensor.matmul(rbp[0:D, sl_], lhsT=onesf_s[0:1, 0:D],
                                 rhs=rinv[0:1, sl_])
            # rbc half kept at base partition 0: DVE inputs must share bases
            rbc = ep1.tile([D, I], F32, tag=f"rb{h % 2}", bufs=1)
            _copy_engine(nc, rbc_eng, rbc[:], rbp[0:D, :])
            nc.vector.tensor_tensor(out=v[half, :], in0=hT[0:D, :],
                                    in1=rbc[:], op=MULT)
            if h % 2 == 1:
                t = ep1.tile([128, I], BF16, tag="t", bufs=2)
                nc.vector.tensor_scalar_min(t[:], v[:], 0.0)
                nc.scalar.activation(t[:], t[:], ACT_EXP)
                nc.vector.scalar_tensor_tensor(
                    out=hcatT[:, h // 2, :], in0=v[:], scalar=0.0, in1=t[:],
                    op0=MAX, op1=ADD)

        def emit_head_prep(h):
            emit_phase0_head(h)
            head_modes = route[h * PAIRS_PER_HEAD:(h + 1) * PAIRS_PER_HEAD]
            need_f1rep = any(m in (2, 3, 4) for m in head_modes)
            need_B = any(m == 7 for m in head_modes)
            f1p = psB.tile([128, I], F32, tag="rep")
            for hf in range(I // 512):
                sl = slice(hf * 512, (hf + 1) * 512)
                nc.tensor.matmul(f1p[:, sl], lhsT=a1rep_s[:, h, :],
                                 rhs=xtl_s[:, sl])
            f1rep_s = B1_s = None
            B2_s = True  # unused (q2 needs only the per-partition scalar)
            if need_f1rep:
                f1rep_s = ep2.tile([128, I], BF16, tag="f1rep")
                _copy_engine(nc, f1rep_eng, f1rep_s[:], f1p[:])
            if need_B:
                B1_s = ep2.tile([128, I], BF16, tag="B1")
                nc.scalar.activation(B1_s[:], f1p[:], ACT_EXP, scale=1.0 - ALPHA)
            return f1rep_s, B1_s, B2_s

        # ---------------- layer 1 ----------------
        pending_ep = None   # deferred half-epilogue (software pipelining)
        preps = emit_head_prep(0)
        for h in range(H):
            head_modes = route[h * PAIRS_PER_HEAD:(h + 1) * PAIRS_PER_HEAD]
            cur = preps
            hT = psC.tile([D + 1, I], F32, tag="acc")
            for jp in range(PAIRS_PER_HEAD):
                if jp == int(cfg.get("ep_defer", 2)) and pending_ep is not None:
                    pending_ep()
                    pending_ep = None
                if jp == int(cfg.get("prep_at", 4)) and h + 1 < H:
                    preps = emit_head_prep(h + 1)
                mode = head_modes[jp]
                subs = []
                for k in range(2):
                    jt = jp * 2 + k
                    subs.append((jt, mb_s[:, jt, :],
                                 fcol[:, h, jt, :], fexp1[:, h, jt, :],
                                 fexp2[:, h, jt, :]))
                p = _emit_pair(nc, work, workp, mode, subs, cur)
                for k in range(2):
                    jt = jp * 2 + k
                    for hf in range(I // 512):
                        sl = slice(hf * 512, (hf + 1) * 512)
                        nc.tensor.matmul(hT[:, sl],
                                         lhsT=whbuf[:, h, jt, :],
                                         rhs=p[:, k, sl],
                                         start=(jt == 0), stop=(jt == JT - 1))

            pending_ep = (lambda t_=hT, h_=h: emit_half_ep(t_, h_))
        if pending_ep is not None:
            pending_ep()
            pending_ep = None

        # ---------------- layer 2 projection + gather (bf16 payload) ------
        wh2loc = singles.tile([128, IC, C + 2], F32)
        gin = dram.tile([I, C + 2], F32)
        for ic in range(IC):
            w2p = psA.tile([128, 4, D + 2], F32, tag="ph")
            for kt in range(KT):
                nc.tensor.matmul(
                    w2p[:, 0, 0:C + 2],
                    lhsT=hcatT[:, kt, ic * 128:(ic + 1) * 128],
                    rhs=woext_s[:, kt, :],
                    start=(kt == 0), stop=False)
            nc.tensor.matmul(w2p[:, 0, 0:C + 2], lhsT=onesf_s[0:1, :],
                             rhs=wcorr_s[:], start=False, stop=True)
            nc.vector.tensor_copy(out=wh2loc[:, ic, :], in_=w2p[:, 0, 0:C + 2])
        nc.sync.dma_start(
            out=gin.rearrange("(ic p) c -> p ic c", p=128),
            in_=wh2loc[:])

        gout = dram.tile([N, C + 2], F32)
        if with_collective:
            nc.gpsimd.collective_compute(
                "AllGather", mybir.AluOpType.bypass,
                replica_groups=REPLICA_GROUPS,
                ins=[gin.opt()], outs=[gout.opt()])
        else:  # timing-model variant: fake the exchange with two local copies
            nc.sync.dma_start(out=gout[0:I, :], in_=gin[:])
            nc.sync.dma_start(out=gout[I:N, :], in_=gin[:])

        # g1 row (local queries) -> replicated [128, I]
        g1rowp = psB.tile([128, I], F32, tag="rep")
        for ic in range(IC):
            nc.tensor.transpose(g1rowp[0:1, ic * 128:(ic + 1) * 128],
                                in_=wh2loc[:, ic, 0:1], identity=ident_s[:])
        g1row_s = epL2.tile([1, I], BF16, tag="g1row")
        nc.scalar.activation(g1row_s[:], g1rowp[0:1, :], ACT_COPY)
        g1rp = psB.tile([128, I], F32, tag="rep")
        for hf in range(I // 512):
            sl = slice(hf * 512, (hf + 1) * 512)
            nc.tensor.matmul(g1rp[:, sl], lhsT=ones_s[0:1, :],
                             rhs=g1row_s[0:1, sl])
        l2_modes = route[H * PAIRS_PER_HEAD:]
        g1rep_s = B1L2 = None
        B2L2 = True
        if any(m in (2, 3, 4) for m in l2_modes):
            g1rep_s = singles.tile([128, I], BF16)
            nc.vector.tensor_copy(out=g1rep_s[:], in_=g1rp[:])
        if any(m == 7 for m in l2_modes):
            B1L2 = singles.tile([128, I], BF16)
            nc.scalar.activation(B1L2[:], g1rp[:], ACT_EXP, scale=1.0 - ALPHA)

        # gathered rows: [g1, g2, Wh2(32)] f32 staged, bf16 for the PV lhsT
        wh2tmp = singles.tile([128, JT, C + 2], F32)
        nc.sync.dma_start(
            out=wh2tmp[:],
            in_=gout.rearrange("(jt p) c -> p jt c", p=128))
        wh2gr = singles.tile([128, JT, C + 3], BF16)
        nc.gpsimd.memset(wh2gr[:, :, C + 2:C + 3], 1.0)
        its1 = singles.tile([128, JT, 1], F32)
        its2 = singles.tile([128, JT, 1], F32)
        for jg in range(JT // 4):
            s4 = slice(jg * 4, (jg + 1) * 4)
            nc.gpsimd.tensor_copy(out=wh2gr[:, s4, 0:C + 2],
                                  in_=wh2tmp[:, s4, :])
            nc.scalar.activation(its1[:, s4, :], wh2tmp[:, s4, 1:2], ACT_EXP)
            nc.scalar.activation(its2[:, s4, :], wh2tmp[:, s4, 1:2], ACT_EXP,
                                 scale=ALPHA)

        # ---------------- layer 2 attention ----------------
        # hoist the gather-independent combines (mask + g1rep) so DVE/GPS
        # work while the AllGather is still in flight; reuse the idle q1/q2
        # rings so the main u-ring keeps flowing
        l2_u = {}
        hoist_tags = ["q1", "q1", "q2", "q2"]
        for jp in range(PAIRS_PER_HEAD):
            if l2_modes[jp] not in (3, 4) or not hoist_tags:
                continue
            u = work.tile([128, 2, I], BF16, tag=hoist_tags.pop(0), bufs=QB[0])
            for k in range(2):
                jt = jp * 2 + k
                if l2_modes[jp] == 4:
                    nc.gpsimd.tensor_add(u[:, k, :], mb_s[:, jt, :],
                                         g1rep_s[:])
                else:
                    nc.vector.tensor_tensor(out=u[:, k, :],
                                            in0=mb_s[:, jt, :],
                                            in1=g1rep_s[:], op=ADD)
            l2_u[jp] = u

        o2T = psC.tile([D + 1, I], F32, tag="acc")
        for jp in range(PAIRS_PER_HEAD):
            mode = l2_modes[jp]
            subs = []
            for k in range(2):
                jt = jp * 2 + k
                subs.append((jt, mb_s[:, jt, :],
                             wh2tmp[:, jt, 1:2], its1[:, jt, :],
                             its2[:, jt, :]))
            if jp in l2_u:
                u = l2_u[jp]
                for k, (jt, mb_ap, f2c, e1c, e2c) in enumerate(subs):
                    nc.scalar.activation(u[:, k, :], u[:, k, :], ACT_LRELU,
                                         bias=f2c, alpha=ALPHA)
                p = workp.tile([128, 2, I], BF16, tag="p")
                nc.scalar.activation(p[:], u[:], ACT_EXP)
            else:
                p = _emit_pair(nc, work, workp, mode, subs,
                               (g1rep_s, B1L2, B2L2))
            for k in range(2):
                jt = jp * 2 + k
                for hf in range(I // 512):
                    sl = slice(hf * 512, (hf + 1) * 512)
                    nc.tensor.matmul(o2T[0:C + 1, sl],
                                     lhsT=wh2gr[:, jt, 2:C + 3],
                                     rhs=p[:, k, sl],
                                     start=(jt == 0), stop=(jt == JT - 1))

        # ---------------- finalize (transposed: per-query reciprocal) -----
        if cfg.get("dbg_simple_fin"):
            r2ln = epL2.tile([1, I], F32, tag="lnS2")
            nc.scalar.activation(r2ln[:], o2T[C:C + 1, :], ACT_LN)
            r2 = epL2.tile([1, I], BF16, tag="r2")
            nc.scalar.activation(r2[:], r2ln[:], ACT_EXP, scale=-1.0)
            rbc2p = psB.tile([128, I], F32, tag="rep")
            for hf in range(I // 512):
                sl = slice(hf * 512, (hf + 1) * 512)
                nc.tensor.matmul(rbc2p[0:C, sl], lhsT=ones_s[0:1, 0:C],
                                 rhs=r2[0:1, sl])
            rbc2_s = epL2.tile([C, I], F32, tag="rbc2")
            nc.vector.tensor_copy(out=rbc2_s[:], in_=rbc2p[0:C, :])
            oT_s = epL2.tile([C, I], F32, tag="oT")
            nc.vector.tensor_tensor(out=oT_s[:], in0=o2T[0:C, :],
                                    in1=rbc2_s[:], op=MULT)
            for k in range(IC):
                ofp = psA.tile([128, 4, D + 2], F32, tag="ph")
                nc.tensor.transpose(ofp[:, 0, 0:C],
                                    in_=oT_s[:, k * 128:(k + 1) * 128],
                                    identity=ident_s[0:C, 0:C])
                ofs = ep2.tile([128, C], F32, tag="ofs")
                nc.vector.tensor_copy(out=ofs[:], in_=ofp[:, 0, 0:C])
                nc.sync.dma_start(out=outp_d.ap()[k * 128:(k + 1) * 128, :],
                                  in_=ofs[:])
        else:
            o2s = epL2.tile([C, I], F32, tag="o2s")
            nc.vector.tensor_copy(out=o2s[:], in_=o2T[0:C, :])
            o2r = epL2.tile([1, I], F32, tag="o2r")
            nc.scalar.activation(o2r[:], o2T[C:C + 1, :], ACT_COPY)
            for k in range(IC):
                ck = slice(k * 128, (k + 1) * 128)
                ofp = psA.tile([128, 4, D + 2], F32, tag="ph")
                nc.tensor.transpose(ofp[:, 0, 0:C], in_=o2s[:, ck],
                                    identity=ident_s[0:C, 0:C])
                ofq = psA.tile([128, 4, D + 2], F32, tag="ph")
                nc.tensor.transpose(ofq[:, 0, 0:1], in_=o2r[:, ck],
                                    identity=ident_s[0:1, 0:1])
                s2t = ep2.tile([128, 1], F32, tag="s2t")
                nc.vector.reciprocal(s2t[:], ofq[:, 0, 0:1])
                ofs = ep2.tile([128, C], F32, tag="ofs")
                nc.vector.tensor_scalar_mul(ofs[:], ofp[:, 0, 0:C], s2t[:])
                nc.sync.dma_start(out=outp_d.ap()[k * 128:(k + 1) * 128, :],
                                  in_=ofs[:])


# --------------------------------------------------------------------------
# host side
# --------------------------------------------------------------------------

def shard_inputs(x, adj, W, a1, a2, Wo, ao1, ao2):
    x = np.asarray(x, np.float32)
    adj = np.asarray(adj)
    W = np.asarray(W, np.float32)
    a1 = np.asarray(a1, np.float32)
    a2 = np.asarray(a2, np.float32)
    Wo = np.asarray(Wo, np.float32)
    ao1 = np.asarray(ao1, np.float32)
    ao2 = np.asarray(ao2, np.float32)
    BF = ml_dtypes.bfloat16

    wvec1 = np.einsum("hfd,hd->hf", W, a1)          # [H, F]
    wvec2 = np.einsum("hfd,hd->hf", W, a2)
    wext = np.concatenate([W, wvec1[:, :, None], wvec2[:, :, None]],
                          axis=2).astype(BF)
    a1rep = np.repeat(wvec1[:, :, None], 128, axis=2).astype(BF)
    wo1 = Wo @ ao1                                   # [512]
    wo2 = Wo @ ao2
    woflat = np.concatenate([wo1[:, None], wo2[:, None], Wo], 1)  # [512, 34]
    woext = woflat.reshape(KT, 128, C + 2).astype(np.float32)
    wcorr = (-woflat.sum(0))[None, :].astype(np.float32)
    ident = np.eye(128, dtype=np.float32)

    in_maps = []
    for c in range(N_CORES):
        b, half = c // 2, c % 2
        i0 = half * I
        xt = np.ascontiguousarray(x[b].T).astype(BF)   # [F, N]
        xtl = np.ascontiguousarray(xt[:, i0:i0 + I])
        adjt = adj[b, i0:i0 + I, :].T                # [N, I] = (j, i)
        mb = np.where(adjt > 0, np.float32(0.0), np.float32(-BIG))
        mb = np.ascontiguousarray(mb.reshape(JT, 128, I)).astype(BF)
        in_maps.append({
            "xt": xt, "xtl": xtl, "mb": mb, "wext": wext,
            "a1rep": a1rep, "woext": woext, "wcorr": wcorr, "ident": ident,
        })
    return in_maps


# Engine routing chosen by cost-model sweep (TimelineSim).
DEFAULT_CFG = {"p4": 44, "p7": 27, "ep_defer": 2, "f1rep_eng": "dve",
               "rbc_eng": "dve", "prep_at": 3,
               "l2route": [4, 4, 7, 7, 4, 4, 2, 2]}

_CACHE = {}


def _program():
    if "nc" not in _CACHE:
        _CACHE["nc"] = build_program(with_collective=True, cfg=DEFAULT_CFG)
    return _CACHE["nc"]


def kernel(**inputs):
    nc = _program()
    in_maps = shard_inputs(**inputs)
    res = run_bass_kernel_spmd(nc, in_maps, list(range(N_CORES)))
    _CACHE["last_results"] = res
    out = np.empty((B, N, C), np.float32)
    for c in range(N_CORES):
        b, half = c // 2, c % 2
        out[b, half * I:(half + 1) * I, :] = res.results[c]["outp"]
    return out
